# revision 1
# baseline (speedup 1.0000x reference)
"""CharDecoder LSTM kernel for 8 Trainium2 NeuronCores.

Data-parallel over the flattened (B*W)=8192 batch axis: each of the 8 cores
processes 1024 sequences. Small LSTM/projection weights are replicated; the
hidden/cell state stays resident in SBUF (in transposed [feature, batch]
layout) across all 20 decode steps.

Per-core layout ("layout B", feature-on-partitions):
  hT/cT: [512, 1024] as 4 SBUF tiles [128, 1024] (one per 128-feature block)
  gatesT: [2048, 1024] computed as 16 (m) x 2 (n) PSUM tiles [128, 512]
  xT (one-hot next-char): [128 vocab, 1024] as 2 half tiles [128, 512]
  logitsT: [128 vocab, 1024]
All matmuls contract over the partition dim (K) in native fp32 (4 cyc/row)
for exact-fp32 numerics -- the argmax feedback makes the computation
chaotically sensitive, so reduced-precision matmul formats are unusable.

argmax -> one-hot per step: PE-transpose logits into [batch, vocab] blocks,
DVE reduce_max along free dim, is_ge compare against the per-row max, and
PE-transpose the resulting 0/1 mask back into [vocab, batch] for use as the
next step's matmul operand.
"""

import numpy as np
from contextlib import ExitStack

import concourse.bass as bass
import concourse.bacc as bacc
import concourse.mybir as mybir
import concourse.tile as tile
from concourse.bass_utils import run_bass_kernel_spmd

B, W, Q, H, A = 64, 128, 256, 512, 128
C = 20
NCORES = 8
BW = B * W
N_FULL = BW // NCORES  # 1024 batch rows per core

F32 = mybir.dt.float32
AF = mybir.ActivationFunctionType
ALU = mybir.AluOpType
AX = mybir.AxisListType

KH = H // 128       # 4 hidden k-tiles
KQ = Q // 128       # 2 input k-tiles
MG = 4 * H // 128   # 16 gate m-tiles

LAST_RESULTS = None  # BassKernelResults of the most recent run (for test.py)


def build_program(c_steps: int = C, n: int = N_FULL) -> bass.Bass:
    assert n % 512 == 0
    nt = n // 512    # PSUM n-tiles per row block
    nb = n // 128    # 128-wide argmax blocks

    nc = bacc.Bacc("TRN2", target_bir_lowering=False, debug=False)

    qrT = nc.dram_tensor("qrT", [Q, n], F32, kind="ExternalInput").ap()
    winT = nc.dram_tensor("winT", [Q, H], F32, kind="ExternalInput").ap()
    whhT = nc.dram_tensor("whhT", [H, 4 * H], F32, kind="ExternalInput").ap()
    wihT = nc.dram_tensor("wihT", [A, 4 * H], F32, kind="ExternalInput").ap()
    woutT = nc.dram_tensor("woutT", [H, A], F32, kind="ExternalInput").ap()
    b_in_c = nc.dram_tensor("b_in_c", [128, KH], F32, kind="ExternalInput").ap()
    bias_c = nc.dram_tensor("bias_c", [128, MG], F32, kind="ExternalInput").ap()
    bias0_c = nc.dram_tensor("bias0_c", [128, MG], F32, kind="ExternalInput").ap()
    b_out_c = nc.dram_tensor("b_out_c", [128, 1], F32, kind="ExternalInput").ap()
    ident = nc.dram_tensor("ident", [128, 128], F32, kind="ExternalInput").ap()
    out = nc.dram_tensor("out_logits", [c_steps, A, n], F32, kind="ExternalOutput").ap()

    with tile.TileContext(nc) as tc, ExitStack() as ctx:
        wp = ctx.enter_context(tc.tile_pool(name="weights", bufs=1))
        sp = ctx.enter_context(tc.tile_pool(name="state", bufs=2))
        gp = ctx.enter_context(tc.tile_pool(name="gates", bufs=2))
        tp = ctx.enter_context(tc.tile_pool(name="tmp", bufs=2))
        lp = ctx.enter_context(tc.tile_pool(name="logits", bufs=2))
        mp = ctx.enter_context(tc.tile_pool(name="small", bufs=4))
        pg = ctx.enter_context(tc.tile_pool(name="pgate", bufs=3, space="PSUM"))
        pl = ctx.enter_context(tc.tile_pool(name="plog", bufs=2, space="PSUM"))
        pt = ctx.enter_context(tc.tile_pool(name="ptr", bufs=2, space="PSUM"))

        # --- load replicated weights / biases ---
        w_hh_t = []
        for k in range(KH):
            t = wp.tile([128, 4 * H], F32, tag=f"whh{k}", name=f"whh{k}")
            nc.sync.dma_start(t[:], whhT[k * 128:(k + 1) * 128, :])
            w_hh_t.append(t)
        w_ih_t = wp.tile([128, 4 * H], F32, tag="wih")
        nc.sync.dma_start(w_ih_t[:], wihT[:, :])
        w_out_t = wp.tile([128, KH * A], F32, tag="wout")
        for k in range(KH):
            nc.sync.dma_start(w_out_t[:, k * A:(k + 1) * A], woutT[k * 128:(k + 1) * 128, :])
        w_in_t = []
        for k in range(KQ):
            t = wp.tile([128, H], F32, tag=f"win{k}", name=f"win{k}")
            nc.sync.dma_start(t[:], winT[k * 128:(k + 1) * 128, :])
            w_in_t.append(t)
        bias_t = wp.tile([128, MG], F32, tag="bias")
        nc.sync.dma_start(bias_t[:], bias_c[:, :])
        bias0_t = wp.tile([128, MG], F32, tag="bias0")
        nc.sync.dma_start(bias0_t[:], bias0_c[:, :])
        b_in_t = wp.tile([128, KH], F32, tag="b_in")
        nc.sync.dma_start(b_in_t[:], b_in_c[:, :])
        b_out_t = wp.tile([128, 1], F32, tag="b_out")
        nc.sync.dma_start(b_out_t[:], b_out_c[:, :])
        id_t = wp.tile([128, 128], F32, tag="ident")
        nc.sync.dma_start(id_t[:], ident[:, :])
        qr_t = []
        for k in range(KQ):
            t = wp.tile([128, n], F32, tag=f"qr{k}", name=f"qr{k}")
            nc.sync.dma_start(t[:], qrT[k * 128:(k + 1) * 128, :])
            qr_t.append(t)

        # --- initial state: h0 = w_in @ qr^T + b_in ; c0 = 0 ---
        ht = [sp.tile([128, n], F32, tag=f"h{j}", name=f"h{j}") for j in range(KH)]
        ct = [sp.tile([128, n], F32, tag=f"c{j}", name=f"c{j}") for j in range(KH)]
        for j in range(KH):
            nc.vector.memset(ct[j][:], 0.0)
            for n0 in range(nt):
                ps = pg.tile([128, 512], F32, tag="pg")
                for k in range(KQ):
                    nc.tensor.matmul(
                        ps[:],
                        lhsT=w_in_t[k][:, j * 128:(j + 1) * 128],
                        rhs=qr_t[k][:, n0 * 512:(n0 + 1) * 512],
                        start=(k == 0), stop=(k == KQ - 1),
                    )
                nc.scalar.activation(
                    ht[j][:, n0 * 512:(n0 + 1) * 512], ps[:],
                    AF.Identity, bias=b_in_t[:, j:j + 1],
                )

        # collapse all setup deps so steady-state instructions carry few waits
        tc.strict_bb_all_engine_barrier()

        xt = None  # one-hot input halves, [vocab, batch]; step 0 folds it into bias0

        for t in range(c_steps):
            ht_next = [sp.tile([128, n], F32, tag=f"h{j}", name=f"h{j}") for j in range(KH)]
            ct_next = [sp.tile([128, n], F32, tag=f"c{j}", name=f"c{j}") for j in range(KH)]
            for j in range(KH):
                g_j = [gp.tile([128, n], F32, tag=f"g{q}", name=f"g{q}") for q in range(4)]
                for q in range(4):  # i, f, g, o
                    m = q * KH + j
                    for n0 in range(nt):
                        ps = pg.tile([128, 512], F32, tag="pg")
                        for k in range(KH):
                            nc.tensor.matmul(
                                ps[:],
                                lhsT=w_hh_t[k][:, m * 128:(m + 1) * 128],
                                rhs=ht[k][:, n0 * 512:(n0 + 1) * 512],
                                start=(k == 0), stop=(t == 0 and k == KH - 1),
                            )
                        if t > 0:
                            nc.tensor.matmul(
                                ps[:],
                                lhsT=w_ih_t[:, m * 128:(m + 1) * 128],
                                rhs=xt[n0][:],
                                start=False, stop=True,
                            )
                        bias_ap = (bias0_t if t == 0 else bias_t)[:, m:m + 1]
                        func = AF.Tanh if q == 2 else AF.Sigmoid
                        nc.scalar.activation(
                            g_j[q][:, n0 * 512:(n0 + 1) * 512],
                            ps[:], func, bias=bias_ap,
                        )
                # cell/hidden update block j: c = f*c + i*g ; h = o*tanh(c)
                gi, gf, gg, go = (g[:] for g in g_j)
                cs = ct_next[j][:]
                hs = ht_next[j][:]
                t1 = tp.tile([128, n], F32, tag="t1")
                nc.vector.tensor_mul(t1[:], gi, gg)
                nc.vector.tensor_mul(cs, gf, ct[j][:])
                nc.vector.tensor_add(cs, cs, t1[:])
                t2 = tp.tile([128, n], F32, tag="t2")
                nc.scalar.activation(t2[:], cs, AF.Tanh)
                nc.vector.tensor_mul(hs, go, t2[:])

            # logits = w_out @ h + b_out  (in [vocab, batch] layout)
            lg = lp.tile([128, n], F32, tag="logits")
            for n0 in range(nt):
                ps = pl.tile([128, 512], F32, tag="pl")
                for k in range(KH):
                    nc.tensor.matmul(
                        ps[:],
                        lhsT=w_out_t[:, k * A:(k + 1) * A],
                        rhs=ht_next[k][:, n0 * 512:(n0 + 1) * 512],
                        start=(k == 0), stop=(k == KH - 1),
                    )
                nc.scalar.activation(
                    lg[:, n0 * 512:(n0 + 1) * 512], ps[:],
                    AF.Identity, bias=b_out_t[:, 0:1],
                )
                nc.sync.dma_start(
                    out[t, :, n0 * 512:(n0 + 1) * 512],
                    lg[:, n0 * 512:(n0 + 1) * 512],
                )

            # next input: one-hot(argmax(logits)) back in [vocab, batch] layout
            if t < c_steps - 1:
                xt = [sp.tile([128, 512], F32, tag=f"x{h}", name=f"x{h}") for h in range(nt)]
                for b in range(nb):
                    ptile = pt.tile([128, 128], F32, tag="pt")
                    nc.tensor.transpose(ptile[:], lg[:, b * 128:(b + 1) * 128], id_t[:])
                    mx = mp.tile([128, 1], F32, tag="mx")
                    nc.vector.reduce_max(mx[:], ptile[:], axis=AX.X)
                    mb = mp.tile([128, 128], F32, tag="mb")
                    nc.vector.tensor_scalar(mb[:], ptile[:], mx[:, 0:1], None, ALU.is_ge)
                    pback = pt.tile([128, 128], F32, tag="pt")
                    nc.tensor.transpose(pback[:], mb[:], id_t[:])
                    nc.vector.tensor_copy(
                        xt[b // 4][:, (b % 4) * 128:(b % 4 + 1) * 128], pback[:])

            ht, ct = ht_next, ct_next

    nc.compile()
    return nc


def make_in_maps(inputs: dict, c_steps: int = C, n: int = N_FULL, ncores: int = NCORES):
    f32 = np.float32
    qr = np.ascontiguousarray(np.asarray(inputs["quantized_repr"], f32)).reshape(BW, Q)
    w_in = np.asarray(inputs["w_in"], f32)
    b_in = np.asarray(inputs["b_in"], f32)
    w_ih = np.asarray(inputs["w_ih"], f32)
    w_hh = np.asarray(inputs["w_hh"], f32)
    b_ih = np.asarray(inputs["b_ih"], f32)
    b_hh = np.asarray(inputs["b_hh"], f32)
    w_out = np.asarray(inputs["w_out"], f32)
    b_out = np.asarray(inputs["b_out"], f32)

    bias = b_ih + b_hh                    # fp32, same as reference
    bias0 = bias + w_ih[:, 0]             # step-0 one-hot(0) contribution folded in

    shared = {
        "winT": np.ascontiguousarray(w_in.T),
        "whhT": np.ascontiguousarray(w_hh.T),
        "wihT": np.ascontiguousarray(w_ih.T),
        "woutT": np.ascontiguousarray(w_out.T),
        "b_in_c": np.ascontiguousarray(b_in.reshape(KH, 128).T),
        "bias_c": np.ascontiguousarray(bias.reshape(MG, 128).T),
        "bias0_c": np.ascontiguousarray(bias0.reshape(MG, 128).T),
        "b_out_c": np.ascontiguousarray(b_out.reshape(128, 1)),
        "ident": np.eye(128, dtype=f32),
    }
    in_maps = []
    for i in range(ncores):
        m = dict(shared)
        m["qrT"] = np.ascontiguousarray(qr[i * n:(i + 1) * n].T)
        in_maps.append(m)
    return in_maps


def kernel(**inputs) -> np.ndarray:
    global LAST_RESULTS
    assert int(inputs["max_char_len"]) == C
    nc = build_program(C, N_FULL)
    in_maps = make_in_maps(inputs, C, N_FULL, NCORES)
    res = run_bass_kernel_spmd(nc, in_maps, core_ids=list(range(NCORES)))
    LAST_RESULTS = res
    # per-core [C, A, N] -> [N, C, A]; concat cores -> [BW, C, A] -> [B, W, C, A]
    parts = [np.transpose(r["out_logits"], (2, 0, 1)) for r in res.results]
    full = np.concatenate(parts, axis=0).reshape(B, W, C, A)
    return np.ascontiguousarray(full)



# revision 4
# speedup vs baseline: 1.3844x; 1.3844x over previous
"""CharDecoder LSTM kernel for 8 Trainium2 NeuronCores.

Data-parallel over the flattened (B*W)=8192 batch axis: each of the 8 cores
processes 1024 sequences. Small LSTM/projection weights are replicated; the
hidden/cell state stays resident in SBUF (in transposed [feature, batch]
layout) across all 20 decode steps.

Numerics: all matmuls run in float32r (TF32-like: 8-bit exponent, 11-bit
stored mantissa, 1 cycle/row for free dim >= 256) using an exact Dekker
hi/lo split of both operands:
    w = w1 + w2 (exact, host-side), h = h1 + h2 (exact, 2 DVE ops/tile)
    w @ h ~= w1@h1 + w2@h1 + w1@h2     (dropped w2@h2 term ~ 2^-24)
which is fp32-quality (verified ~1e-6 max rel err end-to-end, 0 argmax
flips) at 3 cycles/row instead of fp32's 4 cycles/row on the PE.

Per-core layout ("layout B", feature-on-partitions):
  h1/h2 splits: [512, 1024] as 4+4 SBUF f32r tiles [128, 1024]
  cT: [512, 1024] as 4 fp32 tiles, updated in place
  gatesT: [2048, 1024] computed as 16 (m) x 2 (n) PSUM tiles [128, 512]
  xT (one-hot next-char): [128 vocab, 1024] f32r, 2 half tiles [128, 512]

argmax -> one-hot per step: PE-transpose logits (fp32, 2 cyc/row) into
[batch, vocab] blocks, DVE reduce_max along free dim, is_ge against the
per-row max emitting an exact 0/1 f32r mask, and PE-transpose the mask back
(f32r, 1.5 cyc/row) into [vocab, batch] for the next step's x matmul.
"""

import numpy as np
from contextlib import ExitStack

import concourse.bass as bass
import concourse.bacc as bacc
import concourse.mybir as mybir
import concourse.tile as tile
from concourse.bass_utils import run_bass_kernel_spmd

B, W, Q, H, A = 64, 128, 256, 512, 128
C = 20
NCORES = 8
BW = B * W
N_FULL = BW // NCORES  # 1024 batch rows per core

F32 = mybir.dt.float32
F32R = mybir.dt.float32r
AF = mybir.ActivationFunctionType
ALU = mybir.AluOpType
AX = mybir.AxisListType

KH = H // 128       # 4 hidden k-tiles
KQ = Q // 128       # 2 input k-tiles
MG = 4 * H // 128   # 16 gate m-tiles

LAST_RESULTS = None  # BassKernelResults of the most recent run (for test.py)


def split12(v):
    """Exact Dekker split of fp32 into two 12-bit-significand (f32r) parts."""
    v = np.ascontiguousarray(np.asarray(v, np.float32))
    u = v.view(np.uint32)
    r = u + (np.uint32(0x7FF) + ((u >> np.uint32(12)) & np.uint32(1)))
    hi = (r & np.uint32(0xFFFFF000)).view(np.float32)
    lo = (v - hi).astype(np.float32)
    return hi, lo


def build_program(c_steps: int = C, n: int = N_FULL) -> bass.Bass:
    assert n % 512 == 0
    nt = n // 512    # PSUM n-tiles per row block
    nb = n // 128    # 128-wide argmax blocks

    nc = bacc.Bacc("TRN2", target_bir_lowering=False, debug=False)

    qrT = nc.dram_tensor("qrT", [Q, n], F32, kind="ExternalInput").ap()
    winT = nc.dram_tensor("winT", [Q, H], F32, kind="ExternalInput").ap()
    whhT1 = nc.dram_tensor("whhT1", [H, 4 * H], F32R, kind="ExternalInput").ap()
    whhT2 = nc.dram_tensor("whhT2", [H, 4 * H], F32R, kind="ExternalInput").ap()
    wihT1 = nc.dram_tensor("wihT1", [A, 4 * H], F32R, kind="ExternalInput").ap()
    wihT2 = nc.dram_tensor("wihT2", [A, 4 * H], F32R, kind="ExternalInput").ap()
    woutT1 = nc.dram_tensor("woutT1", [H, A], F32R, kind="ExternalInput").ap()
    woutT2 = nc.dram_tensor("woutT2", [H, A], F32R, kind="ExternalInput").ap()
    b_in_c = nc.dram_tensor("b_in_c", [128, KH], F32, kind="ExternalInput").ap()
    bias_c = nc.dram_tensor("bias_c", [128, MG], F32, kind="ExternalInput").ap()
    bias0_c = nc.dram_tensor("bias0_c", [128, MG], F32, kind="ExternalInput").ap()
    b_out_c = nc.dram_tensor("b_out_c", [128, 1], F32, kind="ExternalInput").ap()
    ident = nc.dram_tensor("ident", [128, 128], F32, kind="ExternalInput").ap()
    identr = nc.dram_tensor("identr", [128, 128], F32R, kind="ExternalInput").ap()
    out = nc.dram_tensor("out_logits", [c_steps, A, n], F32, kind="ExternalOutput").ap()

    with tile.TileContext(nc) as tc, ExitStack() as ctx:
        wp = ctx.enter_context(tc.tile_pool(name="weights", bufs=1))
        sp = ctx.enter_context(tc.tile_pool(name="state", bufs=2))
        cp = ctx.enter_context(tc.tile_pool(name="cell", bufs=1))
        gp = ctx.enter_context(tc.tile_pool(name="gates", bufs=1))
        tp = ctx.enter_context(tc.tile_pool(name="tmp", bufs=1))
        lp = ctx.enter_context(tc.tile_pool(name="logits", bufs=1))
        mp = ctx.enter_context(tc.tile_pool(name="small", bufs=2))
        pg = ctx.enter_context(tc.tile_pool(name="pgate", bufs=4, space="PSUM"))
        pl = ctx.enter_context(tc.tile_pool(name="plog", bufs=2, space="PSUM"))
        pt = ctx.enter_context(tc.tile_pool(name="ptr", bufs=1, space="PSUM"))

        # --- load replicated weights / biases (hi/lo f32r splits) ---
        whh1_t, whh2_t = [], []
        for k in range(KH):
            t1 = wp.tile([128, 4 * H], F32R, tag=f"whh1{k}", name=f"whh1{k}")
            nc.sync.dma_start(t1[:], whhT1[k * 128:(k + 1) * 128, :])
            whh1_t.append(t1)
            t2 = wp.tile([128, 4 * H], F32R, tag=f"whh2{k}", name=f"whh2{k}")
            nc.sync.dma_start(t2[:], whhT2[k * 128:(k + 1) * 128, :])
            whh2_t.append(t2)
        wih1_t = wp.tile([128, 4 * H], F32R, tag="wih1")
        nc.sync.dma_start(wih1_t[:], wihT1[:, :])
        wih2_t = wp.tile([128, 4 * H], F32R, tag="wih2")
        nc.sync.dma_start(wih2_t[:], wihT2[:, :])
        wout1_t = wp.tile([128, KH * A], F32R, tag="wout1")
        wout2_t = wp.tile([128, KH * A], F32R, tag="wout2")
        for k in range(KH):
            nc.sync.dma_start(wout1_t[:, k * A:(k + 1) * A], woutT1[k * 128:(k + 1) * 128, :])
            nc.sync.dma_start(wout2_t[:, k * A:(k + 1) * A], woutT2[k * 128:(k + 1) * 128, :])
        bias_t = wp.tile([128, MG], F32, tag="bias")
        nc.sync.dma_start(bias_t[:], bias_c[:, :])
        bias0_t = wp.tile([128, MG], F32, tag="bias0")
        nc.sync.dma_start(bias0_t[:], bias0_c[:, :])
        b_in_t = wp.tile([128, KH], F32, tag="b_in")
        nc.sync.dma_start(b_in_t[:], b_in_c[:, :])
        b_out_t = wp.tile([128, 1], F32, tag="b_out")
        nc.sync.dma_start(b_out_t[:], b_out_c[:, :])
        id_t = wp.tile([128, 128], F32, tag="ident")
        nc.sync.dma_start(id_t[:], ident[:, :])
        idr_t = wp.tile([128, 128], F32R, tag="identr")
        nc.sync.dma_start(idr_t[:], identr[:, :])

        # --- initial state: h = w_in @ qr^T + b_in (plain fp32 matmul,
        #     one-time), split into f32r hi/lo; c = 0 ---
        h1 = [sp.tile([128, n], F32R, tag=f"h1{j}", name=f"h1{j}") for j in range(KH)]
        h2 = [sp.tile([128, n], F32R, tag=f"h2{j}", name=f"h2{j}") for j in range(KH)]
        ct = [cp.tile([128, n], F32, tag=f"c{j}", name=f"c{j}") for j in range(KH)]
        # setup reuses steady-state tiles: qr k-tiles live in the g0/g1 gate
        # buffers, w_in^T packs both k-tiles into the logits buffer
        win_t = lp.tile([128, n], F32, tag="logits", name="win_all")
        for k in range(KQ):
            nc.sync.dma_start(win_t[:, k * H:k * H + H], winT[k * 128:(k + 1) * 128, :])
        qr_t = []
        for k in range(KQ):
            t = gp.tile([128, n], F32, tag=f"g{k}", name=f"qr{k}")
            nc.sync.dma_start(t[:], qrT[k * 128:(k + 1) * 128, :])
            qr_t.append(t)
        for j in range(KH):
            nc.vector.memset(ct[j][:], 0.0)
            hf = tp.tile([128, n], F32, tag="t1")
            for n0 in range(nt):
                ps = pg.tile([128, 512], F32, tag="pg")
                for k in range(KQ):
                    nc.tensor.matmul(
                        ps[:],
                        lhsT=win_t[:, k * H + j * 128:k * H + (j + 1) * 128],
                        rhs=qr_t[k][:, n0 * 512:(n0 + 1) * 512],
                        start=(k == 0), stop=(k == KQ - 1),
                    )
                nc.scalar.activation(
                    hf[:, n0 * 512:(n0 + 1) * 512], ps[:],
                    AF.Identity, bias=b_in_t[:, j:j + 1],
                )
            nc.vector.tensor_copy(h1[j][:], hf[:])
            nc.vector.tensor_sub(h2[j][:], hf[:], h1[j][:].bitcast(F32))

        # collapse all setup deps so steady-state instructions carry few waits
        tc.strict_bb_all_engine_barrier()

        xt = None  # one-hot input halves [vocab, batch] f32r; step 0 folds into bias0

        for t in range(c_steps):
            h1n = [sp.tile([128, n], F32R, tag=f"h1{j}", name=f"h1{j}") for j in range(KH)]
            h2n = [sp.tile([128, n], F32R, tag=f"h2{j}", name=f"h2{j}") for j in range(KH)]
            for j in range(KH):
                g_j = [gp.tile([128, n], F32, tag=f"g{q}", name=f"g{q}") for q in range(4)]
                for q in range(4):  # i, f, g, o
                    m = q * KH + j
                    for n0 in range(nt):
                        ps = pg.tile([128, 512], F32, tag="pg")
                        for k in range(KH):
                            nc.tensor.matmul(
                                ps[:],
                                lhsT=whh1_t[k][:, m * 128:(m + 1) * 128],
                                rhs=h1[k][:, n0 * 512:(n0 + 1) * 512],
                                start=(k == 0), stop=False,
                            )
                        for k in range(KH):
                            nc.tensor.matmul(
                                ps[:],
                                lhsT=whh2_t[k][:, m * 128:(m + 1) * 128],
                                rhs=h1[k][:, n0 * 512:(n0 + 1) * 512],
                                start=False, stop=False,
                            )
                        for k in range(KH):
                            nc.tensor.matmul(
                                ps[:],
                                lhsT=whh1_t[k][:, m * 128:(m + 1) * 128],
                                rhs=h2[k][:, n0 * 512:(n0 + 1) * 512],
                                start=False, stop=(t == 0 and k == KH - 1),
                            )
                        if t > 0:
                            nc.tensor.matmul(
                                ps[:],
                                lhsT=wih1_t[:, m * 128:(m + 1) * 128],
                                rhs=xt[n0][:],
                                start=False, stop=False,
                            )
                            nc.tensor.matmul(
                                ps[:],
                                lhsT=wih2_t[:, m * 128:(m + 1) * 128],
                                rhs=xt[n0][:],
                                start=False, stop=True,
                            )
                        bias_ap = (bias0_t if t == 0 else bias_t)[:, m:m + 1]
                        func = AF.Tanh if q == 2 else AF.Sigmoid
                        nc.scalar.activation(
                            g_j[q][:, n0 * 512:(n0 + 1) * 512],
                            ps[:], func, bias=bias_ap,
                        )
                # cell/hidden update block j: c = f*c + i*g ; h = o*tanh(c)
                gi, gf, gg, go = (g[:] for g in g_j)
                cs = ct[j][:]
                t1 = tp.tile([128, n], F32, tag="t1")
                nc.vector.tensor_mul(t1[:], gi, gg)
                nc.vector.tensor_mul(cs, gf, cs)
                nc.vector.tensor_add(cs, cs, t1[:])
                t2 = tp.tile([128, n], F32, tag="t2")
                nc.scalar.activation(t2[:], cs, AF.Tanh)
                hf = tp.tile([128, n], F32, tag="t1")
                nc.vector.tensor_mul(hf[:], go, t2[:])
                nc.vector.tensor_copy(h1n[j][:], hf[:])
                nc.vector.tensor_sub(h2n[j][:], hf[:], h1n[j][:].bitcast(F32))

            # logits = w_out @ h + b_out  (in [vocab, batch] layout)
            lg = lp.tile([128, n], F32, tag="logits")
            for n0 in range(nt):
                ps = pl.tile([128, 512], F32, tag="pl")
                for k in range(KH):
                    nc.tensor.matmul(
                        ps[:],
                        lhsT=wout1_t[:, k * A:(k + 1) * A],
                        rhs=h1n[k][:, n0 * 512:(n0 + 1) * 512],
                        start=(k == 0), stop=False,
                    )
                for k in range(KH):
                    nc.tensor.matmul(
                        ps[:],
                        lhsT=wout2_t[:, k * A:(k + 1) * A],
                        rhs=h1n[k][:, n0 * 512:(n0 + 1) * 512],
                        start=False, stop=False,
                    )
                for k in range(KH):
                    nc.tensor.matmul(
                        ps[:],
                        lhsT=wout1_t[:, k * A:(k + 1) * A],
                        rhs=h2n[k][:, n0 * 512:(n0 + 1) * 512],
                        start=False, stop=(k == KH - 1),
                    )
                nc.scalar.activation(
                    lg[:, n0 * 512:(n0 + 1) * 512], ps[:],
                    AF.Identity, bias=b_out_t[:, 0:1],
                )
                nc.sync.dma_start(
                    out[t, :, n0 * 512:(n0 + 1) * 512],
                    lg[:, n0 * 512:(n0 + 1) * 512],
                )

            # next input: one-hot(argmax(logits)) back in [vocab, batch] f32r
            if t < c_steps - 1:
                xt = [sp.tile([128, 512], F32R, tag=f"x{h}", name=f"x{h}") for h in range(nt)]
                for b in range(nb):
                    ptile = pt.tile([128, 128], F32, tag="ptf")
                    nc.tensor.transpose(ptile[:], lg[:, b * 128:(b + 1) * 128], id_t[:])
                    mx = mp.tile([128, 1], F32, tag="mx")
                    nc.vector.reduce_max(mx[:], ptile[:], axis=AX.X)
                    mb = mp.tile([128, 128], F32R, tag="mb")
                    nc.vector.tensor_scalar(mb[:], ptile[:], mx[:, 0:1], None, ALU.is_ge)
                    pback = pt.tile([128, 128], F32R, tag="ptr")
                    nc.tensor.transpose(pback[:], mb[:], idr_t[:])
                    nc.vector.tensor_copy(
                        xt[b // 4][:, (b % 4) * 128:(b % 4 + 1) * 128],
                        pback[:].bitcast(F32))

            h1, h2 = h1n, h2n

    nc.compile()
    return nc


def make_in_maps(inputs: dict, c_steps: int = C, n: int = N_FULL, ncores: int = NCORES):
    f32 = np.float32
    qr = np.ascontiguousarray(np.asarray(inputs["quantized_repr"], f32)).reshape(BW, Q)
    w_in = np.asarray(inputs["w_in"], f32)
    b_in = np.asarray(inputs["b_in"], f32)
    w_ih = np.asarray(inputs["w_ih"], f32)
    w_hh = np.asarray(inputs["w_hh"], f32)
    b_ih = np.asarray(inputs["b_ih"], f32)
    b_hh = np.asarray(inputs["b_hh"], f32)
    w_out = np.asarray(inputs["w_out"], f32)
    b_out = np.asarray(inputs["b_out"], f32)

    bias = b_ih + b_hh                    # fp32, same as reference
    bias0 = bias + w_ih[:, 0]             # step-0 one-hot(0) contribution folded in

    whh1, whh2 = split12(w_hh.T)
    wih1, wih2 = split12(w_ih.T)
    wout1, wout2 = split12(w_out.T)

    shared = {
        "winT": np.ascontiguousarray(w_in.T),
        "whhT1": whh1, "whhT2": whh2,
        "wihT1": wih1, "wihT2": wih2,
        "woutT1": wout1, "woutT2": wout2,
        "b_in_c": np.ascontiguousarray(b_in.reshape(KH, 128).T),
        "bias_c": np.ascontiguousarray(bias.reshape(MG, 128).T),
        "bias0_c": np.ascontiguousarray(bias0.reshape(MG, 128).T),
        "b_out_c": np.ascontiguousarray(b_out.reshape(128, 1)),
        "ident": np.eye(128, dtype=f32),
        "identr": np.eye(128, dtype=f32),
    }
    in_maps = []
    for i in range(ncores):
        m = dict(shared)
        m["qrT"] = np.ascontiguousarray(qr[i * n:(i + 1) * n].T)
        in_maps.append(m)
    return in_maps


def kernel(**inputs) -> np.ndarray:
    global LAST_RESULTS
    assert int(inputs["max_char_len"]) == C
    nc = build_program(C, N_FULL)
    in_maps = make_in_maps(inputs, C, N_FULL, NCORES)
    res = run_bass_kernel_spmd(nc, in_maps, core_ids=list(range(NCORES)))
    LAST_RESULTS = res
    # per-core [C, A, N] -> [N, C, A]; concat cores -> [BW, C, A] -> [B, W, C, A]
    parts = [np.transpose(r["out_logits"], (2, 0, 1)) for r in res.results]
    full = np.concatenate(parts, axis=0).reshape(B, W, C, A)
    return np.ascontiguousarray(full)


# revision 7
# speedup vs baseline: 1.3963x; 1.0086x over previous
"""CharDecoder LSTM kernel for 8 Trainium2 NeuronCores.

Data-parallel over the flattened (B*W)=8192 batch axis: each of the 8 cores
processes 1024 sequences. Small LSTM/projection weights are replicated; the
hidden/cell state stays resident in SBUF (in transposed [feature, batch]
layout) across all 20 decode steps.

Numerics: all matmuls run in float32r (TF32-like: 8-bit exponent, 11-bit
stored mantissa, 1 cycle/row for free dim >= 256) using an exact Dekker
hi/lo split of both operands:
    w = w1 + w2 (exact, host-side), h = h1 + h2 (exact, 2 DVE ops/tile)
    w @ h ~= w1@h1 + w2@h1 + w1@h2     (dropped w2@h2 term ~ 2^-24)
which is fp32-quality (verified ~1e-6 max rel err end-to-end, 0 argmax
flips) at 3 cycles/row instead of fp32's 4 cycles/row on the PE.

Per-core layout ("layout B", feature-on-partitions):
  h1/h2 splits: [512, 1024] as 4+4 SBUF f32r tiles [128, 1024]
  cT: [512, 1024] as 4 fp32 tiles, updated in place
  gatesT: [2048, 1024] computed as 16 (m) x 2 (n) PSUM tiles [128, 512]
  xT (one-hot next-char): [128 vocab, 1024] f32r, 2 half tiles [128, 512]

argmax -> one-hot per step: PE-transpose logits (fp32, 2 cyc/row) into
[batch, vocab] blocks, DVE reduce_max along free dim, is_ge against the
per-row max emitting an exact 0/1 f32r mask, and PE-transpose the mask back
(f32r, 1.5 cyc/row) into [vocab, batch] for the next step's x matmul.
"""

import numpy as np
from contextlib import ExitStack

import concourse.bass as bass
import concourse.bacc as bacc
import concourse.mybir as mybir
import concourse.tile as tile
from concourse.bass_utils import run_bass_kernel_spmd

B, W, Q, H, A = 64, 128, 256, 512, 128
C = 20
NCORES = 8
BW = B * W
N_FULL = BW // NCORES  # 1024 batch rows per core

F32 = mybir.dt.float32
F32R = mybir.dt.float32r
AF = mybir.ActivationFunctionType
ALU = mybir.AluOpType
AX = mybir.AxisListType

KH = H // 128       # 4 hidden k-tiles
KQ = Q // 128       # 2 input k-tiles
MG = 4 * H // 128   # 16 gate m-tiles

LAST_RESULTS = None  # BassKernelResults of the most recent run (for test.py)


def split12(v):
    """Exact Dekker split of fp32 into two 12-bit-significand (f32r) parts."""
    v = np.ascontiguousarray(np.asarray(v, np.float32))
    u = v.view(np.uint32)
    r = u + (np.uint32(0x7FF) + ((u >> np.uint32(12)) & np.uint32(1)))
    hi = (r & np.uint32(0xFFFFF000)).view(np.float32)
    lo = (v - hi).astype(np.float32)
    return hi, lo


def build_program(c_steps: int = C, n: int = N_FULL) -> bass.Bass:
    assert n % 512 == 0
    nt = n // 512    # PSUM n-tiles per row block
    nb = n // 128    # 128-wide argmax blocks

    nc = bacc.Bacc("TRN2", target_bir_lowering=False, debug=False)

    qrT = nc.dram_tensor("qrT", [Q, n], F32, kind="ExternalInput").ap()
    winT = nc.dram_tensor("winT", [Q, H], F32, kind="ExternalInput").ap()
    whhT1 = nc.dram_tensor("whhT1", [H, 4 * H], F32R, kind="ExternalInput").ap()
    whhT2 = nc.dram_tensor("whhT2", [H, 4 * H], F32R, kind="ExternalInput").ap()
    wihT1 = nc.dram_tensor("wihT1", [A, 4 * H], F32R, kind="ExternalInput").ap()
    wihT2 = nc.dram_tensor("wihT2", [A, 4 * H], F32R, kind="ExternalInput").ap()
    woutT1 = nc.dram_tensor("woutT1", [H, A], F32R, kind="ExternalInput").ap()
    woutT2 = nc.dram_tensor("woutT2", [H, A], F32R, kind="ExternalInput").ap()
    b_in_c = nc.dram_tensor("b_in_c", [128, KH], F32, kind="ExternalInput").ap()
    bias_c = nc.dram_tensor("bias_c", [128, MG], F32, kind="ExternalInput").ap()
    bias0_c = nc.dram_tensor("bias0_c", [128, MG], F32, kind="ExternalInput").ap()
    b_out_c = nc.dram_tensor("b_out_c", [128, 1], F32, kind="ExternalInput").ap()
    ident = nc.dram_tensor("ident", [128, 128], F32, kind="ExternalInput").ap()
    identr = nc.dram_tensor("identr", [128, 128], F32R, kind="ExternalInput").ap()
    out = nc.dram_tensor("out_logits", [c_steps, A, n], F32, kind="ExternalOutput").ap()

    with tile.TileContext(nc) as tc, ExitStack() as ctx:
        wp = ctx.enter_context(tc.tile_pool(name="weights", bufs=1))
        sp = ctx.enter_context(tc.tile_pool(name="state", bufs=2))
        cp = ctx.enter_context(tc.tile_pool(name="cell", bufs=1))
        gp = ctx.enter_context(tc.tile_pool(name="gates", bufs=1))
        tp = ctx.enter_context(tc.tile_pool(name="tmp", bufs=1))
        lp = ctx.enter_context(tc.tile_pool(name="logits", bufs=1))
        mp = ctx.enter_context(tc.tile_pool(name="small", bufs=2))
        pg = ctx.enter_context(tc.tile_pool(name="pgate", bufs=6, space="PSUM"))
        pl = ctx.enter_context(tc.tile_pool(name="plog", bufs=2, space="PSUM"))

        # --- DMAs ordered by first use: h0 inputs, then hh weights (k-
        #     interleaved hi/lo to match the k-major accumulation order),
        #     then step-0 logits/argmax operands, then step-1 x operands ---
        h1 = [sp.tile([128, n], F32R, tag=f"h1{j}", name=f"h1{j}") for j in range(KH)]
        h2 = [sp.tile([128, n], F32R, tag=f"h2{j}", name=f"h2{j}") for j in range(KH)]
        ct = [cp.tile([128, n], F32, tag=f"c{j}", name=f"c{j}") for j in range(KH)]
        # setup reuses steady-state tiles: qr k-tiles live in the g0/g1 gate
        # buffers, w_in^T packs both k-tiles into the logits buffer
        win_t = lp.tile([128, n], F32, tag="logits", name="win_all")
        for k in range(KQ):
            nc.sync.dma_start(win_t[:, k * H:k * H + H], winT[k * 128:(k + 1) * 128, :])
        qr_t = []
        for k in range(KQ):
            t = gp.tile([128, n], F32, tag=f"g{k}", name=f"qr{k}")
            nc.sync.dma_start(t[:], qrT[k * 128:(k + 1) * 128, :])
            qr_t.append(t)
        b_in_t = wp.tile([128, KH], F32, tag="b_in")
        nc.sync.dma_start(b_in_t[:], b_in_c[:, :])
        bias0_t = wp.tile([128, MG], F32, tag="bias0")
        nc.sync.dma_start(bias0_t[:], bias0_c[:, :])
        whh1_t, whh2_t = [], []
        for k in range(KH):
            t1 = wp.tile([128, 4 * H], F32R, tag=f"whh1{k}", name=f"whh1{k}")
            nc.sync.dma_start(t1[:], whhT1[k * 128:(k + 1) * 128, :])
            whh1_t.append(t1)
            t2 = wp.tile([128, 4 * H], F32R, tag=f"whh2{k}", name=f"whh2{k}")
            nc.sync.dma_start(t2[:], whhT2[k * 128:(k + 1) * 128, :])
            whh2_t.append(t2)
        wout1_t = wp.tile([128, KH * A], F32R, tag="wout1")
        wout2_t = wp.tile([128, KH * A], F32R, tag="wout2")
        for k in range(KH):
            nc.sync.dma_start(wout1_t[:, k * A:(k + 1) * A], woutT1[k * 128:(k + 1) * 128, :])
            nc.sync.dma_start(wout2_t[:, k * A:(k + 1) * A], woutT2[k * 128:(k + 1) * 128, :])
        b_out_t = wp.tile([128, 1], F32, tag="b_out")
        nc.sync.dma_start(b_out_t[:], b_out_c[:, :])
        id_t = wp.tile([128, 128], F32, tag="ident")
        nc.sync.dma_start(id_t[:], ident[:, :])
        idr_t = wp.tile([128, 128], F32R, tag="identr")
        nc.sync.dma_start(idr_t[:], identr[:, :])
        wih1_t = wp.tile([128, 4 * H], F32R, tag="wih1")
        nc.sync.dma_start(wih1_t[:], wihT1[:, :])
        wih2_t = wp.tile([128, 4 * H], F32R, tag="wih2")
        nc.sync.dma_start(wih2_t[:], wihT2[:, :])
        bias_t = wp.tile([128, MG], F32, tag="bias")
        nc.sync.dma_start(bias_t[:], bias_c[:, :])
        for j in range(KH):
            nc.vector.memset(ct[j][:], 0.0)
            hf = tp.tile([128, n], F32, tag="t1")
            for n0 in range(nt):
                ps = pg.tile([128, 512], F32, tag="pg")
                for k in range(KQ):
                    nc.tensor.matmul(
                        ps[:],
                        lhsT=win_t[:, k * H + j * 128:k * H + (j + 1) * 128],
                        rhs=qr_t[k][:, n0 * 512:(n0 + 1) * 512],
                        start=(k == 0), stop=(k == KQ - 1),
                    )
                nc.scalar.activation(
                    hf[:, n0 * 512:(n0 + 1) * 512], ps[:],
                    AF.Identity, bias=b_in_t[:, j:j + 1],
                )
            nc.vector.tensor_copy(h1[j][:], hf[:])
            nc.vector.tensor_sub(h2[j][:], hf[:], h1[j][:].bitcast(F32))

        # collapse all setup deps so steady-state instructions carry few waits
        tc.strict_bb_all_engine_barrier()

        xt = None  # one-hot input halves [vocab, batch] f32r; step 0 folds into bias0

        for t in range(c_steps):
            h1n = [sp.tile([128, n], F32R, tag=f"h1{j}", name=f"h1{j}") for j in range(KH)]
            h2n = [sp.tile([128, n], F32R, tag=f"h2{j}", name=f"h2{j}") for j in range(KH)]
            for j in range(KH):
                g_j = [gp.tile([128, n], F32, tag=f"g{q}", name=f"g{q}") for q in range(4)]
                for q in range(4):  # i, f, g, o
                    m = q * KH + j
                    for n0 in range(nt):
                        ps = pg.tile([128, 512], F32, tag="pg")
                        for k in range(KH):
                            nc.tensor.matmul(
                                ps[:],
                                lhsT=whh1_t[k][:, m * 128:(m + 1) * 128],
                                rhs=h1[k][:, n0 * 512:(n0 + 1) * 512],
                                start=(k == 0), stop=False,
                            )
                            nc.tensor.matmul(
                                ps[:],
                                lhsT=whh2_t[k][:, m * 128:(m + 1) * 128],
                                rhs=h1[k][:, n0 * 512:(n0 + 1) * 512],
                                start=False, stop=False,
                            )
                            nc.tensor.matmul(
                                ps[:],
                                lhsT=whh1_t[k][:, m * 128:(m + 1) * 128],
                                rhs=h2[k][:, n0 * 512:(n0 + 1) * 512],
                                start=False, stop=(t == 0 and k == KH - 1),
                            )
                        if t > 0:
                            nc.tensor.matmul(
                                ps[:],
                                lhsT=wih1_t[:, m * 128:(m + 1) * 128],
                                rhs=xt[n0][:],
                                start=False, stop=False,
                            )
                            nc.tensor.matmul(
                                ps[:],
                                lhsT=wih2_t[:, m * 128:(m + 1) * 128],
                                rhs=xt[n0][:],
                                start=False, stop=True,
                            )
                        bias_ap = (bias0_t if t == 0 else bias_t)[:, m:m + 1]
                        func = AF.Tanh if q == 2 else AF.Sigmoid
                        nc.scalar.activation(
                            g_j[q][:, n0 * 512:(n0 + 1) * 512],
                            ps[:], func, bias=bias_ap,
                        )
                # cell/hidden update block j: c = f*c + i*g ; h = o*tanh(c)
                gi, gf, gg, go = (g[:] for g in g_j)
                cs = ct[j][:]
                t1 = tp.tile([128, n], F32, tag="t1")
                nc.vector.tensor_mul(t1[:], gi, gg)
                nc.vector.tensor_mul(cs, gf, cs)
                nc.vector.tensor_add(cs, cs, t1[:])
                t2 = tp.tile([128, n], F32, tag="t2")
                nc.scalar.activation(t2[:], cs, AF.Tanh)
                hf = tp.tile([128, n], F32, tag="t1")
                nc.vector.tensor_mul(hf[:], go, t2[:])
                nc.vector.tensor_copy(h1n[j][:], hf[:])
                nc.vector.tensor_sub(h2n[j][:], hf[:], h1n[j][:].bitcast(F32))

            # logits = w_out @ h + b_out  (in [vocab, batch] layout)
            lg = lp.tile([128, n], F32, tag="logits")
            for n0 in range(nt):
                ps = pl.tile([128, 512], F32, tag="pl")
                for k in range(KH):
                    nc.tensor.matmul(
                        ps[:],
                        lhsT=wout1_t[:, k * A:(k + 1) * A],
                        rhs=h1n[k][:, n0 * 512:(n0 + 1) * 512],
                        start=(k == 0), stop=False,
                    )
                for k in range(KH):
                    nc.tensor.matmul(
                        ps[:],
                        lhsT=wout2_t[:, k * A:(k + 1) * A],
                        rhs=h1n[k][:, n0 * 512:(n0 + 1) * 512],
                        start=False, stop=False,
                    )
                for k in range(KH):
                    nc.tensor.matmul(
                        ps[:],
                        lhsT=wout1_t[:, k * A:(k + 1) * A],
                        rhs=h2n[k][:, n0 * 512:(n0 + 1) * 512],
                        start=False, stop=(k == KH - 1),
                    )
                nc.scalar.activation(
                    lg[:, n0 * 512:(n0 + 1) * 512], ps[:],
                    AF.Identity, bias=b_out_t[:, 0:1],
                )
                nc.sync.dma_start(
                    out[t, :, n0 * 512:(n0 + 1) * 512],
                    lg[:, n0 * 512:(n0 + 1) * 512],
                )

            # next input: one-hot(argmax(logits)) back in [vocab, batch] f32r
            if t < c_steps - 1:
                xt = [sp.tile([128, 512], F32R, tag=f"x{h}", name=f"x{h}") for h in range(nt)]
                for b in range(nb):
                    ptile = pl.tile([128, 128], F32, tag="pl")
                    nc.tensor.transpose(ptile[:], lg[:, b * 128:(b + 1) * 128], id_t[:])
                    mx = mp.tile([128, 1], F32, tag="mx")
                    nc.vector.reduce_max(mx[:], ptile[:], axis=AX.X)
                    mb = mp.tile([128, 128], F32R, tag="mb")
                    nc.vector.tensor_scalar(mb[:], ptile[:], mx[:, 0:1], None, ALU.is_ge)
                    pback = pl.tile([128, 128], F32R, tag="pl")
                    nc.tensor.transpose(pback[:], mb[:], idr_t[:])
                    nc.vector.tensor_copy(
                        xt[b // 4][:, (b % 4) * 128:(b % 4 + 1) * 128],
                        pback[:].bitcast(F32))

            h1, h2 = h1n, h2n

    nc.compile()
    return nc


def make_in_maps(inputs: dict, c_steps: int = C, n: int = N_FULL, ncores: int = NCORES):
    f32 = np.float32
    qr = np.ascontiguousarray(np.asarray(inputs["quantized_repr"], f32)).reshape(BW, Q)
    w_in = np.asarray(inputs["w_in"], f32)
    b_in = np.asarray(inputs["b_in"], f32)
    w_ih = np.asarray(inputs["w_ih"], f32)
    w_hh = np.asarray(inputs["w_hh"], f32)
    b_ih = np.asarray(inputs["b_ih"], f32)
    b_hh = np.asarray(inputs["b_hh"], f32)
    w_out = np.asarray(inputs["w_out"], f32)
    b_out = np.asarray(inputs["b_out"], f32)

    bias = b_ih + b_hh                    # fp32, same as reference
    bias0 = bias + w_ih[:, 0]             # step-0 one-hot(0) contribution folded in

    whh1, whh2 = split12(w_hh.T)
    wih1, wih2 = split12(w_ih.T)
    wout1, wout2 = split12(w_out.T)

    shared = {
        "winT": np.ascontiguousarray(w_in.T),
        "whhT1": whh1, "whhT2": whh2,
        "wihT1": wih1, "wihT2": wih2,
        "woutT1": wout1, "woutT2": wout2,
        "b_in_c": np.ascontiguousarray(b_in.reshape(KH, 128).T),
        "bias_c": np.ascontiguousarray(bias.reshape(MG, 128).T),
        "bias0_c": np.ascontiguousarray(bias0.reshape(MG, 128).T),
        "b_out_c": np.ascontiguousarray(b_out.reshape(128, 1)),
        "ident": np.eye(128, dtype=f32),
        "identr": np.eye(128, dtype=f32),
    }
    in_maps = []
    for i in range(ncores):
        m = dict(shared)
        m["qrT"] = np.ascontiguousarray(qr[i * n:(i + 1) * n].T)
        in_maps.append(m)
    return in_maps


def kernel(**inputs) -> np.ndarray:
    global LAST_RESULTS
    assert int(inputs["max_char_len"]) == C
    nc = build_program(C, N_FULL)
    in_maps = make_in_maps(inputs, C, N_FULL, NCORES)
    res = run_bass_kernel_spmd(nc, in_maps, core_ids=list(range(NCORES)))
    LAST_RESULTS = res
    # per-core [C, A, N] -> [N, C, A]; concat cores -> [BW, C, A] -> [B, W, C, A]
    parts = [np.transpose(r["out_logits"], (2, 0, 1)) for r in res.results]
    full = np.concatenate(parts, axis=0).reshape(B, W, C, A)
    return np.ascontiguousarray(full)


# revision 9
# speedup vs baseline: 1.4236x; 1.0195x over previous
"""CharDecoder LSTM kernel for 8 Trainium2 NeuronCores.

Data-parallel over the flattened (B*W)=8192 batch axis: each of the 8 cores
processes 1024 sequences. Small LSTM/projection weights are replicated; the
hidden/cell state stays resident in SBUF (in transposed [feature, batch]
layout) across all 20 decode steps.

Numerics: all matmuls run in float32r (TF32-like: 8-bit exponent, 11-bit
stored mantissa, 1 cycle/row for free dim >= 256) using an exact Dekker
hi/lo split of both operands:
    w = w1 + w2 (exact, host-side), h = h1 + h2 (exact, 2 DVE ops/tile)
    w @ h ~= w1@h1 + w2@h1 + w1@h2     (dropped w2@h2 term ~ 2^-24)
which is fp32-quality (verified ~1e-6 max rel err end-to-end, 0 argmax
flips) at 3 cycles/row instead of fp32's 4 cycles/row on the PE.

Per-core layout ("layout B", feature-on-partitions):
  h1/h2 splits: [512, 1024] as 4+4 SBUF f32r tiles [128, 1024]
  cT: [512, 1024] as 4 fp32 tiles, updated in place
  gatesT: [2048, 1024] computed as 16 (m) x 2 (n) PSUM tiles [128, 512]
  xT (one-hot next-char): [128 vocab, 1024] f32r, 2 half tiles [128, 512]

argmax -> one-hot per step: PE-transpose logits (fp32, 2 cyc/row) into
[batch, vocab] blocks, DVE reduce_max along free dim, is_ge against the
per-row max emitting an exact 0/1 f32r mask, and PE-transpose the mask back
(f32r, 1.5 cyc/row) into [vocab, batch] for the next step's x matmul.
"""

import numpy as np
from contextlib import ExitStack

import concourse.bass as bass
import concourse.bass_isa as bass_isa
import concourse.bacc as bacc
import concourse.mybir as mybir
import concourse.tile as tile
from concourse.bass_utils import run_bass_kernel_spmd

B, W, Q, H, A = 64, 128, 256, 512, 128
C = 20
NCORES = 8
BW = B * W
N_FULL = BW // NCORES  # 1024 batch rows per core

F32 = mybir.dt.float32
F32R = mybir.dt.float32r
AF = mybir.ActivationFunctionType
ALU = mybir.AluOpType
AX = mybir.AxisListType

KH = H // 128       # 4 hidden k-tiles
KQ = Q // 128       # 2 input k-tiles
MG = 4 * H // 128   # 16 gate m-tiles

LAST_RESULTS = None  # BassKernelResults of the most recent run (for test.py)


def split12(v):
    """Exact Dekker split of fp32 into two 12-bit-significand (f32r) parts."""
    v = np.ascontiguousarray(np.asarray(v, np.float32))
    u = v.view(np.uint32)
    r = u + (np.uint32(0x7FF) + ((u >> np.uint32(12)) & np.uint32(1)))
    hi = (r & np.uint32(0xFFFFF000)).view(np.float32)
    lo = (v - hi).astype(np.float32)
    return hi, lo


def build_program(c_steps: int = C, n: int = N_FULL) -> bass.Bass:
    assert n % 512 == 0
    nt = n // 512    # PSUM n-tiles per row block
    nb = n // 128    # 128-wide argmax blocks

    nc = bacc.Bacc("TRN2", target_bir_lowering=False, debug=False)

    qrT = nc.dram_tensor("qrT", [Q, n], F32, kind="ExternalInput").ap()
    winT = nc.dram_tensor("winT", [Q, H], F32, kind="ExternalInput").ap()
    whhT1 = nc.dram_tensor("whhT1", [H, 4 * H], F32R, kind="ExternalInput").ap()
    whhT2 = nc.dram_tensor("whhT2", [H, 4 * H], F32R, kind="ExternalInput").ap()
    wihT1 = nc.dram_tensor("wihT1", [A, 4 * H], F32R, kind="ExternalInput").ap()
    wihT2 = nc.dram_tensor("wihT2", [A, 4 * H], F32R, kind="ExternalInput").ap()
    woutT1 = nc.dram_tensor("woutT1", [H, A], F32R, kind="ExternalInput").ap()
    woutT2 = nc.dram_tensor("woutT2", [H, A], F32R, kind="ExternalInput").ap()
    b_in_c = nc.dram_tensor("b_in_c", [128, KH], F32, kind="ExternalInput").ap()
    bias_c = nc.dram_tensor("bias_c", [128, MG], F32, kind="ExternalInput").ap()
    bias0_c = nc.dram_tensor("bias0_c", [128, MG], F32, kind="ExternalInput").ap()
    b_out_c = nc.dram_tensor("b_out_c", [128, 1], F32, kind="ExternalInput").ap()
    out = nc.dram_tensor("out_logits", [c_steps, A, n], F32, kind="ExternalOutput").ap()

    with tile.TileContext(nc) as tc, ExitStack() as ctx:
        wp = ctx.enter_context(tc.tile_pool(name="weights", bufs=1))
        sp = ctx.enter_context(tc.tile_pool(name="state", bufs=2))
        cp = ctx.enter_context(tc.tile_pool(name="cell", bufs=1))
        gp = ctx.enter_context(tc.tile_pool(name="gates", bufs=1))
        tp = ctx.enter_context(tc.tile_pool(name="tmp", bufs=1))
        lp = ctx.enter_context(tc.tile_pool(name="logits", bufs=1))
        mp = ctx.enter_context(tc.tile_pool(name="small", bufs=2))
        pg = ctx.enter_context(tc.tile_pool(name="pgate", bufs=6, space="PSUM"))
        pl = ctx.enter_context(tc.tile_pool(name="plog", bufs=2, space="PSUM"))

        # --- DMAs ordered by first use: h0 inputs, then hh weights (k-
        #     interleaved hi/lo to match the k-major accumulation order),
        #     then step-0 logits/argmax operands, then step-1 x operands ---
        h1 = [sp.tile([128, n], F32R, tag=f"h1{j}", name=f"h1{j}") for j in range(KH)]
        h2 = [sp.tile([128, n], F32R, tag=f"h2{j}", name=f"h2{j}") for j in range(KH)]
        ct = [cp.tile([128, n], F32, tag=f"c{j}", name=f"c{j}") for j in range(KH)]
        # setup reuses steady-state tiles: qr k-tiles live in the g0/g1 gate
        # buffers, w_in^T packs both k-tiles into the logits buffer
        win_t = lp.tile([128, n], F32, tag="logits", name="win_all")
        for k in range(KQ):
            nc.sync.dma_start(win_t[:, k * H:k * H + H], winT[k * 128:(k + 1) * 128, :])
        qr_t = []
        for k in range(KQ):
            t = gp.tile([128, n], F32, tag=f"g{k}", name=f"qr{k}")
            nc.sync.dma_start(t[:], qrT[k * 128:(k + 1) * 128, :])
            qr_t.append(t)
        b_in_t = wp.tile([128, KH], F32, tag="b_in")
        nc.sync.dma_start(b_in_t[:], b_in_c[:, :])
        bias0_t = wp.tile([128, MG], F32, tag="bias0")
        nc.sync.dma_start(bias0_t[:], bias0_c[:, :])
        whh1_t, whh2_t = [], []
        for k in range(KH):
            t1 = wp.tile([128, 4 * H], F32R, tag=f"whh1{k}", name=f"whh1{k}")
            nc.sync.dma_start(t1[:], whhT1[k * 128:(k + 1) * 128, :])
            whh1_t.append(t1)
            t2 = wp.tile([128, 4 * H], F32R, tag=f"whh2{k}", name=f"whh2{k}")
            nc.sync.dma_start(t2[:], whhT2[k * 128:(k + 1) * 128, :])
            whh2_t.append(t2)
        wout1_t = wp.tile([128, KH * A], F32R, tag="wout1")
        wout2_t = wp.tile([128, KH * A], F32R, tag="wout2")
        for k in range(KH):
            nc.sync.dma_start(wout1_t[:, k * A:(k + 1) * A], woutT1[k * 128:(k + 1) * 128, :])
            nc.sync.dma_start(wout2_t[:, k * A:(k + 1) * A], woutT2[k * 128:(k + 1) * 128, :])
        b_out_t = wp.tile([128, 1], F32, tag="b_out")
        nc.sync.dma_start(b_out_t[:], b_out_c[:, :])
        wih1_t = wp.tile([128, 4 * H], F32R, tag="wih1")
        nc.sync.dma_start(wih1_t[:], wihT1[:, :])
        wih2_t = wp.tile([128, 4 * H], F32R, tag="wih2")
        nc.sync.dma_start(wih2_t[:], wihT2[:, :])
        bias_t = wp.tile([128, MG], F32, tag="bias")
        nc.sync.dma_start(bias_t[:], bias_c[:, :])
        for j in range(KH):
            nc.vector.memset(ct[j][:], 0.0)
            hf = tp.tile([128, n], F32, tag="t1")
            for n0 in range(nt):
                ps = pg.tile([128, 512], F32, tag="pg")
                for k in range(KQ):
                    nc.tensor.matmul(
                        ps[:],
                        lhsT=win_t[:, k * H + j * 128:k * H + (j + 1) * 128],
                        rhs=qr_t[k][:, n0 * 512:(n0 + 1) * 512],
                        start=(k == 0), stop=(k == KQ - 1),
                    )
                nc.scalar.activation(
                    hf[:, n0 * 512:(n0 + 1) * 512], ps[:],
                    AF.Identity, bias=b_in_t[:, j:j + 1],
                )
            nc.vector.tensor_copy(h1[j][:], hf[:])
            nc.vector.tensor_sub(h2[j][:], hf[:], h1[j][:].bitcast(F32))

        # collapse all setup deps so steady-state instructions carry few waits
        tc.strict_bb_all_engine_barrier()

        xt = None  # one-hot input halves [vocab, batch] f32r; step 0 folds into bias0

        for t in range(c_steps):
            h1n = [sp.tile([128, n], F32R, tag=f"h1{j}", name=f"h1{j}") for j in range(KH)]
            h2n = [sp.tile([128, n], F32R, tag=f"h2{j}", name=f"h2{j}") for j in range(KH)]
            for j in range(KH):
                g_j = [gp.tile([128, n], F32, tag=f"g{q}", name=f"g{q}") for q in range(4)]
                for q in range(4):  # i, f, g, o
                    m = q * KH + j
                    for n0 in range(nt):
                        ps = pg.tile([128, 512], F32, tag="pg")
                        for k in range(KH):
                            nc.tensor.matmul(
                                ps[:],
                                lhsT=whh1_t[k][:, m * 128:(m + 1) * 128],
                                rhs=h1[k][:, n0 * 512:(n0 + 1) * 512],
                                start=(k == 0), stop=False,
                            )
                            nc.tensor.matmul(
                                ps[:],
                                lhsT=whh2_t[k][:, m * 128:(m + 1) * 128],
                                rhs=h1[k][:, n0 * 512:(n0 + 1) * 512],
                                start=False, stop=False,
                            )
                            nc.tensor.matmul(
                                ps[:],
                                lhsT=whh1_t[k][:, m * 128:(m + 1) * 128],
                                rhs=h2[k][:, n0 * 512:(n0 + 1) * 512],
                                start=False, stop=(t == 0 and k == KH - 1),
                            )
                        if t > 0:
                            nc.tensor.matmul(
                                ps[:],
                                lhsT=wih1_t[:, m * 128:(m + 1) * 128],
                                rhs=xt[n0][:],
                                start=False, stop=False,
                            )
                            nc.tensor.matmul(
                                ps[:],
                                lhsT=wih2_t[:, m * 128:(m + 1) * 128],
                                rhs=xt[n0][:],
                                start=False, stop=True,
                            )
                        bias_ap = (bias0_t if t == 0 else bias_t)[:, m:m + 1]
                        func = AF.Tanh if q == 2 else AF.Sigmoid
                        nc.scalar.activation(
                            g_j[q][:, n0 * 512:(n0 + 1) * 512],
                            ps[:], func, bias=bias_ap,
                        )
                # cell/hidden update block j: c = f*c + i*g ; h = o*tanh(c)
                gi, gf, gg, go = (g[:] for g in g_j)
                cs = ct[j][:]
                t1 = tp.tile([128, n], F32, tag="t1")
                nc.vector.tensor_mul(t1[:], gi, gg)
                nc.vector.tensor_mul(cs, gf, cs)
                nc.vector.tensor_add(cs, cs, t1[:])
                t2 = tp.tile([128, n], F32, tag="t2")
                nc.scalar.activation(t2[:], cs, AF.Tanh)
                hf = tp.tile([128, n], F32, tag="t1")
                nc.vector.tensor_mul(hf[:], go, t2[:])
                nc.vector.tensor_copy(h1n[j][:], hf[:])
                nc.vector.tensor_sub(h2n[j][:], hf[:], h1n[j][:].bitcast(F32))

            # logits = w_out @ h + b_out  (in [vocab, batch] layout)
            lg = lp.tile([128, n], F32, tag="logits")
            for n0 in range(nt):
                ps = pl.tile([128, 512], F32, tag="pl")
                for k in range(KH):
                    nc.tensor.matmul(
                        ps[:],
                        lhsT=wout1_t[:, k * A:(k + 1) * A],
                        rhs=h1n[k][:, n0 * 512:(n0 + 1) * 512],
                        start=(k == 0), stop=False,
                    )
                for k in range(KH):
                    nc.tensor.matmul(
                        ps[:],
                        lhsT=wout2_t[:, k * A:(k + 1) * A],
                        rhs=h1n[k][:, n0 * 512:(n0 + 1) * 512],
                        start=False, stop=False,
                    )
                for k in range(KH):
                    nc.tensor.matmul(
                        ps[:],
                        lhsT=wout1_t[:, k * A:(k + 1) * A],
                        rhs=h2n[k][:, n0 * 512:(n0 + 1) * 512],
                        start=False, stop=(k == KH - 1),
                    )
                nc.scalar.activation(
                    lg[:, n0 * 512:(n0 + 1) * 512], ps[:],
                    AF.Identity, bias=b_out_t[:, 0:1],
                )
                nc.sync.dma_start(
                    out[t, :, n0 * 512:(n0 + 1) * 512],
                    lg[:, n0 * 512:(n0 + 1) * 512],
                )

            # next input: one-hot(argmax(logits)) in [vocab, batch] f32r via
            # GPSIMD cross-partition max + DVE is_ge (no PE transposes)
            if t < c_steps - 1:
                xt = [sp.tile([128, 512], F32R, tag=f"x{h}", name=f"x{h}") for h in range(nt)]
                mxb = tp.tile([128, n], F32, tag="t2")
                for n0 in range(nt):
                    nc.gpsimd.partition_all_reduce(
                        mxb[:, n0 * 512:(n0 + 1) * 512],
                        lg[:, n0 * 512:(n0 + 1) * 512],
                        channels=128, reduce_op=bass_isa.ReduceOp.max)
                    nc.vector.tensor_tensor(
                        xt[n0][:],
                        lg[:, n0 * 512:(n0 + 1) * 512],
                        mxb[:, n0 * 512:(n0 + 1) * 512],
                        ALU.is_ge)

            h1, h2 = h1n, h2n

    nc.compile()
    return nc


def make_in_maps(inputs: dict, c_steps: int = C, n: int = N_FULL, ncores: int = NCORES):
    f32 = np.float32
    qr = np.ascontiguousarray(np.asarray(inputs["quantized_repr"], f32)).reshape(BW, Q)
    w_in = np.asarray(inputs["w_in"], f32)
    b_in = np.asarray(inputs["b_in"], f32)
    w_ih = np.asarray(inputs["w_ih"], f32)
    w_hh = np.asarray(inputs["w_hh"], f32)
    b_ih = np.asarray(inputs["b_ih"], f32)
    b_hh = np.asarray(inputs["b_hh"], f32)
    w_out = np.asarray(inputs["w_out"], f32)
    b_out = np.asarray(inputs["b_out"], f32)

    bias = b_ih + b_hh                    # fp32, same as reference
    bias0 = bias + w_ih[:, 0]             # step-0 one-hot(0) contribution folded in

    whh1, whh2 = split12(w_hh.T)
    wih1, wih2 = split12(w_ih.T)
    wout1, wout2 = split12(w_out.T)

    shared = {
        "winT": np.ascontiguousarray(w_in.T),
        "whhT1": whh1, "whhT2": whh2,
        "wihT1": wih1, "wihT2": wih2,
        "woutT1": wout1, "woutT2": wout2,
        "b_in_c": np.ascontiguousarray(b_in.reshape(KH, 128).T),
        "bias_c": np.ascontiguousarray(bias.reshape(MG, 128).T),
        "bias0_c": np.ascontiguousarray(bias0.reshape(MG, 128).T),
        "b_out_c": np.ascontiguousarray(b_out.reshape(128, 1)),
    }
    in_maps = []
    for i in range(ncores):
        m = dict(shared)
        m["qrT"] = np.ascontiguousarray(qr[i * n:(i + 1) * n].T)
        in_maps.append(m)
    return in_maps


def kernel(**inputs) -> np.ndarray:
    global LAST_RESULTS
    assert int(inputs["max_char_len"]) == C
    nc = build_program(C, N_FULL)
    in_maps = make_in_maps(inputs, C, N_FULL, NCORES)
    res = run_bass_kernel_spmd(nc, in_maps, core_ids=list(range(NCORES)))
    LAST_RESULTS = res
    # per-core [C, A, N] -> [N, C, A]; concat cores -> [BW, C, A] -> [B, W, C, A]
    parts = [np.transpose(r["out_logits"], (2, 0, 1)) for r in res.results]
    full = np.concatenate(parts, axis=0).reshape(B, W, C, A)
    return np.ascontiguousarray(full)


# revision 17
# speedup vs baseline: 1.4251x; 1.0010x over previous
"""CharDecoder LSTM kernel for 8 Trainium2 NeuronCores.

Data-parallel over the flattened (B*W)=8192 batch axis: each of the 8 cores
processes 1024 sequences. Small LSTM/projection weights are replicated; the
hidden/cell state stays resident in SBUF (in transposed [feature, batch]
layout) across all 20 decode steps.

Numerics: all matmuls run in float32r (TF32-like: 8-bit exponent, 11-bit
stored mantissa, 1 cycle/row for free dim >= 256) using an exact Dekker
hi/lo split of both operands:
    w = w1 + w2 (exact, host-side), h = h1 + h2 (exact, 2 DVE ops/tile)
    w @ h ~= w1@h1 + w2@h1 + w1@h2     (dropped w2@h2 term ~ 2^-24)
which is fp32-quality (verified ~1e-6 max rel err end-to-end, 0 argmax
flips) at 3 cycles/row instead of fp32's 4 cycles/row on the PE.

Per-core layout ("layout B", feature-on-partitions):
  h1/h2 splits: [512, 1024] as 4+4 SBUF f32r tiles [128, 1024]
  cT: [512, 1024] as 4 fp32 tiles, updated in place
  gatesT: [2048, 1024] computed as 16 (m) x 2 (n) PSUM tiles [128, 512]
  xT (one-hot next-char): [128 vocab, 1024] f32r, 2 half tiles [128, 512]

argmax -> one-hot per step: PE-transpose logits (fp32, 2 cyc/row) into
[batch, vocab] blocks, DVE reduce_max along free dim, is_ge against the
per-row max emitting an exact 0/1 f32r mask, and PE-transpose the mask back
(f32r, 1.5 cyc/row) into [vocab, batch] for the next step's x matmul.
"""

import numpy as np
from contextlib import ExitStack

import concourse.bass as bass
import concourse.bass_isa as bass_isa
import concourse.bacc as bacc
import concourse.mybir as mybir
import concourse.tile as tile
from concourse.bass_utils import run_bass_kernel_spmd

B, W, Q, H, A = 64, 128, 256, 512, 128
C = 20
NCORES = 8
BW = B * W
N_FULL = BW // NCORES  # 1024 batch rows per core

F32 = mybir.dt.float32
F32R = mybir.dt.float32r
AF = mybir.ActivationFunctionType
ALU = mybir.AluOpType
AX = mybir.AxisListType

KH = H // 128       # 4 hidden k-tiles
KQ = Q // 128       # 2 input k-tiles
MG = 4 * H // 128   # 16 gate m-tiles

LAST_RESULTS = None  # BassKernelResults of the most recent run (for test.py)


def split12(v):
    """Exact Dekker split of fp32 into two 12-bit-significand (f32r) parts."""
    v = np.ascontiguousarray(np.asarray(v, np.float32))
    u = v.view(np.uint32)
    r = u + (np.uint32(0x7FF) + ((u >> np.uint32(12)) & np.uint32(1)))
    hi = (r & np.uint32(0xFFFFF000)).view(np.float32)
    lo = (v - hi).astype(np.float32)
    return hi, lo


def build_program(c_steps: int = C, n: int = N_FULL) -> bass.Bass:
    assert n % 512 == 0
    nt = n // 512    # PSUM n-tiles per row block
    nb = n // 128    # 128-wide argmax blocks

    nc = bacc.Bacc("TRN2", target_bir_lowering=False, debug=False)

    qrT = nc.dram_tensor("qrT", [Q, n], F32, kind="ExternalInput").ap()
    winT = nc.dram_tensor("winT", [Q, H], F32, kind="ExternalInput").ap()
    whhT1 = nc.dram_tensor("whhT1", [H, 4 * H], F32R, kind="ExternalInput").ap()
    whhT2 = nc.dram_tensor("whhT2", [H, 4 * H], F32R, kind="ExternalInput").ap()
    wihT1 = nc.dram_tensor("wihT1", [A, 4 * H], F32R, kind="ExternalInput").ap()
    wihT2 = nc.dram_tensor("wihT2", [A, 4 * H], F32R, kind="ExternalInput").ap()
    woutT1 = nc.dram_tensor("woutT1", [H, A], F32R, kind="ExternalInput").ap()
    woutT2 = nc.dram_tensor("woutT2", [H, A], F32R, kind="ExternalInput").ap()
    b_in_c = nc.dram_tensor("b_in_c", [128, KH], F32, kind="ExternalInput").ap()
    bias_c = nc.dram_tensor("bias_c", [128, MG], F32, kind="ExternalInput").ap()
    bias0_c = nc.dram_tensor("bias0_c", [128, MG], F32, kind="ExternalInput").ap()
    b_out_c = nc.dram_tensor("b_out_c", [128, 1], F32, kind="ExternalInput").ap()
    out = nc.dram_tensor("out_logits", [c_steps, A, n], F32, kind="ExternalOutput").ap()

    with tile.TileContext(nc) as tc, ExitStack() as ctx:
        wp = ctx.enter_context(tc.tile_pool(name="weights", bufs=1))
        sp = ctx.enter_context(tc.tile_pool(name="state", bufs=2))
        cp = ctx.enter_context(tc.tile_pool(name="cell", bufs=1))
        gp = ctx.enter_context(tc.tile_pool(name="gates", bufs=1))
        tp = ctx.enter_context(tc.tile_pool(name="tmp", bufs=1))
        lp = ctx.enter_context(tc.tile_pool(name="logits", bufs=1))
        mp = ctx.enter_context(tc.tile_pool(name="small", bufs=2))
        pg = ctx.enter_context(tc.tile_pool(name="pgate", bufs=6, space="PSUM"))
        pl = ctx.enter_context(tc.tile_pool(name="plog", bufs=2, space="PSUM"))

        # --- DMAs ordered by first use: h0 inputs, then hh weights (k-
        #     interleaved hi/lo to match the k-major accumulation order),
        #     then step-0 logits/argmax operands, then step-1 x operands ---
        h1 = [sp.tile([128, n], F32R, tag=f"h1{j}", name=f"h1{j}") for j in range(KH)]
        h2 = [sp.tile([128, n], F32R, tag=f"h2{j}", name=f"h2{j}") for j in range(KH)]
        ct = [cp.tile([128, n], F32, tag=f"c{j}", name=f"c{j}") for j in range(KH)]
        # setup reuses steady-state tiles: qr k-tiles live in the g0/g1 gate
        # buffers, w_in^T packs both k-tiles into the logits buffer
        win_t = lp.tile([128, n], F32, tag="logits", name="win_all")
        for k in range(KQ):
            nc.sync.dma_start(win_t[:, k * H:k * H + H], winT[k * 128:(k + 1) * 128, :])
        qr_t = [gp.tile([128, n], F32, tag=f"g{k}", name=f"qr{k}") for k in range(KQ)]
        b_in_t = wp.tile([128, KH], F32, tag="b_in")
        for n0 in range(nt):
            for k in range(KQ):
                nc.sync.dma_start(qr_t[k][:, n0 * 512:(n0 + 1) * 512],
                                  qrT[k * 128:(k + 1) * 128, n0 * 512:(n0 + 1) * 512])
            if n0 == 0:
                nc.sync.dma_start(b_in_t[:], b_in_c[:, :])
        bias0_t = wp.tile([128, MG], F32, tag="bias0")
        nc.sync.dma_start(bias0_t[:], bias0_c[:, :])
        whh1_t, whh2_t = [], []
        for k in range(KH):
            t1 = wp.tile([128, 4 * H], F32R, tag=f"whh1{k}", name=f"whh1{k}")
            nc.sync.dma_start(t1[:], whhT1[k * 128:(k + 1) * 128, :])
            whh1_t.append(t1)
            t2 = wp.tile([128, 4 * H], F32R, tag=f"whh2{k}", name=f"whh2{k}")
            nc.sync.dma_start(t2[:], whhT2[k * 128:(k + 1) * 128, :])
            whh2_t.append(t2)
        wout1_t = wp.tile([128, KH * A], F32R, tag="wout1")
        wout2_t = wp.tile([128, KH * A], F32R, tag="wout2")
        for k in range(KH):
            nc.sync.dma_start(wout1_t[:, k * A:(k + 1) * A], woutT1[k * 128:(k + 1) * 128, :])
            nc.sync.dma_start(wout2_t[:, k * A:(k + 1) * A], woutT2[k * 128:(k + 1) * 128, :])
        b_out_t = wp.tile([128, 1], F32, tag="b_out")
        nc.sync.dma_start(b_out_t[:], b_out_c[:, :])
        wih1_t = wp.tile([128, 4 * H], F32R, tag="wih1")
        nc.sync.dma_start(wih1_t[:], wihT1[:, :])
        wih2_t = wp.tile([128, 4 * H], F32R, tag="wih2")
        nc.sync.dma_start(wih2_t[:], wihT2[:, :])
        bias_t = wp.tile([128, MG], F32, tag="bias")
        nc.sync.dma_start(bias_t[:], bias_c[:, :])
        for j in range(KH):
            nc.vector.memset(ct[j][:], 0.0)
            hf = tp.tile([128, n], F32, tag="t1")
            for n0 in range(nt):
                ps = pg.tile([128, 512], F32, tag="pg")
                for k in range(KQ):
                    nc.tensor.matmul(
                        ps[:],
                        lhsT=win_t[:, k * H + j * 128:k * H + (j + 1) * 128],
                        rhs=qr_t[k][:, n0 * 512:(n0 + 1) * 512],
                        start=(k == 0), stop=(k == KQ - 1),
                    )
                nc.scalar.activation(
                    hf[:, n0 * 512:(n0 + 1) * 512], ps[:],
                    AF.Identity, bias=b_in_t[:, j:j + 1],
                )
            nc.vector.tensor_copy(h1[j][:], hf[:])
            nc.vector.tensor_sub(h2[j][:], hf[:], h1[j][:].bitcast(F32))

        # collapse all setup deps so steady-state instructions carry few waits
        tc.strict_bb_all_engine_barrier()

        xt = None  # one-hot input halves [vocab, batch] f32r; step 0 folds into bias0

        for t in range(c_steps):
            h1n = [sp.tile([128, n], F32R, tag=f"h1{j}", name=f"h1{j}") for j in range(KH)]
            h2n = [sp.tile([128, n], F32R, tag=f"h2{j}", name=f"h2{j}") for j in range(KH)]
            for j in range(KH):
                g_j = [gp.tile([128, n], F32, tag=f"g{q}", name=f"g{q}") for q in range(4)]
                for q in range(4):  # i, f, g, o
                    m = q * KH + j
                    for n0 in range(nt):
                        ps = pg.tile([128, 512], F32, tag="pg")
                        for k in range(KH):
                            nc.tensor.matmul(
                                ps[:],
                                lhsT=whh1_t[k][:, m * 128:(m + 1) * 128],
                                rhs=h1[k][:, n0 * 512:(n0 + 1) * 512],
                                start=(k == 0), stop=False,
                            )
                            nc.tensor.matmul(
                                ps[:],
                                lhsT=whh2_t[k][:, m * 128:(m + 1) * 128],
                                rhs=h1[k][:, n0 * 512:(n0 + 1) * 512],
                                start=False, stop=False,
                            )
                            nc.tensor.matmul(
                                ps[:],
                                lhsT=whh1_t[k][:, m * 128:(m + 1) * 128],
                                rhs=h2[k][:, n0 * 512:(n0 + 1) * 512],
                                start=False, stop=(t == 0 and k == KH - 1),
                            )
                        if t > 0:
                            nc.tensor.matmul(
                                ps[:],
                                lhsT=wih1_t[:, m * 128:(m + 1) * 128],
                                rhs=xt[n0][:],
                                start=False, stop=False,
                            )
                            nc.tensor.matmul(
                                ps[:],
                                lhsT=wih2_t[:, m * 128:(m + 1) * 128],
                                rhs=xt[n0][:],
                                start=False, stop=True,
                            )
                        bias_ap = (bias0_t if t == 0 else bias_t)[:, m:m + 1]
                        func = AF.Tanh if q == 2 else AF.Sigmoid
                        nc.scalar.activation(
                            g_j[q][:, n0 * 512:(n0 + 1) * 512],
                            ps[:], func, bias=bias_ap,
                        )
                # cell/hidden update block j: c = f*c + i*g ; h = o*tanh(c)
                gi, gf, gg, go = (g[:] for g in g_j)
                cs = ct[j][:]
                t1 = tp.tile([128, n], F32, tag="t1")
                nc.vector.tensor_mul(t1[:], gi, gg)
                nc.vector.tensor_mul(cs, gf, cs)
                nc.vector.tensor_add(cs, cs, t1[:])
                t2 = tp.tile([128, n], F32, tag="t2")
                nc.scalar.activation(t2[:], cs, AF.Tanh)
                hf = tp.tile([128, n], F32, tag="t1")
                nc.vector.tensor_mul(hf[:], go, t2[:])
                nc.vector.tensor_copy(h1n[j][:], hf[:])
                if t < c_steps - 1:  # h2 unused at the last step (p1 logits)
                    nc.vector.tensor_sub(h2n[j][:], hf[:], h1n[j][:].bitcast(F32))

            # logits = w_out @ h + b_out  (in [vocab, batch] layout).
            # Last step feeds nothing back (no argmax/xt), so a single
            # f32r product (err ~1.5e-4 << 2e-2 tolerance) suffices there.
            last = t == c_steps - 1
            lg = lp.tile([128, n], F32, tag="logits")
            for n0 in range(nt):
                ps = pl.tile([128, 512], F32, tag="pl")
                for k in range(KH):
                    nc.tensor.matmul(
                        ps[:],
                        lhsT=wout1_t[:, k * A:(k + 1) * A],
                        rhs=h1n[k][:, n0 * 512:(n0 + 1) * 512],
                        start=(k == 0), stop=(last and k == KH - 1),
                    )
                if not last:
                    for k in range(KH):
                        nc.tensor.matmul(
                            ps[:],
                            lhsT=wout2_t[:, k * A:(k + 1) * A],
                            rhs=h1n[k][:, n0 * 512:(n0 + 1) * 512],
                            start=False, stop=False,
                        )
                    for k in range(KH):
                        nc.tensor.matmul(
                            ps[:],
                            lhsT=wout1_t[:, k * A:(k + 1) * A],
                            rhs=h2n[k][:, n0 * 512:(n0 + 1) * 512],
                            start=False, stop=(k == KH - 1),
                        )
                nc.scalar.activation(
                    lg[:, n0 * 512:(n0 + 1) * 512], ps[:],
                    AF.Identity, bias=b_out_t[:, 0:1],
                )
                nc.sync.dma_start(
                    out[t, :, n0 * 512:(n0 + 1) * 512],
                    lg[:, n0 * 512:(n0 + 1) * 512],
                )

            # next input: one-hot(argmax(logits)) in [vocab, batch] f32r via
            # GPSIMD cross-partition max + DVE is_ge (no PE transposes)
            if t < c_steps - 1:
                xt = [sp.tile([128, 512], F32R, tag=f"x{h}", name=f"x{h}") for h in range(nt)]
                mxb = tp.tile([128, n], F32, tag="t2")
                for n0 in range(nt):
                    nc.gpsimd.partition_all_reduce(
                        mxb[:, n0 * 512:(n0 + 1) * 512],
                        lg[:, n0 * 512:(n0 + 1) * 512],
                        channels=128, reduce_op=bass_isa.ReduceOp.max)
                    nc.vector.tensor_tensor(
                        xt[n0][:],
                        lg[:, n0 * 512:(n0 + 1) * 512],
                        mxb[:, n0 * 512:(n0 + 1) * 512],
                        ALU.is_ge)

            h1, h2 = h1n, h2n

    nc.compile()
    return nc


def make_in_maps(inputs: dict, c_steps: int = C, n: int = N_FULL, ncores: int = NCORES):
    f32 = np.float32
    qr = np.ascontiguousarray(np.asarray(inputs["quantized_repr"], f32)).reshape(BW, Q)
    w_in = np.asarray(inputs["w_in"], f32)
    b_in = np.asarray(inputs["b_in"], f32)
    w_ih = np.asarray(inputs["w_ih"], f32)
    w_hh = np.asarray(inputs["w_hh"], f32)
    b_ih = np.asarray(inputs["b_ih"], f32)
    b_hh = np.asarray(inputs["b_hh"], f32)
    w_out = np.asarray(inputs["w_out"], f32)
    b_out = np.asarray(inputs["b_out"], f32)

    bias = b_ih + b_hh                    # fp32, same as reference
    bias0 = bias + w_ih[:, 0]             # step-0 one-hot(0) contribution folded in

    whh1, whh2 = split12(w_hh.T)
    wih1, wih2 = split12(w_ih.T)
    wout1, wout2 = split12(w_out.T)

    shared = {
        "winT": np.ascontiguousarray(w_in.T),
        "whhT1": whh1, "whhT2": whh2,
        "wihT1": wih1, "wihT2": wih2,
        "woutT1": wout1, "woutT2": wout2,
        "b_in_c": np.ascontiguousarray(b_in.reshape(KH, 128).T),
        "bias_c": np.ascontiguousarray(bias.reshape(MG, 128).T),
        "bias0_c": np.ascontiguousarray(bias0.reshape(MG, 128).T),
        "b_out_c": np.ascontiguousarray(b_out.reshape(128, 1)),
    }
    in_maps = []
    for i in range(ncores):
        m = dict(shared)
        m["qrT"] = np.ascontiguousarray(qr[i * n:(i + 1) * n].T)
        in_maps.append(m)
    return in_maps


def kernel(**inputs) -> np.ndarray:
    global LAST_RESULTS
    assert int(inputs["max_char_len"]) == C
    nc = build_program(C, N_FULL)
    in_maps = make_in_maps(inputs, C, N_FULL, NCORES)
    res = run_bass_kernel_spmd(nc, in_maps, core_ids=list(range(NCORES)))
    LAST_RESULTS = res
    # per-core [C, A, N] -> [N, C, A]; concat cores -> [BW, C, A] -> [B, W, C, A]
    parts = [np.transpose(r["out_logits"], (2, 0, 1)) for r in res.results]
    full = np.concatenate(parts, axis=0).reshape(B, W, C, A)
    return np.ascontiguousarray(full)


# revision 21
# speedup vs baseline: 1.4312x; 1.0043x over previous
"""CharDecoder LSTM kernel for 8 Trainium2 NeuronCores.

Data-parallel over the flattened (B*W)=8192 batch axis: each of the 8 cores
processes 1024 sequences. Small LSTM/projection weights are replicated; the
hidden/cell state stays resident in SBUF (in transposed [feature, batch]
layout) across all 20 decode steps.

Numerics: all matmuls run in float32r (TF32-like: 8-bit exponent, 11-bit
stored mantissa, 1 cycle/row for free dim >= 256) using an exact Dekker
hi/lo split of both operands:
    w = w1 + w2 (exact, host-side), h = h1 + h2 (exact, 2 DVE ops/tile)
    w @ h ~= w1@h1 + w2@h1 + w1@h2     (dropped w2@h2 term ~ 2^-24)
which is fp32-quality (verified ~1e-6 max rel err end-to-end, 0 argmax
flips) at 3 cycles/row instead of fp32's 4 cycles/row on the PE.

Per-core layout ("layout B", feature-on-partitions):
  h1/h2 splits: [512, 1024] as 4+4 SBUF f32r tiles [128, 1024]
  cT: [512, 1024] as 4 fp32 tiles, updated in place
  gatesT: [2048, 1024] computed as 16 (m) x 2 (n) PSUM tiles [128, 512]
  xT (one-hot next-char): [128 vocab, 1024] f32r, 2 half tiles [128, 512]

argmax -> one-hot per step with zero PE work: GPSIMD partition_all_reduce
(max over the 128 vocab partitions, broadcast to all partitions), then a
DVE is_ge against the logits emitting the exact 0/1 one-hot directly in
[vocab, batch] layout as f32r for the next step's x matmul.
"""

import numpy as np
from contextlib import ExitStack

import concourse.bass as bass
import concourse.bass_isa as bass_isa
import concourse.bacc as bacc
import concourse.mybir as mybir
import concourse.tile as tile
from concourse.bass_utils import run_bass_kernel_spmd

B, W, Q, H, A = 64, 128, 256, 512, 128
C = 20
NCORES = 8
BW = B * W
N_FULL = BW // NCORES  # 1024 batch rows per core

F32 = mybir.dt.float32
F32R = mybir.dt.float32r
AF = mybir.ActivationFunctionType
ALU = mybir.AluOpType

KH = H // 128       # 4 hidden k-tiles
KQ = Q // 128       # 2 input k-tiles
MG = 4 * H // 128   # 16 gate m-tiles

LAST_RESULTS = None  # BassKernelResults of the most recent run (for test.py)


def split12(v):
    """Exact Dekker split of fp32 into two 12-bit-significand (f32r) parts."""
    v = np.ascontiguousarray(np.asarray(v, np.float32))
    u = v.view(np.uint32)
    r = u + (np.uint32(0x7FF) + ((u >> np.uint32(12)) & np.uint32(1)))
    hi = (r & np.uint32(0xFFFFF000)).view(np.float32)
    lo = (v - hi).astype(np.float32)
    return hi, lo


def build_program(c_steps: int = C, n: int = N_FULL) -> bass.Bass:
    assert n % 512 == 0
    nt = n // 512    # PSUM n-tiles per row block

    nc = bacc.Bacc("TRN2", target_bir_lowering=False, debug=False)

    qrT = nc.dram_tensor("qrT", [Q, n], F32, kind="ExternalInput").ap()
    winT = nc.dram_tensor("winT", [128, KQ * H], F32, kind="ExternalInput").ap()
    whhT1 = nc.dram_tensor("whhT1", [128, KH * 4 * H], F32R, kind="ExternalInput").ap()
    whhT2 = nc.dram_tensor("whhT2", [128, KH * 4 * H], F32R, kind="ExternalInput").ap()
    wihT1 = nc.dram_tensor("wihT1", [A, 4 * H], F32R, kind="ExternalInput").ap()
    wihT2 = nc.dram_tensor("wihT2", [A, 4 * H], F32R, kind="ExternalInput").ap()
    woutT1 = nc.dram_tensor("woutT1", [128, KH * A], F32R, kind="ExternalInput").ap()
    woutT2 = nc.dram_tensor("woutT2", [128, KH * A], F32R, kind="ExternalInput").ap()
    b_in_c = nc.dram_tensor("b_in_c", [128, KH], F32, kind="ExternalInput").ap()
    bias_c = nc.dram_tensor("bias_c", [128, MG], F32, kind="ExternalInput").ap()
    bias0_c = nc.dram_tensor("bias0_c", [128, MG], F32, kind="ExternalInput").ap()
    b_out_c = nc.dram_tensor("b_out_c", [128, 1], F32, kind="ExternalInput").ap()
    out = nc.dram_tensor("out_logits", [c_steps, A, n], F32, kind="ExternalOutput").ap()

    with tile.TileContext(nc) as tc, ExitStack() as ctx:
        wp = ctx.enter_context(tc.tile_pool(name="weights", bufs=1))
        sp = ctx.enter_context(tc.tile_pool(name="state", bufs=2))
        cp = ctx.enter_context(tc.tile_pool(name="cell", bufs=1))
        gp = ctx.enter_context(tc.tile_pool(name="gates", bufs=1))
        tp = ctx.enter_context(tc.tile_pool(name="tmp", bufs=1))
        lp = ctx.enter_context(tc.tile_pool(name="logits", bufs=1))
        pg = ctx.enter_context(tc.tile_pool(name="pgate", bufs=6, space="PSUM"))
        pl = ctx.enter_context(tc.tile_pool(name="plog", bufs=2, space="PSUM"))

        # --- DMAs ordered by first use: h0 inputs, then hh weights (k-
        #     interleaved hi/lo to match the k-major accumulation order),
        #     then step-0 logits/argmax operands, then step-1 x operands ---
        h1 = [sp.tile([128, n], F32R, tag=f"h1{j}", name=f"h1{j}") for j in range(KH)]
        h2 = [sp.tile([128, n], F32R, tag=f"h2{j}", name=f"h2{j}") for j in range(KH)]
        ct = [cp.tile([128, n], F32, tag=f"c{j}", name=f"c{j}") for j in range(KH)]
        # setup reuses steady-state tiles: qr k-tiles live in the g0/g1 gate
        # buffers, w_in^T packs both k-tiles into the logits buffer
        win_t = lp.tile([128, n], F32, tag="logits", name="win_all")
        nc.sync.dma_start(win_t[:], winT[:, :])
        qr_t = [gp.tile([128, n], F32, tag=f"g{k}", name=f"qr{k}") for k in range(KQ)]
        b_in_t = wp.tile([128, KH], F32, tag="b_in")
        for n0 in range(nt):
            for k in range(KQ):
                nc.sync.dma_start(qr_t[k][:, n0 * 512:(n0 + 1) * 512],
                                  qrT[k * 128:(k + 1) * 128, n0 * 512:(n0 + 1) * 512])
            if n0 == 0:
                nc.sync.dma_start(b_in_t[:], b_in_c[:, :])
        bias0_t = wp.tile([128, MG], F32, tag="bias0")
        nc.sync.dma_start(bias0_t[:], bias0_c[:, :])
        whh1_all = wp.tile([128, KH * 4 * H], F32R, tag="whh1")
        nc.sync.dma_start(whh1_all[:], whhT1[:, :])
        whh2_all = wp.tile([128, KH * 4 * H], F32R, tag="whh2")
        nc.sync.dma_start(whh2_all[:], whhT2[:, :])
        whh1_t = [whh1_all[:, k * 4 * H:(k + 1) * 4 * H] for k in range(KH)]
        whh2_t = [whh2_all[:, k * 4 * H:(k + 1) * 4 * H] for k in range(KH)]
        wout1_t = wp.tile([128, KH * A], F32R, tag="wout1")
        nc.sync.dma_start(wout1_t[:], woutT1[:, :])
        wout2_t = wp.tile([128, KH * A], F32R, tag="wout2")
        nc.sync.dma_start(wout2_t[:], woutT2[:, :])
        b_out_t = wp.tile([128, 1], F32, tag="b_out")
        nc.sync.dma_start(b_out_t[:], b_out_c[:, :])
        wih1_t = wp.tile([128, 4 * H], F32R, tag="wih1")
        nc.sync.dma_start(wih1_t[:], wihT1[:, :])
        wih2_t = wp.tile([128, 4 * H], F32R, tag="wih2")
        nc.sync.dma_start(wih2_t[:], wihT2[:, :])
        bias_t = wp.tile([128, MG], F32, tag="bias")
        nc.sync.dma_start(bias_t[:], bias_c[:, :])
        for j in range(KH):
            nc.vector.memset(ct[j][:], 0.0)
            hf = tp.tile([128, n], F32, tag="t1")
            for n0 in range(nt):
                ps = pg.tile([128, 512], F32, tag="pg")
                for k in range(KQ):
                    nc.tensor.matmul(
                        ps[:],
                        lhsT=win_t[:, k * H + j * 128:k * H + (j + 1) * 128],
                        rhs=qr_t[k][:, n0 * 512:(n0 + 1) * 512],
                        start=(k == 0), stop=(k == KQ - 1),
                    )
                nc.scalar.activation(
                    hf[:, n0 * 512:(n0 + 1) * 512], ps[:],
                    AF.Identity, bias=b_in_t[:, j:j + 1],
                )
            nc.vector.tensor_copy(h1[j][:], hf[:])
            nc.vector.tensor_sub(h2[j][:], hf[:], h1[j][:].bitcast(F32))

        # collapse all setup deps so steady-state instructions carry few waits
        tc.strict_bb_all_engine_barrier()

        xt = None  # one-hot input halves [vocab, batch] f32r; step 0 folds into bias0

        for t in range(c_steps):
            h1n = [sp.tile([128, n], F32R, tag=f"h1{j}", name=f"h1{j}") for j in range(KH)]
            h2n = [sp.tile([128, n], F32R, tag=f"h2{j}", name=f"h2{j}") for j in range(KH)]
            for j in range(KH):
                g_j = [gp.tile([128, n], F32, tag=f"g{q}", name=f"g{q}") for q in range(4)]
                for q in range(4):  # i, f, g, o
                    m = q * KH + j
                    for n0 in range(nt):
                        ps = pg.tile([128, 512], F32, tag="pg")
                        for k in range(KH):
                            nc.tensor.matmul(
                                ps[:],
                                lhsT=whh1_t[k][:, m * 128:(m + 1) * 128],
                                rhs=h1[k][:, n0 * 512:(n0 + 1) * 512],
                                start=(k == 0), stop=False,
                            )
                            nc.tensor.matmul(
                                ps[:],
                                lhsT=whh2_t[k][:, m * 128:(m + 1) * 128],
                                rhs=h1[k][:, n0 * 512:(n0 + 1) * 512],
                                start=False, stop=False,
                            )
                            nc.tensor.matmul(
                                ps[:],
                                lhsT=whh1_t[k][:, m * 128:(m + 1) * 128],
                                rhs=h2[k][:, n0 * 512:(n0 + 1) * 512],
                                start=False, stop=(t == 0 and k == KH - 1),
                            )
                        if t > 0:
                            nc.tensor.matmul(
                                ps[:],
                                lhsT=wih1_t[:, m * 128:(m + 1) * 128],
                                rhs=xt[n0][:],
                                start=False, stop=False,
                            )
                            nc.tensor.matmul(
                                ps[:],
                                lhsT=wih2_t[:, m * 128:(m + 1) * 128],
                                rhs=xt[n0][:],
                                start=False, stop=True,
                            )
                        bias_ap = (bias0_t if t == 0 else bias_t)[:, m:m + 1]
                        func = AF.Tanh if q == 2 else AF.Sigmoid
                        nc.scalar.activation(
                            g_j[q][:, n0 * 512:(n0 + 1) * 512],
                            ps[:], func, bias=bias_ap,
                        )
                # cell/hidden update block j: c = f*c + i*g ; h = o*tanh(c)
                gi, gf, gg, go = (g[:] for g in g_j)
                cs = ct[j][:]
                t1 = tp.tile([128, n], F32, tag="t1")
                nc.vector.tensor_mul(t1[:], gi, gg)
                nc.vector.tensor_mul(cs, gf, cs)
                nc.vector.tensor_add(cs, cs, t1[:])
                t2 = tp.tile([128, n], F32, tag="t2")
                nc.scalar.activation(t2[:], cs, AF.Tanh)
                hf = tp.tile([128, n], F32, tag="t1")
                nc.vector.tensor_mul(hf[:], go, t2[:])
                nc.vector.tensor_copy(h1n[j][:], hf[:])
                nc.vector.tensor_sub(h2n[j][:], hf[:], h1n[j][:].bitcast(F32))

            # logits = w_out @ h + b_out  (in [vocab, batch] layout)
            lg = lp.tile([128, n], F32, tag="logits")
            for n0 in range(nt):
                ps = pl.tile([128, 512], F32, tag="pl")
                for k in range(KH):
                    nc.tensor.matmul(
                        ps[:],
                        lhsT=wout1_t[:, k * A:(k + 1) * A],
                        rhs=h1n[k][:, n0 * 512:(n0 + 1) * 512],
                        start=(k == 0), stop=False,
                    )
                for k in range(KH):
                    nc.tensor.matmul(
                        ps[:],
                        lhsT=wout2_t[:, k * A:(k + 1) * A],
                        rhs=h1n[k][:, n0 * 512:(n0 + 1) * 512],
                        start=False, stop=False,
                    )
                for k in range(KH):
                    nc.tensor.matmul(
                        ps[:],
                        lhsT=wout1_t[:, k * A:(k + 1) * A],
                        rhs=h2n[k][:, n0 * 512:(n0 + 1) * 512],
                        start=False, stop=(k == KH - 1),
                    )
                nc.scalar.activation(
                    lg[:, n0 * 512:(n0 + 1) * 512], ps[:],
                    AF.Identity, bias=b_out_t[:, 0:1],
                )
                nc.sync.dma_start(
                    out[t, :, n0 * 512:(n0 + 1) * 512],
                    lg[:, n0 * 512:(n0 + 1) * 512],
                )

            # next input: one-hot(argmax(logits)) in [vocab, batch] f32r via
            # GPSIMD cross-partition max + DVE is_ge (no PE transposes)
            if t < c_steps - 1:
                xt = [sp.tile([128, 512], F32R, tag=f"x{h}", name=f"x{h}") for h in range(nt)]
                mxb = tp.tile([128, n], F32, tag="t2")
                for n0 in range(nt):
                    nc.gpsimd.partition_all_reduce(
                        mxb[:, n0 * 512:(n0 + 1) * 512],
                        lg[:, n0 * 512:(n0 + 1) * 512],
                        channels=128, reduce_op=bass_isa.ReduceOp.max)
                    nc.vector.tensor_tensor(
                        xt[n0][:],
                        lg[:, n0 * 512:(n0 + 1) * 512],
                        mxb[:, n0 * 512:(n0 + 1) * 512],
                        ALU.is_ge)

            h1, h2 = h1n, h2n

    nc.compile()
    return nc


def make_in_maps(inputs: dict, c_steps: int = C, n: int = N_FULL, ncores: int = NCORES):
    f32 = np.float32
    qr = np.ascontiguousarray(np.asarray(inputs["quantized_repr"], f32)).reshape(BW, Q)
    w_in = np.asarray(inputs["w_in"], f32)
    b_in = np.asarray(inputs["b_in"], f32)
    w_ih = np.asarray(inputs["w_ih"], f32)
    w_hh = np.asarray(inputs["w_hh"], f32)
    b_ih = np.asarray(inputs["b_ih"], f32)
    b_hh = np.asarray(inputs["b_hh"], f32)
    w_out = np.asarray(inputs["w_out"], f32)
    b_out = np.asarray(inputs["b_out"], f32)

    bias = b_ih + b_hh                    # fp32, same as reference
    bias0 = bias + w_ih[:, 0]             # step-0 one-hot(0) contribution folded in

    whh1, whh2 = split12(w_hh.T)
    wih1, wih2 = split12(w_ih.T)
    wout1, wout2 = split12(w_out.T)

    def packk(a, ktiles):  # [ktiles*128, cols] -> [128, ktiles*cols]
        cols = a.shape[1]
        out = np.empty((128, ktiles * cols), np.float32)
        for k in range(ktiles):
            out[:, k * cols:(k + 1) * cols] = a[k * 128:(k + 1) * 128, :]
        return np.ascontiguousarray(out)

    shared = {
        "winT": packk(np.ascontiguousarray(w_in.T), KQ),
        "whhT1": packk(whh1, KH), "whhT2": packk(whh2, KH),
        "wihT1": wih1, "wihT2": wih2,
        "woutT1": packk(wout1, KH), "woutT2": packk(wout2, KH),
        "b_in_c": np.ascontiguousarray(b_in.reshape(KH, 128).T),
        "bias_c": np.ascontiguousarray(bias.reshape(MG, 128).T),
        "bias0_c": np.ascontiguousarray(bias0.reshape(MG, 128).T),
        "b_out_c": np.ascontiguousarray(b_out.reshape(128, 1)),
    }
    in_maps = []
    for i in range(ncores):
        m = dict(shared)
        m["qrT"] = np.ascontiguousarray(qr[i * n:(i + 1) * n].T)
        in_maps.append(m)
    return in_maps


def kernel(**inputs) -> np.ndarray:
    global LAST_RESULTS
    assert int(inputs["max_char_len"]) == C
    nc = build_program(C, N_FULL)
    in_maps = make_in_maps(inputs, C, N_FULL, NCORES)
    res = run_bass_kernel_spmd(nc, in_maps, core_ids=list(range(NCORES)))
    LAST_RESULTS = res
    # per-core [C, A, N] -> [N, C, A]; concat cores -> [BW, C, A] -> [B, W, C, A]
    parts = [np.transpose(r["out_logits"], (2, 0, 1)) for r in res.results]
    full = np.concatenate(parts, axis=0).reshape(B, W, C, A)
    return np.ascontiguousarray(full)


# revision 26
# speedup vs baseline: 1.4368x; 1.0040x over previous
"""CharDecoder LSTM kernel for 8 Trainium2 NeuronCores.

Data-parallel over the flattened (B*W)=8192 batch axis: each of the 8 cores
processes 1024 sequences. Small LSTM/projection weights are replicated; the
hidden/cell state stays resident in SBUF (in transposed [feature, batch]
layout) across all 20 decode steps.

Numerics: all matmuls run in float32r (TF32-like: 8-bit exponent, 11-bit
stored mantissa, 1 cycle/row for free dim >= 256) using an exact Dekker
hi/lo split of both operands:
    w = w1 + w2 (exact, host-side), h = h1 + h2 (exact, 2 DVE ops/tile)
    w @ h ~= w1@h1 + w2@h1 + w1@h2     (dropped w2@h2 term ~ 2^-24)
which is fp32-quality (verified ~1e-6 max rel err end-to-end, 0 argmax
flips) at 3 cycles/row instead of fp32's 4 cycles/row on the PE.

Per-core layout ("layout B", feature-on-partitions):
  h1/h2 splits: [512, 1024] as 4+4 SBUF f32r tiles [128, 1024]
  cT: [512, 1024] as 4 fp32 tiles, updated in place
  gatesT: [2048, 1024] computed as 16 (m) x 2 (n) PSUM tiles [128, 512]
  xT (one-hot next-char): [128 vocab, 1024] f32r, 2 half tiles [128, 512]

argmax -> one-hot per step with zero PE work: GPSIMD partition_all_reduce
(max over the 128 vocab partitions, broadcast to all partitions), then a
DVE is_ge against the logits emitting the exact 0/1 one-hot directly in
[vocab, batch] layout as f32r for the next step's x matmul.
"""

import numpy as np
from contextlib import ExitStack

import concourse.bass as bass
import concourse.bass_isa as bass_isa
import concourse.bacc as bacc
import concourse.mybir as mybir
import concourse.tile as tile
from concourse.bass_utils import run_bass_kernel_spmd

B, W, Q, H, A = 64, 128, 256, 512, 128
C = 20
NCORES = 8
BW = B * W
N_FULL = BW // NCORES  # 1024 batch rows per core

F32 = mybir.dt.float32
F32R = mybir.dt.float32r
AF = mybir.ActivationFunctionType
ALU = mybir.AluOpType

KH = H // 128       # 4 hidden k-tiles
KQ = Q // 128       # 2 input k-tiles
MG = 4 * H // 128   # 16 gate m-tiles

LAST_RESULTS = None  # BassKernelResults of the most recent run (for test.py)


def split12(v):
    """Exact Dekker split of fp32 into two 12-bit-significand (f32r) parts."""
    v = np.ascontiguousarray(np.asarray(v, np.float32))
    u = v.view(np.uint32)
    r = u + (np.uint32(0x7FF) + ((u >> np.uint32(12)) & np.uint32(1)))
    hi = (r & np.uint32(0xFFFFF000)).view(np.float32)
    lo = (v - hi).astype(np.float32)
    return hi, lo


def build_program(c_steps: int = C, n: int = N_FULL) -> bass.Bass:
    assert n % 512 == 0
    nt = n // 512    # PSUM n-tiles per row block

    nc = bacc.Bacc("TRN2", target_bir_lowering=False, debug=False)

    qrT = nc.dram_tensor("qrT", [Q, n], F32, kind="ExternalInput").ap()
    w0T = nc.dram_tensor("w0T", [128, KQ * 4 * H], F32, kind="ExternalInput").ap()
    whhT1 = nc.dram_tensor("whhT1", [128, KH * 4 * H], F32R, kind="ExternalInput").ap()
    whhT2 = nc.dram_tensor("whhT2", [128, KH * 4 * H], F32R, kind="ExternalInput").ap()
    wihT1 = nc.dram_tensor("wihT1", [A, 4 * H], F32R, kind="ExternalInput").ap()
    wihT2 = nc.dram_tensor("wihT2", [A, 4 * H], F32R, kind="ExternalInput").ap()
    woutT1 = nc.dram_tensor("woutT1", [128, KH * A], F32R, kind="ExternalInput").ap()
    woutT2 = nc.dram_tensor("woutT2", [128, KH * A], F32R, kind="ExternalInput").ap()
    bias_c = nc.dram_tensor("bias_c", [128, MG], F32, kind="ExternalInput").ap()
    bias0_c = nc.dram_tensor("bias0_c", [128, MG], F32, kind="ExternalInput").ap()
    b_out_c = nc.dram_tensor("b_out_c", [128, 1], F32, kind="ExternalInput").ap()
    out = nc.dram_tensor("out_logits", [c_steps, A, n], F32, kind="ExternalOutput").ap()

    with tile.TileContext(nc) as tc, ExitStack() as ctx:
        wp = ctx.enter_context(tc.tile_pool(name="weights", bufs=1))
        sp = ctx.enter_context(tc.tile_pool(name="state", bufs=2))
        cp = ctx.enter_context(tc.tile_pool(name="cell", bufs=1))
        gp = ctx.enter_context(tc.tile_pool(name="gates", bufs=1))
        tp = ctx.enter_context(tc.tile_pool(name="tmp", bufs=1))
        lp = ctx.enter_context(tc.tile_pool(name="logits", bufs=1))
        xp = ctx.enter_context(tc.tile_pool(name="xhot", bufs=1))
        qp = ctx.enter_context(tc.tile_pool(name="qr", bufs=1))
        pg = ctx.enter_context(tc.tile_pool(name="pgate", bufs=6, space="PSUM"))
        pl = ctx.enter_context(tc.tile_pool(name="plog", bufs=2, space="PSUM"))

        # --- DMAs ordered by first use: h0 inputs, then hh weights (k-
        #     interleaved hi/lo to match the k-major accumulation order),
        #     then step-0 logits/argmax operands, then step-1 x operands ---
        h1 = [sp.tile([128, n], F32R, tag=f"h1{j}", name=f"h1{j}") for j in range(KH)]
        h2 = [sp.tile([128, n], F32R, tag=f"h2{j}", name=f"h2{j}") for j in range(KH)]
        ct = [cp.tile([128, n], F32, tag=f"c{j}", name=f"c{j}") for j in range(KH)]
        # step-0 gates come straight from W0 = w_hh @ w_in (host-folded, so
        # h0 is never materialized); W0^T k-tiles borrow the w_ih tag
        # buffers (dead until t=1), qr gets dedicated buffers
        qr_t = [qp.tile([128, n], F32, tag=f"q{k}", name=f"qr{k}") for k in range(KQ)]
        for n0 in range(nt):
            for k in range(KQ):
                nc.sync.dma_start(qr_t[k][:, n0 * 512:(n0 + 1) * 512],
                                  qrT[k * 128:(k + 1) * 128, n0 * 512:(n0 + 1) * 512])
        w0_t = []
        for k in range(KQ):
            t = wp.tile([128, 4 * H], F32, tag=f"wih{k + 1}", name=f"w0{k}")
            nc.sync.dma_start(t[:], w0T[:, k * 4 * H:(k + 1) * 4 * H])
            w0_t.append(t)
        bias0_t = wp.tile([128, MG], F32, tag="bias0")
        nc.sync.dma_start(bias0_t[:], bias0_c[:, :])
        whh1_all = wp.tile([128, KH * 4 * H], F32R, tag="whh1")
        nc.sync.dma_start(whh1_all[:], whhT1[:, :])
        whh2_all = wp.tile([128, KH * 4 * H], F32R, tag="whh2")
        nc.sync.dma_start(whh2_all[:], whhT2[:, :])
        whh1_t = [whh1_all[:, k * 4 * H:(k + 1) * 4 * H] for k in range(KH)]
        whh2_t = [whh2_all[:, k * 4 * H:(k + 1) * 4 * H] for k in range(KH)]
        wout1_t = wp.tile([128, KH * A], F32R, tag="wout1")
        nc.sync.dma_start(wout1_t[:], woutT1[:, :])
        wout2_t = wp.tile([128, KH * A], F32R, tag="wout2")
        nc.sync.dma_start(wout2_t[:], woutT2[:, :])
        b_out_t = wp.tile([128, 1], F32, tag="b_out")
        nc.sync.dma_start(b_out_t[:], b_out_c[:, :])
        bias_t = wp.tile([128, MG], F32, tag="bias")
        nc.sync.dma_start(bias_t[:], bias_c[:, :])
        for j in range(KH):
            nc.vector.memset(ct[j][:], 0.0)

        # collapse all setup deps so steady-state instructions carry few waits
        tc.strict_bb_all_engine_barrier()

        # w_ih reuses the W0 tag buffers; issued after the barrier so its
        # wait-for-W0-reads (t=0 gates) cannot deadlock against the barrier
        wih1_t = wp.tile([128, 4 * H], F32R, tag="wih1")
        nc.sync.dma_start(wih1_t[:], wihT1[:, :])
        wih2_t = wp.tile([128, 4 * H], F32R, tag="wih2")
        nc.sync.dma_start(wih2_t[:], wihT2[:, :])

        xt = None  # one-hot input halves [vocab, batch] f32r; step 0 folds into bias0

        for t in range(c_steps):
            h1n = [sp.tile([128, n], F32R, tag=f"h1{j}", name=f"h1{j}") for j in range(KH)]
            h2n = [sp.tile([128, n], F32R, tag=f"h2{j}", name=f"h2{j}") for j in range(KH)]
            for j in range(KH):
                g_j = [gp.tile([128, n], F32, tag=f"g{q}", name=f"g{q}") for q in range(4)]
                for q in range(4):  # i, f, g, o
                    m = q * KH + j
                    for n0 in range(nt):
                        ps = pg.tile([128, 512], F32, tag="pg")
                        if t == 0:
                            for k in range(KQ):
                                nc.tensor.matmul(
                                    ps[:],
                                    lhsT=w0_t[k][:, m * 128:(m + 1) * 128],
                                    rhs=qr_t[k][:, n0 * 512:(n0 + 1) * 512],
                                    start=(k == 0), stop=(k == KQ - 1),
                                )
                        else:
                            for k in range(KH):
                                nc.tensor.matmul(
                                    ps[:],
                                    lhsT=whh1_t[k][:, m * 128:(m + 1) * 128],
                                    rhs=h1[k][:, n0 * 512:(n0 + 1) * 512],
                                    start=(k == 0), stop=False,
                                )
                                nc.tensor.matmul(
                                    ps[:],
                                    lhsT=whh2_t[k][:, m * 128:(m + 1) * 128],
                                    rhs=h1[k][:, n0 * 512:(n0 + 1) * 512],
                                    start=False, stop=False,
                                )
                                nc.tensor.matmul(
                                    ps[:],
                                    lhsT=whh1_t[k][:, m * 128:(m + 1) * 128],
                                    rhs=h2[k][:, n0 * 512:(n0 + 1) * 512],
                                    start=False, stop=False,
                                )
                        if t > 0:
                            nc.tensor.matmul(
                                ps[:],
                                lhsT=wih1_t[:, m * 128:(m + 1) * 128],
                                rhs=xt[n0][:],
                                start=False, stop=False,
                            )
                            nc.tensor.matmul(
                                ps[:],
                                lhsT=wih2_t[:, m * 128:(m + 1) * 128],
                                rhs=xt[n0][:],
                                start=False, stop=True,
                            )
                        bias_ap = (bias0_t if t == 0 else bias_t)[:, m:m + 1]
                        func = AF.Tanh if q == 2 else AF.Sigmoid
                        nc.scalar.activation(
                            g_j[q][:, n0 * 512:(n0 + 1) * 512],
                            ps[:], func, bias=bias_ap,
                        )
                # cell/hidden update block j: c = f*c + i*g ; h = o*tanh(c)
                gi, gf, gg, go = (g[:] for g in g_j)
                cs = ct[j][:]
                t1 = tp.tile([128, n], F32, tag="t1")
                nc.vector.tensor_mul(t1[:], gi, gg)
                nc.vector.tensor_mul(cs, gf, cs)
                nc.vector.tensor_add(cs, cs, t1[:])
                t2 = tp.tile([128, n], F32, tag="t2")
                nc.scalar.activation(t2[:], cs, AF.Tanh)
                hf = tp.tile([128, n], F32, tag="t1")
                nc.vector.tensor_mul(hf[:], go, t2[:])
                nc.vector.tensor_copy(h1n[j][:], hf[:])
                nc.vector.tensor_sub(h2n[j][:], hf[:], h1n[j][:].bitcast(F32))

            # logits = w_out @ h + b_out  (in [vocab, batch] layout)
            lg = lp.tile([128, n], F32, tag="logits")
            for n0 in range(nt):
                ps = pl.tile([128, 512], F32, tag="pl")
                for k in range(KH):
                    nc.tensor.matmul(
                        ps[:],
                        lhsT=wout1_t[:, k * A:(k + 1) * A],
                        rhs=h1n[k][:, n0 * 512:(n0 + 1) * 512],
                        start=(k == 0), stop=False,
                    )
                for k in range(KH):
                    nc.tensor.matmul(
                        ps[:],
                        lhsT=wout2_t[:, k * A:(k + 1) * A],
                        rhs=h1n[k][:, n0 * 512:(n0 + 1) * 512],
                        start=False, stop=False,
                    )
                for k in range(KH):
                    nc.tensor.matmul(
                        ps[:],
                        lhsT=wout1_t[:, k * A:(k + 1) * A],
                        rhs=h2n[k][:, n0 * 512:(n0 + 1) * 512],
                        start=False, stop=(k == KH - 1),
                    )
                nc.scalar.activation(
                    lg[:, n0 * 512:(n0 + 1) * 512], ps[:],
                    AF.Identity, bias=b_out_t[:, 0:1],
                )
                nc.sync.dma_start(
                    out[t, :, n0 * 512:(n0 + 1) * 512],
                    lg[:, n0 * 512:(n0 + 1) * 512],
                )

            # next input: one-hot(argmax(logits)) in [vocab, batch] f32r via
            # GPSIMD cross-partition max + DVE is_ge (no PE transposes)
            if t < c_steps - 1:
                xt = [xp.tile([128, 512], F32R, tag=f"x{h}", name=f"x{h}") for h in range(nt)]
                mxb = tp.tile([128, n], F32, tag="t2")
                for n0 in range(nt):
                    nc.gpsimd.partition_all_reduce(
                        mxb[:, n0 * 512:(n0 + 1) * 512],
                        lg[:, n0 * 512:(n0 + 1) * 512],
                        channels=128, reduce_op=bass_isa.ReduceOp.max)
                    nc.vector.tensor_tensor(
                        xt[n0][:],
                        lg[:, n0 * 512:(n0 + 1) * 512],
                        mxb[:, n0 * 512:(n0 + 1) * 512],
                        ALU.is_ge)

            h1, h2 = h1n, h2n

    nc.compile()
    return nc


def make_in_maps(inputs: dict, c_steps: int = C, n: int = N_FULL, ncores: int = NCORES):
    f32 = np.float32
    qr = np.ascontiguousarray(np.asarray(inputs["quantized_repr"], f32)).reshape(BW, Q)
    w_in = np.asarray(inputs["w_in"], f32)
    b_in = np.asarray(inputs["b_in"], f32)
    w_ih = np.asarray(inputs["w_ih"], f32)
    w_hh = np.asarray(inputs["w_hh"], f32)
    b_ih = np.asarray(inputs["b_ih"], f32)
    b_hh = np.asarray(inputs["b_hh"], f32)
    w_out = np.asarray(inputs["w_out"], f32)
    b_out = np.asarray(inputs["b_out"], f32)

    bias = b_ih + b_hh                    # fp32, same as reference
    # step-0 fold: gates0 = w_hh @ (w_in @ qr + b_in) + bias + w_ih[:, 0]
    #            = W0 @ qr + bias0   with W0, bias0 precomputed in fp64
    W0 = (w_hh.astype(np.float64) @ w_in.astype(np.float64)).astype(f32)
    bias0 = (bias.astype(np.float64) + w_ih[:, 0].astype(np.float64)
             + w_hh.astype(np.float64) @ b_in.astype(np.float64)).astype(f32)

    whh1, whh2 = split12(w_hh.T)
    wih1, wih2 = split12(w_ih.T)
    wout1, wout2 = split12(w_out.T)

    def packk(a, ktiles):  # [ktiles*128, cols] -> [128, ktiles*cols]
        cols = a.shape[1]
        out = np.empty((128, ktiles * cols), np.float32)
        for k in range(ktiles):
            out[:, k * cols:(k + 1) * cols] = a[k * 128:(k + 1) * 128, :]
        return np.ascontiguousarray(out)

    shared = {
        "w0T": packk(np.ascontiguousarray(W0.T), KQ),
        "whhT1": packk(whh1, KH), "whhT2": packk(whh2, KH),
        "wihT1": wih1, "wihT2": wih2,
        "woutT1": packk(wout1, KH), "woutT2": packk(wout2, KH),
        "bias_c": np.ascontiguousarray(bias.reshape(MG, 128).T),
        "bias0_c": np.ascontiguousarray(bias0.reshape(MG, 128).T),
        "b_out_c": np.ascontiguousarray(b_out.reshape(128, 1)),
    }
    in_maps = []
    for i in range(ncores):
        m = dict(shared)
        m["qrT"] = np.ascontiguousarray(qr[i * n:(i + 1) * n].T)
        in_maps.append(m)
    return in_maps


def kernel(**inputs) -> np.ndarray:
    global LAST_RESULTS
    assert int(inputs["max_char_len"]) == C
    nc = build_program(C, N_FULL)
    in_maps = make_in_maps(inputs, C, N_FULL, NCORES)
    res = run_bass_kernel_spmd(nc, in_maps, core_ids=list(range(NCORES)))
    LAST_RESULTS = res
    # per-core [C, A, N] -> [N, C, A]; concat cores -> [BW, C, A] -> [B, W, C, A]
    parts = [np.transpose(r["out_logits"], (2, 0, 1)) for r in res.results]
    full = np.concatenate(parts, axis=0).reshape(B, W, C, A)
    return np.ascontiguousarray(full)


# revision 27
# speedup vs baseline: 1.4627x; 1.0180x over previous
"""CharDecoder LSTM kernel for 8 Trainium2 NeuronCores.

Data-parallel over the flattened (B*W)=8192 batch axis: each of the 8 cores
processes 1024 sequences. Small LSTM/projection weights are replicated; the
hidden/cell state stays resident in SBUF (in transposed [feature, batch]
layout) across all 20 decode steps.

Numerics: all matmuls run in float32r (TF32-like: 8-bit exponent, 11-bit
stored mantissa, 1 cycle/row for free dim >= 256) using an exact Dekker
hi/lo split of both operands:
    w = w1 + w2 (exact, host-side), h = h1 + h2 (exact, 2 DVE ops/tile)
    w @ h ~= w1@h1 + w2@h1 + w1@h2     (dropped w2@h2 term ~ 2^-24)
which is fp32-quality (verified ~1e-6 max rel err end-to-end, 0 argmax
flips) at 3 cycles/row instead of fp32's 4 cycles/row on the PE.

Per-core layout ("layout B", feature-on-partitions):
  h1/h2 splits: [512, 1024] as 4+4 SBUF f32r tiles [128, 1024]
  cT: [512, 1024] as 4 fp32 tiles, updated in place
  gatesT: [2048, 1024] computed as 16 (m) x 2 (n) PSUM tiles [128, 512]
  xT (one-hot next-char): [128 vocab, 1024] f32r, 2 half tiles [128, 512]

argmax -> one-hot per step with zero PE work: GPSIMD partition_all_reduce
(max over the 128 vocab partitions, broadcast to all partitions), then a
DVE is_ge against the logits emitting the exact 0/1 one-hot directly in
[vocab, batch] layout as f32r for the next step's x matmul.
"""

import numpy as np
from contextlib import ExitStack

import concourse.bass as bass
import concourse.bass_isa as bass_isa
import concourse.bacc as bacc
import concourse.mybir as mybir
import concourse.tile as tile
from concourse.bass_utils import run_bass_kernel_spmd

B, W, Q, H, A = 64, 128, 256, 512, 128
C = 20
NCORES = 8
BW = B * W
N_FULL = BW // NCORES  # 1024 batch rows per core

F32 = mybir.dt.float32
F32R = mybir.dt.float32r
AF = mybir.ActivationFunctionType
ALU = mybir.AluOpType

KH = H // 128       # 4 hidden k-tiles
KQ = Q // 128       # 2 input k-tiles
MG = 4 * H // 128   # 16 gate m-tiles

LAST_RESULTS = None  # BassKernelResults of the most recent run (for test.py)


def split12(v):
    """Exact Dekker split of fp32 into two 12-bit-significand (f32r) parts."""
    v = np.ascontiguousarray(np.asarray(v, np.float32))
    u = v.view(np.uint32)
    r = u + (np.uint32(0x7FF) + ((u >> np.uint32(12)) & np.uint32(1)))
    hi = (r & np.uint32(0xFFFFF000)).view(np.float32)
    lo = (v - hi).astype(np.float32)
    return hi, lo


def build_program(c_steps: int = C, n: int = N_FULL) -> bass.Bass:
    assert n % 512 == 0
    nt = n // 512    # PSUM n-tiles per row block

    nc = bacc.Bacc("TRN2", target_bir_lowering=False, debug=False)

    qrT = nc.dram_tensor("qrT", [Q, n], F32, kind="ExternalInput").ap()
    w0T = nc.dram_tensor("w0T", [128, KQ * 4 * H], F32, kind="ExternalInput").ap()
    whhT1 = nc.dram_tensor("whhT1", [128, KH * 4 * H], F32R, kind="ExternalInput").ap()
    whhT2 = nc.dram_tensor("whhT2", [128, KH * 4 * H], F32R, kind="ExternalInput").ap()
    wihT1 = nc.dram_tensor("wihT1", [A, 4 * H], F32R, kind="ExternalInput").ap()
    wihT2 = nc.dram_tensor("wihT2", [A, 4 * H], F32R, kind="ExternalInput").ap()
    woutT1 = nc.dram_tensor("woutT1", [128, KH * A], F32R, kind="ExternalInput").ap()
    woutT2 = nc.dram_tensor("woutT2", [128, KH * A], F32R, kind="ExternalInput").ap()
    bias_c = nc.dram_tensor("bias_c", [128, MG], F32, kind="ExternalInput").ap()
    bias0_c = nc.dram_tensor("bias0_c", [128, MG], F32, kind="ExternalInput").ap()
    b_out_c = nc.dram_tensor("b_out_c", [128, 1], F32, kind="ExternalInput").ap()
    out = nc.dram_tensor("out_logits", [c_steps, A, n], F32, kind="ExternalOutput").ap()

    with tile.TileContext(nc) as tc, ExitStack() as ctx:
        wp = ctx.enter_context(tc.tile_pool(name="weights", bufs=1))
        sp = ctx.enter_context(tc.tile_pool(name="state", bufs=2))
        cp = ctx.enter_context(tc.tile_pool(name="cell", bufs=1))
        gp = ctx.enter_context(tc.tile_pool(name="gates", bufs=1))
        tp = ctx.enter_context(tc.tile_pool(name="tmp", bufs=1))
        lp = ctx.enter_context(tc.tile_pool(name="logits", bufs=1))
        xp = ctx.enter_context(tc.tile_pool(name="xhot", bufs=1))
        qp = ctx.enter_context(tc.tile_pool(name="qr", bufs=1))
        pg = ctx.enter_context(tc.tile_pool(name="pgate", bufs=6, space="PSUM"))
        pl = ctx.enter_context(tc.tile_pool(name="plog", bufs=2, space="PSUM"))

        # --- DMAs ordered by first use: h0 inputs, then hh weights (k-
        #     interleaved hi/lo to match the k-major accumulation order),
        #     then step-0 logits/argmax operands, then step-1 x operands ---
        h1 = [sp.tile([128, n], F32R, tag=f"h1{j}", name=f"h1{j}") for j in range(KH)]
        h2 = [sp.tile([128, n], F32R, tag=f"h2{j}", name=f"h2{j}") for j in range(KH)]
        ct = [cp.tile([128, n], F32, tag=f"c{j}", name=f"c{j}") for j in range(KH)]
        # step-0 gates come straight from W0 = w_hh @ w_in (host-folded, so
        # h0 is never materialized); W0^T k-tiles borrow the w_ih tag
        # buffers (dead until t=1), qr gets dedicated buffers
        qr_t = [qp.tile([128, n], F32, tag=f"q{k}", name=f"qr{k}") for k in range(KQ)]
        for n0 in range(nt):
            for k in range(KQ):
                nc.sync.dma_start(qr_t[k][:, n0 * 512:(n0 + 1) * 512],
                                  qrT[k * 128:(k + 1) * 128, n0 * 512:(n0 + 1) * 512])
        w0_t = []
        for k in range(KQ):
            t = wp.tile([128, 4 * H], F32, tag=f"wih{k + 1}", name=f"w0{k}")
            nc.sync.dma_start(t[:], w0T[:, k * 4 * H:(k + 1) * 4 * H])
            w0_t.append(t)
        bias0_t = wp.tile([128, MG], F32, tag="bias0")
        nc.sync.dma_start(bias0_t[:], bias0_c[:, :])
        whh1_all = wp.tile([128, KH * 4 * H], F32R, tag="whh1")
        nc.sync.dma_start(whh1_all[:], whhT1[:, :])
        whh2_all = wp.tile([128, KH * 4 * H], F32R, tag="whh2")
        nc.sync.dma_start(whh2_all[:], whhT2[:, :])
        whh1_t = [whh1_all[:, k * 4 * H:(k + 1) * 4 * H] for k in range(KH)]
        whh2_t = [whh2_all[:, k * 4 * H:(k + 1) * 4 * H] for k in range(KH)]
        wout1_t = wp.tile([128, KH * A], F32R, tag="wout1")
        nc.sync.dma_start(wout1_t[:], woutT1[:, :])
        wout2_t = wp.tile([128, KH * A], F32R, tag="wout2")
        nc.sync.dma_start(wout2_t[:], woutT2[:, :])
        b_out_t = wp.tile([128, 1], F32, tag="b_out")
        nc.sync.dma_start(b_out_t[:], b_out_c[:, :])
        bias_t = wp.tile([128, MG], F32, tag="bias")
        nc.sync.dma_start(bias_t[:], bias_c[:, :])
        for j in range(KH):
            nc.vector.memset(ct[j][:], 0.0)

        # w_ih reuses the W0 tag buffers; issued after the barrier so its
        # wait-for-W0-reads (t=0 gates) cannot deadlock against the barrier
        wih1_t = wp.tile([128, 4 * H], F32R, tag="wih1")
        nc.sync.dma_start(wih1_t[:], wihT1[:, :])
        wih2_t = wp.tile([128, 4 * H], F32R, tag="wih2")
        nc.sync.dma_start(wih2_t[:], wihT2[:, :])

        xt = None  # one-hot input halves [vocab, batch] f32r; step 0 folds into bias0

        for t in range(c_steps):
            h1n = [sp.tile([128, n], F32R, tag=f"h1{j}", name=f"h1{j}") for j in range(KH)]
            h2n = [sp.tile([128, n], F32R, tag=f"h2{j}", name=f"h2{j}") for j in range(KH)]
            for j in range(KH):
                g_j = [gp.tile([128, n], F32, tag=f"g{q}", name=f"g{q}") for q in range(4)]
                for q in range(4):  # i, f, g, o
                    m = q * KH + j
                    for n0 in range(nt):
                        ps = pg.tile([128, 512], F32, tag="pg")
                        if t == 0:
                            for k in range(KQ):
                                nc.tensor.matmul(
                                    ps[:],
                                    lhsT=w0_t[k][:, m * 128:(m + 1) * 128],
                                    rhs=qr_t[k][:, n0 * 512:(n0 + 1) * 512],
                                    start=(k == 0), stop=(k == KQ - 1),
                                )
                        else:
                            for k in range(KH):
                                nc.tensor.matmul(
                                    ps[:],
                                    lhsT=whh1_t[k][:, m * 128:(m + 1) * 128],
                                    rhs=h1[k][:, n0 * 512:(n0 + 1) * 512],
                                    start=(k == 0), stop=False,
                                )
                                nc.tensor.matmul(
                                    ps[:],
                                    lhsT=whh2_t[k][:, m * 128:(m + 1) * 128],
                                    rhs=h1[k][:, n0 * 512:(n0 + 1) * 512],
                                    start=False, stop=False,
                                )
                                nc.tensor.matmul(
                                    ps[:],
                                    lhsT=whh1_t[k][:, m * 128:(m + 1) * 128],
                                    rhs=h2[k][:, n0 * 512:(n0 + 1) * 512],
                                    start=False, stop=False,
                                )
                        if t > 0:
                            nc.tensor.matmul(
                                ps[:],
                                lhsT=wih1_t[:, m * 128:(m + 1) * 128],
                                rhs=xt[n0][:],
                                start=False, stop=False,
                            )
                            nc.tensor.matmul(
                                ps[:],
                                lhsT=wih2_t[:, m * 128:(m + 1) * 128],
                                rhs=xt[n0][:],
                                start=False, stop=True,
                            )
                        bias_ap = (bias0_t if t == 0 else bias_t)[:, m:m + 1]
                        func = AF.Tanh if q == 2 else AF.Sigmoid
                        nc.scalar.activation(
                            g_j[q][:, n0 * 512:(n0 + 1) * 512],
                            ps[:], func, bias=bias_ap,
                        )
                # cell/hidden update block j: c = f*c + i*g ; h = o*tanh(c)
                gi, gf, gg, go = (g[:] for g in g_j)
                cs = ct[j][:]
                t1 = tp.tile([128, n], F32, tag="t1")
                nc.vector.tensor_mul(t1[:], gi, gg)
                nc.vector.tensor_mul(cs, gf, cs)
                nc.vector.tensor_add(cs, cs, t1[:])
                t2 = tp.tile([128, n], F32, tag="t2")
                nc.scalar.activation(t2[:], cs, AF.Tanh)
                hf = tp.tile([128, n], F32, tag="t1")
                nc.vector.tensor_mul(hf[:], go, t2[:])
                nc.vector.tensor_copy(h1n[j][:], hf[:])
                nc.vector.tensor_sub(h2n[j][:], hf[:], h1n[j][:].bitcast(F32))

            # logits = w_out @ h + b_out  (in [vocab, batch] layout)
            lg = lp.tile([128, n], F32, tag="logits")
            for n0 in range(nt):
                ps = pl.tile([128, 512], F32, tag="pl")
                for k in range(KH):
                    nc.tensor.matmul(
                        ps[:],
                        lhsT=wout1_t[:, k * A:(k + 1) * A],
                        rhs=h1n[k][:, n0 * 512:(n0 + 1) * 512],
                        start=(k == 0), stop=False,
                    )
                for k in range(KH):
                    nc.tensor.matmul(
                        ps[:],
                        lhsT=wout2_t[:, k * A:(k + 1) * A],
                        rhs=h1n[k][:, n0 * 512:(n0 + 1) * 512],
                        start=False, stop=False,
                    )
                for k in range(KH):
                    nc.tensor.matmul(
                        ps[:],
                        lhsT=wout1_t[:, k * A:(k + 1) * A],
                        rhs=h2n[k][:, n0 * 512:(n0 + 1) * 512],
                        start=False, stop=(k == KH - 1),
                    )
                nc.scalar.activation(
                    lg[:, n0 * 512:(n0 + 1) * 512], ps[:],
                    AF.Identity, bias=b_out_t[:, 0:1],
                )
                nc.sync.dma_start(
                    out[t, :, n0 * 512:(n0 + 1) * 512],
                    lg[:, n0 * 512:(n0 + 1) * 512],
                )

            # next input: one-hot(argmax(logits)) in [vocab, batch] f32r via
            # GPSIMD cross-partition max + DVE is_ge (no PE transposes)
            if t < c_steps - 1:
                xt = [xp.tile([128, 512], F32R, tag=f"x{h}", name=f"x{h}") for h in range(nt)]
                mxb = tp.tile([128, n], F32, tag="t2")
                for n0 in range(nt):
                    nc.gpsimd.partition_all_reduce(
                        mxb[:, n0 * 512:(n0 + 1) * 512],
                        lg[:, n0 * 512:(n0 + 1) * 512],
                        channels=128, reduce_op=bass_isa.ReduceOp.max)
                    nc.vector.tensor_tensor(
                        xt[n0][:],
                        lg[:, n0 * 512:(n0 + 1) * 512],
                        mxb[:, n0 * 512:(n0 + 1) * 512],
                        ALU.is_ge)

            h1, h2 = h1n, h2n

    nc.compile()
    return nc


def make_in_maps(inputs: dict, c_steps: int = C, n: int = N_FULL, ncores: int = NCORES):
    f32 = np.float32
    qr = np.ascontiguousarray(np.asarray(inputs["quantized_repr"], f32)).reshape(BW, Q)
    w_in = np.asarray(inputs["w_in"], f32)
    b_in = np.asarray(inputs["b_in"], f32)
    w_ih = np.asarray(inputs["w_ih"], f32)
    w_hh = np.asarray(inputs["w_hh"], f32)
    b_ih = np.asarray(inputs["b_ih"], f32)
    b_hh = np.asarray(inputs["b_hh"], f32)
    w_out = np.asarray(inputs["w_out"], f32)
    b_out = np.asarray(inputs["b_out"], f32)

    bias = b_ih + b_hh                    # fp32, same as reference
    # step-0 fold: gates0 = w_hh @ (w_in @ qr + b_in) + bias + w_ih[:, 0]
    #            = W0 @ qr + bias0   with W0, bias0 precomputed in fp64
    W0 = (w_hh.astype(np.float64) @ w_in.astype(np.float64)).astype(f32)
    bias0 = (bias.astype(np.float64) + w_ih[:, 0].astype(np.float64)
             + w_hh.astype(np.float64) @ b_in.astype(np.float64)).astype(f32)

    whh1, whh2 = split12(w_hh.T)
    wih1, wih2 = split12(w_ih.T)
    wout1, wout2 = split12(w_out.T)

    def packk(a, ktiles):  # [ktiles*128, cols] -> [128, ktiles*cols]
        cols = a.shape[1]
        out = np.empty((128, ktiles * cols), np.float32)
        for k in range(ktiles):
            out[:, k * cols:(k + 1) * cols] = a[k * 128:(k + 1) * 128, :]
        return np.ascontiguousarray(out)

    shared = {
        "w0T": packk(np.ascontiguousarray(W0.T), KQ),
        "whhT1": packk(whh1, KH), "whhT2": packk(whh2, KH),
        "wihT1": wih1, "wihT2": wih2,
        "woutT1": packk(wout1, KH), "woutT2": packk(wout2, KH),
        "bias_c": np.ascontiguousarray(bias.reshape(MG, 128).T),
        "bias0_c": np.ascontiguousarray(bias0.reshape(MG, 128).T),
        "b_out_c": np.ascontiguousarray(b_out.reshape(128, 1)),
    }
    in_maps = []
    for i in range(ncores):
        m = dict(shared)
        m["qrT"] = np.ascontiguousarray(qr[i * n:(i + 1) * n].T)
        in_maps.append(m)
    return in_maps


def kernel(**inputs) -> np.ndarray:
    global LAST_RESULTS
    assert int(inputs["max_char_len"]) == C
    nc = build_program(C, N_FULL)
    in_maps = make_in_maps(inputs, C, N_FULL, NCORES)
    res = run_bass_kernel_spmd(nc, in_maps, core_ids=list(range(NCORES)))
    LAST_RESULTS = res
    # per-core [C, A, N] -> [N, C, A]; concat cores -> [BW, C, A] -> [B, W, C, A]
    parts = [np.transpose(r["out_logits"], (2, 0, 1)) for r in res.results]
    full = np.concatenate(parts, axis=0).reshape(B, W, C, A)
    return np.ascontiguousarray(full)


# revision 28
# speedup vs baseline: 1.4715x; 1.0061x over previous
"""CharDecoder LSTM kernel for 8 Trainium2 NeuronCores.

Data-parallel over the flattened (B*W)=8192 batch axis: each of the 8 cores
processes 1024 sequences. Small LSTM/projection weights are replicated; the
hidden/cell state stays resident in SBUF (in transposed [feature, batch]
layout) across all 20 decode steps.

Numerics: all matmuls run in float32r (TF32-like: 8-bit exponent, 11-bit
stored mantissa, 1 cycle/row for free dim >= 256) using an exact Dekker
hi/lo split of both operands:
    w = w1 + w2 (exact, host-side), h = h1 + h2 (exact, 2 DVE ops/tile)
    w @ h ~= w1@h1 + w2@h1 + w1@h2     (dropped w2@h2 term ~ 2^-24)
which is fp32-quality (verified ~1e-6 max rel err end-to-end, 0 argmax
flips) at 3 cycles/row instead of fp32's 4 cycles/row on the PE.

Per-core layout ("layout B", feature-on-partitions):
  h1/h2 splits: [512, 1024] as 4+4 SBUF f32r tiles [128, 1024]
  cT: [512, 1024] as 4 fp32 tiles, updated in place
  gatesT: [2048, 1024] computed as 16 (m) x 2 (n) PSUM tiles [128, 512]
  xT (one-hot next-char): [128 vocab, 1024] f32r, 2 half tiles [128, 512]

argmax -> one-hot per step with zero PE work: GPSIMD partition_all_reduce
(max over the 128 vocab partitions, broadcast to all partitions), then a
DVE is_ge against the logits emitting the exact 0/1 one-hot directly in
[vocab, batch] layout as f32r for the next step's x matmul.
"""

import numpy as np
from contextlib import ExitStack

import concourse.bass as bass
import concourse.bass_isa as bass_isa
import concourse.bacc as bacc
import concourse.mybir as mybir
import concourse.tile as tile
from concourse.bass_utils import run_bass_kernel_spmd

B, W, Q, H, A = 64, 128, 256, 512, 128
C = 20
NCORES = 8
BW = B * W
N_FULL = BW // NCORES  # 1024 batch rows per core

F32 = mybir.dt.float32
F32R = mybir.dt.float32r
AF = mybir.ActivationFunctionType
ALU = mybir.AluOpType

KH = H // 128       # 4 hidden k-tiles
KQ = Q // 128       # 2 input k-tiles
MG = 4 * H // 128   # 16 gate m-tiles

LAST_RESULTS = None  # BassKernelResults of the most recent run (for test.py)


def split12(v):
    """Exact Dekker split of fp32 into two 12-bit-significand (f32r) parts."""
    v = np.ascontiguousarray(np.asarray(v, np.float32))
    u = v.view(np.uint32)
    r = u + (np.uint32(0x7FF) + ((u >> np.uint32(12)) & np.uint32(1)))
    hi = (r & np.uint32(0xFFFFF000)).view(np.float32)
    lo = (v - hi).astype(np.float32)
    return hi, lo


def build_program(c_steps: int = C, n: int = N_FULL) -> bass.Bass:
    assert n % 512 == 0
    nt = n // 512    # PSUM n-tiles per row block

    nc = bacc.Bacc("TRN2", target_bir_lowering=False, debug=False)

    qrT = nc.dram_tensor("qrT", [Q, n], F32, kind="ExternalInput").ap()
    w0T = nc.dram_tensor("w0T", [128, KQ * 4 * H], F32, kind="ExternalInput").ap()
    whhT1 = nc.dram_tensor("whhT1", [128, KH * 4 * H], F32R, kind="ExternalInput").ap()
    whhT2 = nc.dram_tensor("whhT2", [128, KH * 4 * H], F32R, kind="ExternalInput").ap()
    wihT1 = nc.dram_tensor("wihT1", [A, 4 * H], F32R, kind="ExternalInput").ap()
    wihT2 = nc.dram_tensor("wihT2", [A, 4 * H], F32R, kind="ExternalInput").ap()
    woutT1 = nc.dram_tensor("woutT1", [128, KH * A], F32R, kind="ExternalInput").ap()
    woutT2 = nc.dram_tensor("woutT2", [128, KH * A], F32R, kind="ExternalInput").ap()
    bias_c = nc.dram_tensor("bias_c", [128, MG], F32, kind="ExternalInput").ap()
    bias0_c = nc.dram_tensor("bias0_c", [128, MG], F32, kind="ExternalInput").ap()
    b_out_c = nc.dram_tensor("b_out_c", [128, 1], F32, kind="ExternalInput").ap()
    out = nc.dram_tensor("out_logits", [c_steps, A, n], F32, kind="ExternalOutput").ap()

    with tile.TileContext(nc) as tc, ExitStack() as ctx:
        wp = ctx.enter_context(tc.tile_pool(name="weights", bufs=1))
        sp = ctx.enter_context(tc.tile_pool(name="state", bufs=2))
        cp = ctx.enter_context(tc.tile_pool(name="cell", bufs=1))
        gp = ctx.enter_context(tc.tile_pool(name="gates", bufs=1))
        tp = ctx.enter_context(tc.tile_pool(name="tmp", bufs=1))
        lp = ctx.enter_context(tc.tile_pool(name="logits", bufs=1))
        xp = ctx.enter_context(tc.tile_pool(name="xhot", bufs=1))
        qp = ctx.enter_context(tc.tile_pool(name="qr", bufs=1))
        pg = ctx.enter_context(tc.tile_pool(name="pgate", bufs=6, space="PSUM"))
        pl = ctx.enter_context(tc.tile_pool(name="plog", bufs=2, space="PSUM"))

        # --- DMAs ordered by first use: h0 inputs, then hh weights (k-
        #     interleaved hi/lo to match the k-major accumulation order),
        #     then step-0 logits/argmax operands, then step-1 x operands ---
        h1 = [sp.tile([128, n], F32R, tag=f"h1{j}", name=f"h1{j}") for j in range(KH)]
        h2 = [sp.tile([128, n], F32R, tag=f"h2{j}", name=f"h2{j}") for j in range(KH)]
        ct = [cp.tile([128, n], F32, tag=f"c{j}", name=f"c{j}") for j in range(KH)]
        # step-0 gates come straight from W0 = w_hh @ w_in (host-folded, so
        # h0 is never materialized); W0^T k-tiles borrow the w_ih tag
        # buffers (dead until t=1), qr gets dedicated buffers
        qr_t = [qp.tile([128, n], F32, tag=f"q{k}", name=f"qr{k}") for k in range(KQ)]
        w0_t = [wp.tile([128, 4 * H], F32, tag=f"wih{k + 1}", name=f"w0{k}")
                for k in range(KQ)]
        bias0_t = wp.tile([128, MG], F32, tag="bias0")
        nc.sync.dma_start(bias0_t[:], bias0_c[:, :])
        for k in range(KQ):
            for n0 in range(nt):
                nc.sync.dma_start(qr_t[k][:, n0 * 512:(n0 + 1) * 512],
                                  qrT[k * 128:(k + 1) * 128, n0 * 512:(n0 + 1) * 512])
            nc.sync.dma_start(w0_t[k][:], w0T[:, k * 4 * H:(k + 1) * 4 * H])
        whh1_all = wp.tile([128, KH * 4 * H], F32R, tag="whh1")
        nc.sync.dma_start(whh1_all[:], whhT1[:, :])
        whh2_all = wp.tile([128, KH * 4 * H], F32R, tag="whh2")
        nc.sync.dma_start(whh2_all[:], whhT2[:, :])
        whh1_t = [whh1_all[:, k * 4 * H:(k + 1) * 4 * H] for k in range(KH)]
        whh2_t = [whh2_all[:, k * 4 * H:(k + 1) * 4 * H] for k in range(KH)]
        wout1_t = wp.tile([128, KH * A], F32R, tag="wout1")
        nc.sync.dma_start(wout1_t[:], woutT1[:, :])
        wout2_t = wp.tile([128, KH * A], F32R, tag="wout2")
        nc.sync.dma_start(wout2_t[:], woutT2[:, :])
        b_out_t = wp.tile([128, 1], F32, tag="b_out")
        nc.sync.dma_start(b_out_t[:], b_out_c[:, :])
        bias_t = wp.tile([128, MG], F32, tag="bias")
        nc.sync.dma_start(bias_t[:], bias_c[:, :])
        for j in range(KH):
            nc.vector.memset(ct[j][:], 0.0)

        # w_ih reuses the W0 tag buffers; issued after the barrier so its
        # wait-for-W0-reads (t=0 gates) cannot deadlock against the barrier
        wih1_t = wp.tile([128, 4 * H], F32R, tag="wih1")
        nc.sync.dma_start(wih1_t[:], wihT1[:, :])
        wih2_t = wp.tile([128, 4 * H], F32R, tag="wih2")
        nc.sync.dma_start(wih2_t[:], wihT2[:, :])

        xt = None  # one-hot input halves [vocab, batch] f32r; step 0 folds into bias0

        for t in range(c_steps):
            h1n = [sp.tile([128, n], F32R, tag=f"h1{j}", name=f"h1{j}") for j in range(KH)]
            h2n = [sp.tile([128, n], F32R, tag=f"h2{j}", name=f"h2{j}") for j in range(KH)]
            for j in range(KH):
                g_j = [gp.tile([128, n], F32, tag=f"g{q}", name=f"g{q}") for q in range(4)]
                for q in range(4):  # i, f, g, o
                    m = q * KH + j
                    for n0 in range(nt):
                        ps = pg.tile([128, 512], F32, tag="pg")
                        if t == 0:
                            for k in range(KQ):
                                nc.tensor.matmul(
                                    ps[:],
                                    lhsT=w0_t[k][:, m * 128:(m + 1) * 128],
                                    rhs=qr_t[k][:, n0 * 512:(n0 + 1) * 512],
                                    start=(k == 0), stop=(k == KQ - 1),
                                )
                        else:
                            for k in range(KH):
                                nc.tensor.matmul(
                                    ps[:],
                                    lhsT=whh1_t[k][:, m * 128:(m + 1) * 128],
                                    rhs=h1[k][:, n0 * 512:(n0 + 1) * 512],
                                    start=(k == 0), stop=False,
                                )
                                nc.tensor.matmul(
                                    ps[:],
                                    lhsT=whh2_t[k][:, m * 128:(m + 1) * 128],
                                    rhs=h1[k][:, n0 * 512:(n0 + 1) * 512],
                                    start=False, stop=False,
                                )
                                nc.tensor.matmul(
                                    ps[:],
                                    lhsT=whh1_t[k][:, m * 128:(m + 1) * 128],
                                    rhs=h2[k][:, n0 * 512:(n0 + 1) * 512],
                                    start=False, stop=False,
                                )
                        if t > 0:
                            nc.tensor.matmul(
                                ps[:],
                                lhsT=wih1_t[:, m * 128:(m + 1) * 128],
                                rhs=xt[n0][:],
                                start=False, stop=False,
                            )
                            nc.tensor.matmul(
                                ps[:],
                                lhsT=wih2_t[:, m * 128:(m + 1) * 128],
                                rhs=xt[n0][:],
                                start=False, stop=True,
                            )
                        bias_ap = (bias0_t if t == 0 else bias_t)[:, m:m + 1]
                        func = AF.Tanh if q == 2 else AF.Sigmoid
                        nc.scalar.activation(
                            g_j[q][:, n0 * 512:(n0 + 1) * 512],
                            ps[:], func, bias=bias_ap,
                        )
                # cell/hidden update block j: c = f*c + i*g ; h = o*tanh(c)
                gi, gf, gg, go = (g[:] for g in g_j)
                cs = ct[j][:]
                t1 = tp.tile([128, n], F32, tag="t1")
                nc.vector.tensor_mul(t1[:], gi, gg)
                nc.vector.tensor_mul(cs, gf, cs)
                nc.vector.tensor_add(cs, cs, t1[:])
                t2 = tp.tile([128, n], F32, tag="t2")
                nc.scalar.activation(t2[:], cs, AF.Tanh)
                hf = tp.tile([128, n], F32, tag="t1")
                nc.vector.tensor_mul(hf[:], go, t2[:])
                nc.vector.tensor_copy(h1n[j][:], hf[:])
                nc.vector.tensor_sub(h2n[j][:], hf[:], h1n[j][:].bitcast(F32))

            # logits = w_out @ h + b_out  (in [vocab, batch] layout)
            lg = lp.tile([128, n], F32, tag="logits")
            for n0 in range(nt):
                ps = pl.tile([128, 512], F32, tag="pl")
                for k in range(KH):
                    nc.tensor.matmul(
                        ps[:],
                        lhsT=wout1_t[:, k * A:(k + 1) * A],
                        rhs=h1n[k][:, n0 * 512:(n0 + 1) * 512],
                        start=(k == 0), stop=False,
                    )
                for k in range(KH):
                    nc.tensor.matmul(
                        ps[:],
                        lhsT=wout2_t[:, k * A:(k + 1) * A],
                        rhs=h1n[k][:, n0 * 512:(n0 + 1) * 512],
                        start=False, stop=False,
                    )
                for k in range(KH):
                    nc.tensor.matmul(
                        ps[:],
                        lhsT=wout1_t[:, k * A:(k + 1) * A],
                        rhs=h2n[k][:, n0 * 512:(n0 + 1) * 512],
                        start=False, stop=(k == KH - 1),
                    )
                nc.scalar.activation(
                    lg[:, n0 * 512:(n0 + 1) * 512], ps[:],
                    AF.Identity, bias=b_out_t[:, 0:1],
                )
                nc.sync.dma_start(
                    out[t, :, n0 * 512:(n0 + 1) * 512],
                    lg[:, n0 * 512:(n0 + 1) * 512],
                )

            # next input: one-hot(argmax(logits)) in [vocab, batch] f32r via
            # GPSIMD cross-partition max + DVE is_ge (no PE transposes)
            if t < c_steps - 1:
                xt = [xp.tile([128, 512], F32R, tag=f"x{h}", name=f"x{h}") for h in range(nt)]
                mxb = tp.tile([128, n], F32, tag="t2")
                for n0 in range(nt):
                    nc.gpsimd.partition_all_reduce(
                        mxb[:, n0 * 512:(n0 + 1) * 512],
                        lg[:, n0 * 512:(n0 + 1) * 512],
                        channels=128, reduce_op=bass_isa.ReduceOp.max)
                    nc.vector.tensor_tensor(
                        xt[n0][:],
                        lg[:, n0 * 512:(n0 + 1) * 512],
                        mxb[:, n0 * 512:(n0 + 1) * 512],
                        ALU.is_ge)

            h1, h2 = h1n, h2n

    nc.compile()
    return nc


def make_in_maps(inputs: dict, c_steps: int = C, n: int = N_FULL, ncores: int = NCORES):
    f32 = np.float32
    qr = np.ascontiguousarray(np.asarray(inputs["quantized_repr"], f32)).reshape(BW, Q)
    w_in = np.asarray(inputs["w_in"], f32)
    b_in = np.asarray(inputs["b_in"], f32)
    w_ih = np.asarray(inputs["w_ih"], f32)
    w_hh = np.asarray(inputs["w_hh"], f32)
    b_ih = np.asarray(inputs["b_ih"], f32)
    b_hh = np.asarray(inputs["b_hh"], f32)
    w_out = np.asarray(inputs["w_out"], f32)
    b_out = np.asarray(inputs["b_out"], f32)

    bias = b_ih + b_hh                    # fp32, same as reference
    # step-0 fold: gates0 = w_hh @ (w_in @ qr + b_in) + bias + w_ih[:, 0]
    #            = W0 @ qr + bias0   with W0, bias0 precomputed in fp64
    W0 = (w_hh.astype(np.float64) @ w_in.astype(np.float64)).astype(f32)
    bias0 = (bias.astype(np.float64) + w_ih[:, 0].astype(np.float64)
             + w_hh.astype(np.float64) @ b_in.astype(np.float64)).astype(f32)

    whh1, whh2 = split12(w_hh.T)
    wih1, wih2 = split12(w_ih.T)
    wout1, wout2 = split12(w_out.T)

    def packk(a, ktiles):  # [ktiles*128, cols] -> [128, ktiles*cols]
        cols = a.shape[1]
        out = np.empty((128, ktiles * cols), np.float32)
        for k in range(ktiles):
            out[:, k * cols:(k + 1) * cols] = a[k * 128:(k + 1) * 128, :]
        return np.ascontiguousarray(out)

    shared = {
        "w0T": packk(np.ascontiguousarray(W0.T), KQ),
        "whhT1": packk(whh1, KH), "whhT2": packk(whh2, KH),
        "wihT1": wih1, "wihT2": wih2,
        "woutT1": packk(wout1, KH), "woutT2": packk(wout2, KH),
        "bias_c": np.ascontiguousarray(bias.reshape(MG, 128).T),
        "bias0_c": np.ascontiguousarray(bias0.reshape(MG, 128).T),
        "b_out_c": np.ascontiguousarray(b_out.reshape(128, 1)),
    }
    in_maps = []
    for i in range(ncores):
        m = dict(shared)
        m["qrT"] = np.ascontiguousarray(qr[i * n:(i + 1) * n].T)
        in_maps.append(m)
    return in_maps


def kernel(**inputs) -> np.ndarray:
    global LAST_RESULTS
    assert int(inputs["max_char_len"]) == C
    nc = build_program(C, N_FULL)
    in_maps = make_in_maps(inputs, C, N_FULL, NCORES)
    res = run_bass_kernel_spmd(nc, in_maps, core_ids=list(range(NCORES)))
    LAST_RESULTS = res
    # per-core [C, A, N] -> [N, C, A]; concat cores -> [BW, C, A] -> [B, W, C, A]
    parts = [np.transpose(r["out_logits"], (2, 0, 1)) for r in res.results]
    full = np.concatenate(parts, axis=0).reshape(B, W, C, A)
    return np.ascontiguousarray(full)


# revision 30
# speedup vs baseline: 1.4791x; 1.0051x over previous
"""CharDecoder LSTM kernel for 8 Trainium2 NeuronCores.

Data-parallel over the flattened (B*W)=8192 batch axis: each of the 8 cores
processes 1024 sequences. Small LSTM/projection weights are replicated; the
hidden/cell state stays resident in SBUF (in transposed [feature, batch]
layout) across all 20 decode steps.

Numerics: all matmuls run in float32r (TF32-like: 8-bit exponent, 11-bit
stored mantissa, 1 cycle/row for free dim >= 256) using an exact Dekker
hi/lo split of both operands:
    w = w1 + w2 (exact, host-side), h = h1 + h2 (exact, 2 DVE ops/tile)
    w @ h ~= w1@h1 + w2@h1 + w1@h2     (dropped w2@h2 term ~ 2^-24)
which is fp32-quality (verified ~1e-6 max rel err end-to-end, 0 argmax
flips) at 3 cycles/row instead of fp32's 4 cycles/row on the PE.

Per-core layout ("layout B", feature-on-partitions):
  h1/h2 splits: [512, 1024] as 4+4 SBUF f32r tiles [128, 1024]
  cT: [512, 1024] as 4 fp32 tiles, updated in place
  gatesT: [2048, 1024] computed as 16 (m) x 2 (n) PSUM tiles [128, 512]
  xT (one-hot next-char): [128 vocab, 1024] f32r, 2 half tiles [128, 512]

argmax -> one-hot per step with zero PE work: GPSIMD partition_all_reduce
(max over the 128 vocab partitions, broadcast to all partitions), then a
DVE is_ge against the logits emitting the exact 0/1 one-hot directly in
[vocab, batch] layout as f32r for the next step's x matmul.
"""

import numpy as np
from contextlib import ExitStack

import concourse.bass as bass
import concourse.bass_isa as bass_isa
import concourse.bacc as bacc
import concourse.mybir as mybir
import concourse.tile as tile
from concourse.bass_utils import run_bass_kernel_spmd

B, W, Q, H, A = 64, 128, 256, 512, 128
C = 20
NCORES = 8
BW = B * W
N_FULL = BW // NCORES  # 1024 batch rows per core

F32 = mybir.dt.float32
F32R = mybir.dt.float32r
AF = mybir.ActivationFunctionType
ALU = mybir.AluOpType

KH = H // 128       # 4 hidden k-tiles
KQ = Q // 128       # 2 input k-tiles
MG = 4 * H // 128   # 16 gate m-tiles

LAST_RESULTS = None  # BassKernelResults of the most recent run (for test.py)


def split12(v):
    """Exact Dekker split of fp32 into two 12-bit-significand (f32r) parts."""
    v = np.ascontiguousarray(np.asarray(v, np.float32))
    u = v.view(np.uint32)
    r = u + (np.uint32(0x7FF) + ((u >> np.uint32(12)) & np.uint32(1)))
    hi = (r & np.uint32(0xFFFFF000)).view(np.float32)
    lo = (v - hi).astype(np.float32)
    return hi, lo


def build_program(c_steps: int = C, n: int = N_FULL) -> bass.Bass:
    assert n % 512 == 0
    nt = n // 512    # PSUM n-tiles per row block

    nc = bacc.Bacc("TRN2", target_bir_lowering=False, debug=False)

    qrT1 = nc.dram_tensor("qrT1", [Q, n], F32R, kind="ExternalInput").ap()
    qrT2 = nc.dram_tensor("qrT2", [Q, n], F32R, kind="ExternalInput").ap()
    w0T1 = nc.dram_tensor("w0T1", [128, KQ * 4 * H], F32R, kind="ExternalInput").ap()
    w0T2 = nc.dram_tensor("w0T2", [128, KQ * 4 * H], F32R, kind="ExternalInput").ap()
    whhT1 = nc.dram_tensor("whhT1", [128, KH * 4 * H], F32R, kind="ExternalInput").ap()
    whhT2 = nc.dram_tensor("whhT2", [128, KH * 4 * H], F32R, kind="ExternalInput").ap()
    wihT1 = nc.dram_tensor("wihT1", [A, 4 * H], F32R, kind="ExternalInput").ap()
    wihT2 = nc.dram_tensor("wihT2", [A, 4 * H], F32R, kind="ExternalInput").ap()
    woutT1 = nc.dram_tensor("woutT1", [128, KH * A], F32R, kind="ExternalInput").ap()
    woutT2 = nc.dram_tensor("woutT2", [128, KH * A], F32R, kind="ExternalInput").ap()
    bias_c = nc.dram_tensor("bias_c", [128, MG], F32, kind="ExternalInput").ap()
    bias0_c = nc.dram_tensor("bias0_c", [128, MG], F32, kind="ExternalInput").ap()
    b_out_c = nc.dram_tensor("b_out_c", [128, 1], F32, kind="ExternalInput").ap()
    out = nc.dram_tensor("out_logits", [c_steps, A, n], F32, kind="ExternalOutput").ap()

    with tile.TileContext(nc) as tc, ExitStack() as ctx:
        wp = ctx.enter_context(tc.tile_pool(name="weights", bufs=1))
        sp = ctx.enter_context(tc.tile_pool(name="state", bufs=2))
        cp = ctx.enter_context(tc.tile_pool(name="cell", bufs=1))
        gp = ctx.enter_context(tc.tile_pool(name="gates", bufs=1))
        tp = ctx.enter_context(tc.tile_pool(name="tmp", bufs=1))
        lp = ctx.enter_context(tc.tile_pool(name="logits", bufs=1))
        xp = ctx.enter_context(tc.tile_pool(name="xhot", bufs=1))
        pg = ctx.enter_context(tc.tile_pool(name="pgate", bufs=6, space="PSUM"))
        pl = ctx.enter_context(tc.tile_pool(name="plog", bufs=2, space="PSUM"))

        # --- DMAs ordered by first use: h0 inputs, then hh weights (k-
        #     interleaved hi/lo to match the k-major accumulation order),
        #     then step-0 logits/argmax operands, then step-1 x operands ---
        h1 = [sp.tile([128, n], F32R, tag=f"h1{j}", name=f"h1{j}") for j in range(KH)]
        h2 = [sp.tile([128, n], F32R, tag=f"h2{j}", name=f"h2{j}") for j in range(KH)]
        ct = [cp.tile([128, n], F32, tag=f"c{j}", name=f"c{j}") for j in range(KH)]
        # step-0 gates come straight from W0 = w_hh @ w_in (host-folded, so
        # h0 is never materialized), as f32r hi/lo 3-product like the rest.
        # The gen-1 h tiles are never written (no h0), so qr hi/lo and the
        # W0-lo halves squat in that dead space; W0-hi borrows the w_ih tag
        # buffers (dead until t=1).
        qr1_t = [h1[k] for k in range(KQ)]         # [128, n] f32r each
        qr2_t = [h2[k] for k in range(KQ)]
        w01_t = [wp.tile([128, 4 * H], F32R, tag=f"wih{k + 1}", name=f"w01{k}")
                 for k in range(KQ)]
        w02_halves = [h1[2], h1[3], h2[2], h2[3]]  # [128, n] each, 2 per k
        bias0_t = wp.tile([128, MG], F32, tag="bias0")
        nc.sync.dma_start(bias0_t[:], bias0_c[:, :])
        for k in range(KQ):
            for n0 in range(nt):
                nc.sync.dma_start(qr1_t[k][:, n0 * 512:(n0 + 1) * 512],
                                  qrT1[k * 128:(k + 1) * 128, n0 * 512:(n0 + 1) * 512])
                nc.sync.dma_start(qr2_t[k][:, n0 * 512:(n0 + 1) * 512],
                                  qrT2[k * 128:(k + 1) * 128, n0 * 512:(n0 + 1) * 512])
            nc.sync.dma_start(w01_t[k][:], w0T1[:, k * 4 * H:(k + 1) * 4 * H])
            for hh in range(2):
                nc.sync.dma_start(
                    w02_halves[2 * k + hh][:],
                    w0T2[:, (2 * k + hh) * n:(2 * k + hh + 1) * n])
        whh1_all = wp.tile([128, KH * 4 * H], F32R, tag="whh1")
        nc.sync.dma_start(whh1_all[:], whhT1[:, :])
        whh2_all = wp.tile([128, KH * 4 * H], F32R, tag="whh2")
        nc.sync.dma_start(whh2_all[:], whhT2[:, :])
        whh1_t = [whh1_all[:, k * 4 * H:(k + 1) * 4 * H] for k in range(KH)]
        whh2_t = [whh2_all[:, k * 4 * H:(k + 1) * 4 * H] for k in range(KH)]
        wout1_t = wp.tile([128, KH * A], F32R, tag="wout1")
        nc.sync.dma_start(wout1_t[:], woutT1[:, :])
        wout2_t = wp.tile([128, KH * A], F32R, tag="wout2")
        nc.sync.dma_start(wout2_t[:], woutT2[:, :])
        b_out_t = wp.tile([128, 1], F32, tag="b_out")
        nc.sync.dma_start(b_out_t[:], b_out_c[:, :])
        bias_t = wp.tile([128, MG], F32, tag="bias")
        nc.sync.dma_start(bias_t[:], bias_c[:, :])
        for j in range(KH):
            nc.vector.memset(ct[j][:], 0.0)

        # w_ih reuses the W0 tag buffers; issued after the barrier so its
        # wait-for-W0-reads (t=0 gates) cannot deadlock against the barrier
        wih1_t = wp.tile([128, 4 * H], F32R, tag="wih1")
        nc.sync.dma_start(wih1_t[:], wihT1[:, :])
        wih2_t = wp.tile([128, 4 * H], F32R, tag="wih2")
        nc.sync.dma_start(wih2_t[:], wihT2[:, :])

        xt = None  # one-hot input halves [vocab, batch] f32r; step 0 folds into bias0

        for t in range(c_steps):
            h1n = [sp.tile([128, n], F32R, tag=f"h1{j}", name=f"h1{j}") for j in range(KH)]
            h2n = [sp.tile([128, n], F32R, tag=f"h2{j}", name=f"h2{j}") for j in range(KH)]
            for j in range(KH):
                g_j = [gp.tile([128, n], F32, tag=f"g{q}", name=f"g{q}") for q in range(4)]
                for q in range(4):  # i, f, g, o
                    m = q * KH + j
                    for n0 in range(nt):
                        ps = pg.tile([128, 512], F32, tag="pg")
                        if t == 0:
                            for k in range(KQ):
                                w02 = w02_halves[2 * k + (m // 8)]
                                c0 = (m % 8) * 128
                                nc.tensor.matmul(
                                    ps[:],
                                    lhsT=w01_t[k][:, m * 128:(m + 1) * 128],
                                    rhs=qr1_t[k][:, n0 * 512:(n0 + 1) * 512],
                                    start=(k == 0), stop=False,
                                )
                                nc.tensor.matmul(
                                    ps[:],
                                    lhsT=w02[:, c0:c0 + 128],
                                    rhs=qr1_t[k][:, n0 * 512:(n0 + 1) * 512],
                                    start=False, stop=False,
                                )
                                nc.tensor.matmul(
                                    ps[:],
                                    lhsT=w01_t[k][:, m * 128:(m + 1) * 128],
                                    rhs=qr2_t[k][:, n0 * 512:(n0 + 1) * 512],
                                    start=False, stop=(k == KQ - 1),
                                )
                        else:
                            for k in range(KH):
                                nc.tensor.matmul(
                                    ps[:],
                                    lhsT=whh1_t[k][:, m * 128:(m + 1) * 128],
                                    rhs=h1[k][:, n0 * 512:(n0 + 1) * 512],
                                    start=(k == 0), stop=False,
                                )
                                nc.tensor.matmul(
                                    ps[:],
                                    lhsT=whh2_t[k][:, m * 128:(m + 1) * 128],
                                    rhs=h1[k][:, n0 * 512:(n0 + 1) * 512],
                                    start=False, stop=False,
                                )
                                nc.tensor.matmul(
                                    ps[:],
                                    lhsT=whh1_t[k][:, m * 128:(m + 1) * 128],
                                    rhs=h2[k][:, n0 * 512:(n0 + 1) * 512],
                                    start=False, stop=False,
                                )
                        if t > 0:
                            nc.tensor.matmul(
                                ps[:],
                                lhsT=wih1_t[:, m * 128:(m + 1) * 128],
                                rhs=xt[n0][:],
                                start=False, stop=False,
                            )
                            nc.tensor.matmul(
                                ps[:],
                                lhsT=wih2_t[:, m * 128:(m + 1) * 128],
                                rhs=xt[n0][:],
                                start=False, stop=True,
                            )
                        bias_ap = (bias0_t if t == 0 else bias_t)[:, m:m + 1]
                        func = AF.Tanh if q == 2 else AF.Sigmoid
                        nc.scalar.activation(
                            g_j[q][:, n0 * 512:(n0 + 1) * 512],
                            ps[:], func, bias=bias_ap,
                        )
                # cell/hidden update block j: c = f*c + i*g ; h = o*tanh(c)
                gi, gf, gg, go = (g[:] for g in g_j)
                cs = ct[j][:]
                t1 = tp.tile([128, n], F32, tag="t1")
                nc.vector.tensor_mul(t1[:], gi, gg)
                nc.vector.tensor_mul(cs, gf, cs)
                nc.vector.tensor_add(cs, cs, t1[:])
                t2 = tp.tile([128, n], F32, tag="t2")
                nc.scalar.activation(t2[:], cs, AF.Tanh)
                hf = tp.tile([128, n], F32, tag="t1")
                nc.vector.tensor_mul(hf[:], go, t2[:])
                nc.vector.tensor_copy(h1n[j][:], hf[:])
                nc.vector.tensor_sub(h2n[j][:], hf[:], h1n[j][:].bitcast(F32))

            # logits = w_out @ h + b_out  (in [vocab, batch] layout)
            lg = lp.tile([128, n], F32, tag="logits")
            for n0 in range(nt):
                ps = pl.tile([128, 512], F32, tag="pl")
                for k in range(KH):
                    nc.tensor.matmul(
                        ps[:],
                        lhsT=wout1_t[:, k * A:(k + 1) * A],
                        rhs=h1n[k][:, n0 * 512:(n0 + 1) * 512],
                        start=(k == 0), stop=False,
                    )
                for k in range(KH):
                    nc.tensor.matmul(
                        ps[:],
                        lhsT=wout2_t[:, k * A:(k + 1) * A],
                        rhs=h1n[k][:, n0 * 512:(n0 + 1) * 512],
                        start=False, stop=False,
                    )
                for k in range(KH):
                    nc.tensor.matmul(
                        ps[:],
                        lhsT=wout1_t[:, k * A:(k + 1) * A],
                        rhs=h2n[k][:, n0 * 512:(n0 + 1) * 512],
                        start=False, stop=(k == KH - 1),
                    )
                nc.scalar.activation(
                    lg[:, n0 * 512:(n0 + 1) * 512], ps[:],
                    AF.Identity, bias=b_out_t[:, 0:1],
                )
                nc.sync.dma_start(
                    out[t, :, n0 * 512:(n0 + 1) * 512],
                    lg[:, n0 * 512:(n0 + 1) * 512],
                )

            # next input: one-hot(argmax(logits)) in [vocab, batch] f32r via
            # GPSIMD cross-partition max + DVE is_ge (no PE transposes)
            if t < c_steps - 1:
                xt = [xp.tile([128, 512], F32R, tag=f"x{h}", name=f"x{h}") for h in range(nt)]
                mxb = tp.tile([128, n], F32, tag="t2")
                for n0 in range(nt):
                    nc.gpsimd.partition_all_reduce(
                        mxb[:, n0 * 512:(n0 + 1) * 512],
                        lg[:, n0 * 512:(n0 + 1) * 512],
                        channels=128, reduce_op=bass_isa.ReduceOp.max)
                    nc.vector.tensor_tensor(
                        xt[n0][:],
                        lg[:, n0 * 512:(n0 + 1) * 512],
                        mxb[:, n0 * 512:(n0 + 1) * 512],
                        ALU.is_ge)

            h1, h2 = h1n, h2n

    nc.compile()
    return nc


def make_in_maps(inputs: dict, c_steps: int = C, n: int = N_FULL, ncores: int = NCORES):
    f32 = np.float32
    qr = np.ascontiguousarray(np.asarray(inputs["quantized_repr"], f32)).reshape(BW, Q)
    w_in = np.asarray(inputs["w_in"], f32)
    b_in = np.asarray(inputs["b_in"], f32)
    w_ih = np.asarray(inputs["w_ih"], f32)
    w_hh = np.asarray(inputs["w_hh"], f32)
    b_ih = np.asarray(inputs["b_ih"], f32)
    b_hh = np.asarray(inputs["b_hh"], f32)
    w_out = np.asarray(inputs["w_out"], f32)
    b_out = np.asarray(inputs["b_out"], f32)

    bias = b_ih + b_hh                    # fp32, same as reference
    # step-0 fold: gates0 = w_hh @ (w_in @ qr + b_in) + bias + w_ih[:, 0]
    #            = W0 @ qr + bias0   with W0, bias0 precomputed in fp64
    W0 = (w_hh.astype(np.float64) @ w_in.astype(np.float64)).astype(f32)
    bias0 = (bias.astype(np.float64) + w_ih[:, 0].astype(np.float64)
             + w_hh.astype(np.float64) @ b_in.astype(np.float64)).astype(f32)

    whh1, whh2 = split12(w_hh.T)
    wih1, wih2 = split12(w_ih.T)
    wout1, wout2 = split12(w_out.T)

    def packk(a, ktiles):  # [ktiles*128, cols] -> [128, ktiles*cols]
        cols = a.shape[1]
        out = np.empty((128, ktiles * cols), np.float32)
        for k in range(ktiles):
            out[:, k * cols:(k + 1) * cols] = a[k * 128:(k + 1) * 128, :]
        return np.ascontiguousarray(out)

    w0p1, w0p2 = split12(packk(np.ascontiguousarray(W0.T), KQ))
    shared = {
        "w0T1": w0p1, "w0T2": w0p2,
        "whhT1": packk(whh1, KH), "whhT2": packk(whh2, KH),
        "wihT1": wih1, "wihT2": wih2,
        "woutT1": packk(wout1, KH), "woutT2": packk(wout2, KH),
        "bias_c": np.ascontiguousarray(bias.reshape(MG, 128).T),
        "bias0_c": np.ascontiguousarray(bias0.reshape(MG, 128).T),
        "b_out_c": np.ascontiguousarray(b_out.reshape(128, 1)),
    }
    in_maps = []
    for i in range(ncores):
        m = dict(shared)
        q1, q2 = split12(np.ascontiguousarray(qr[i * n:(i + 1) * n].T))
        m["qrT1"], m["qrT2"] = q1, q2
        in_maps.append(m)
    return in_maps


def kernel(**inputs) -> np.ndarray:
    global LAST_RESULTS
    assert int(inputs["max_char_len"]) == C
    nc = build_program(C, N_FULL)
    in_maps = make_in_maps(inputs, C, N_FULL, NCORES)
    res = run_bass_kernel_spmd(nc, in_maps, core_ids=list(range(NCORES)))
    LAST_RESULTS = res
    # per-core [C, A, N] -> [N, C, A]; concat cores -> [BW, C, A] -> [B, W, C, A]
    parts = [np.transpose(r["out_logits"], (2, 0, 1)) for r in res.results]
    full = np.concatenate(parts, axis=0).reshape(B, W, C, A)
    return np.ascontiguousarray(full)


# revision 31
# speedup vs baseline: 1.4792x; 1.0001x over previous
"""CharDecoder LSTM kernel for 8 Trainium2 NeuronCores.

Data-parallel over the flattened (B*W)=8192 batch axis: each of the 8 cores
processes 1024 sequences. Small LSTM/projection weights are replicated; the
hidden/cell state stays resident in SBUF (in transposed [feature, batch]
layout) across all 20 decode steps.

Numerics: all matmuls run in float32r (TF32-like: 8-bit exponent, 11-bit
stored mantissa, 1 cycle/row for free dim >= 256) using an exact Dekker
hi/lo split of both operands:
    w = w1 + w2 (exact, host-side), h = h1 + h2 (exact, 2 DVE ops/tile)
    w @ h ~= w1@h1 + w2@h1 + w1@h2     (dropped w2@h2 term ~ 2^-24)
which is fp32-quality (verified ~1e-6 max rel err end-to-end, 0 argmax
flips) at 3 cycles/row instead of fp32's 4 cycles/row on the PE.

Per-core layout ("layout B", feature-on-partitions):
  h1/h2 splits: [512, 1024] as 4+4 SBUF f32r tiles [128, 1024]
  cT: [512, 1024] as 4 fp32 tiles, updated in place
  gatesT: [2048, 1024] computed as 16 (m) x 2 (n) PSUM tiles [128, 512]
  xT (one-hot next-char): [128 vocab, 1024] f32r, 2 half tiles [128, 512]

argmax -> one-hot per step with zero PE work: GPSIMD partition_all_reduce
(max over the 128 vocab partitions, broadcast to all partitions), then a
DVE is_ge against the logits emitting the exact 0/1 one-hot directly in
[vocab, batch] layout as f32r for the next step's x matmul.
"""

import numpy as np
from contextlib import ExitStack

import concourse.bass as bass
import concourse.bass_isa as bass_isa
import concourse.bacc as bacc
import concourse.mybir as mybir
import concourse.tile as tile
from concourse.bass_utils import run_bass_kernel_spmd

B, W, Q, H, A = 64, 128, 256, 512, 128
C = 20
NCORES = 8
BW = B * W
N_FULL = BW // NCORES  # 1024 batch rows per core

F32 = mybir.dt.float32
F32R = mybir.dt.float32r
AF = mybir.ActivationFunctionType
ALU = mybir.AluOpType

KH = H // 128       # 4 hidden k-tiles
KQ = Q // 128       # 2 input k-tiles
MG = 4 * H // 128   # 16 gate m-tiles

LAST_RESULTS = None  # BassKernelResults of the most recent run (for test.py)


def split12(v):
    """Exact Dekker split of fp32 into two 12-bit-significand (f32r) parts."""
    v = np.ascontiguousarray(np.asarray(v, np.float32))
    u = v.view(np.uint32)
    r = u + (np.uint32(0x7FF) + ((u >> np.uint32(12)) & np.uint32(1)))
    hi = (r & np.uint32(0xFFFFF000)).view(np.float32)
    lo = (v - hi).astype(np.float32)
    return hi, lo


def build_program(c_steps: int = C, n: int = N_FULL) -> bass.Bass:
    assert n % 512 == 0
    nt = n // 512    # PSUM n-tiles per row block

    nc = bacc.Bacc("TRN2", target_bir_lowering=False, debug=False)

    qrT1 = nc.dram_tensor("qrT1", [Q, n], F32R, kind="ExternalInput").ap()
    qrT2 = nc.dram_tensor("qrT2", [Q, n], F32R, kind="ExternalInput").ap()
    w0T1 = nc.dram_tensor("w0T1", [128, KQ * 4 * H], F32R, kind="ExternalInput").ap()
    w0T2 = nc.dram_tensor("w0T2", [128, KQ * 4 * H], F32R, kind="ExternalInput").ap()
    whhT1 = nc.dram_tensor("whhT1", [128, KH * 4 * H], F32R, kind="ExternalInput").ap()
    whhT2 = nc.dram_tensor("whhT2", [128, KH * 4 * H], F32R, kind="ExternalInput").ap()
    wihT1 = nc.dram_tensor("wihT1", [A, 4 * H], F32R, kind="ExternalInput").ap()
    wihT2 = nc.dram_tensor("wihT2", [A, 4 * H], F32R, kind="ExternalInput").ap()
    woutT1 = nc.dram_tensor("woutT1", [128, KH * A], F32R, kind="ExternalInput").ap()
    woutT2 = nc.dram_tensor("woutT2", [128, KH * A], F32R, kind="ExternalInput").ap()
    bias_c = nc.dram_tensor("bias_c", [128, MG], F32, kind="ExternalInput").ap()
    bias0_c = nc.dram_tensor("bias0_c", [128, MG], F32, kind="ExternalInput").ap()
    b_out_c = nc.dram_tensor("b_out_c", [128, 1], F32, kind="ExternalInput").ap()
    out = nc.dram_tensor("out_logits", [c_steps, A, n], F32, kind="ExternalOutput").ap()

    with tile.TileContext(nc) as tc, ExitStack() as ctx:
        wp = ctx.enter_context(tc.tile_pool(name="weights", bufs=1))
        sp = ctx.enter_context(tc.tile_pool(name="state", bufs=2))
        cp = ctx.enter_context(tc.tile_pool(name="cell", bufs=1))
        gp = ctx.enter_context(tc.tile_pool(name="gates", bufs=1))
        tp = ctx.enter_context(tc.tile_pool(name="tmp", bufs=1))
        lp = ctx.enter_context(tc.tile_pool(name="logits", bufs=1))
        xp = ctx.enter_context(tc.tile_pool(name="xhot", bufs=1))
        pg = ctx.enter_context(tc.tile_pool(name="pgate", bufs=6, space="PSUM"))
        pl = ctx.enter_context(tc.tile_pool(name="plog", bufs=2, space="PSUM"))

        # --- DMAs ordered by first use: h0 inputs, then hh weights (k-
        #     interleaved hi/lo to match the k-major accumulation order),
        #     then step-0 logits/argmax operands, then step-1 x operands ---
        h1 = [sp.tile([128, n], F32R, tag=f"h1{j}", name=f"h1{j}") for j in range(KH)]
        h2 = [sp.tile([128, n], F32R, tag=f"h2{j}", name=f"h2{j}") for j in range(KH)]
        ct = [cp.tile([128, n], F32, tag=f"c{j}", name=f"c{j}") for j in range(KH)]
        # step-0 gates come straight from W0 = w_hh @ w_in (host-folded, so
        # h0 is never materialized), as f32r hi/lo 3-product like the rest.
        # The gen-1 h tiles are never written (no h0), so qr hi/lo and the
        # W0-lo halves squat in that dead space; W0-hi borrows the w_ih tag
        # buffers (dead until t=1).
        qr1_t = [h1[k] for k in range(KQ)]         # [128, n] f32r each
        qr2_t = [h2[k] for k in range(KQ)]
        w01_t = [wp.tile([128, 4 * H], F32R, tag=f"wih{k + 1}", name=f"w01{k}")
                 for k in range(KQ)]
        w02_halves = [h1[2], h1[3], h2[2], h2[3]]  # [128, n] each, 2 per k
        bias0_t = wp.tile([128, MG], F32, tag="bias0")
        nc.sync.dma_start(bias0_t[:], bias0_c[:, :])
        for k in range(KQ):
            # operand order matches the first group's product order so the
            # PE starts as early as possible
            nc.sync.dma_start(w01_t[k][:, 0:1024], w0T1[:, k * 4 * H:k * 4 * H + 1024])
            nc.sync.dma_start(qr1_t[k][:, 0:512], qrT1[k * 128:(k + 1) * 128, 0:512])
            nc.sync.dma_start(w02_halves[2 * k][:], w0T2[:, 2 * k * n:(2 * k + 1) * n])
            nc.sync.dma_start(qr2_t[k][:, 0:512], qrT2[k * 128:(k + 1) * 128, 0:512])
            nc.sync.dma_start(w01_t[k][:, 1024:2048],
                              w0T1[:, k * 4 * H + 1024:k * 4 * H + 2048])
            nc.sync.dma_start(w02_halves[2 * k + 1][:],
                              w0T2[:, (2 * k + 1) * n:(2 * k + 2) * n])
            nc.sync.dma_start(qr1_t[k][:, 512:1024], qrT1[k * 128:(k + 1) * 128, 512:1024])
            nc.sync.dma_start(qr2_t[k][:, 512:1024], qrT2[k * 128:(k + 1) * 128, 512:1024])
        whh1_all = wp.tile([128, KH * 4 * H], F32R, tag="whh1")
        nc.sync.dma_start(whh1_all[:], whhT1[:, :])
        whh2_all = wp.tile([128, KH * 4 * H], F32R, tag="whh2")
        nc.sync.dma_start(whh2_all[:], whhT2[:, :])
        whh1_t = [whh1_all[:, k * 4 * H:(k + 1) * 4 * H] for k in range(KH)]
        whh2_t = [whh2_all[:, k * 4 * H:(k + 1) * 4 * H] for k in range(KH)]
        wout1_t = wp.tile([128, KH * A], F32R, tag="wout1")
        nc.sync.dma_start(wout1_t[:], woutT1[:, :])
        wout2_t = wp.tile([128, KH * A], F32R, tag="wout2")
        nc.sync.dma_start(wout2_t[:], woutT2[:, :])
        b_out_t = wp.tile([128, 1], F32, tag="b_out")
        nc.sync.dma_start(b_out_t[:], b_out_c[:, :])
        bias_t = wp.tile([128, MG], F32, tag="bias")
        nc.sync.dma_start(bias_t[:], bias_c[:, :])
        for j in range(KH):
            nc.vector.memset(ct[j][:], 0.0)

        # w_ih reuses the W0 tag buffers; issued after the barrier so its
        # wait-for-W0-reads (t=0 gates) cannot deadlock against the barrier
        wih1_t = wp.tile([128, 4 * H], F32R, tag="wih1")
        nc.sync.dma_start(wih1_t[:], wihT1[:, :])
        wih2_t = wp.tile([128, 4 * H], F32R, tag="wih2")
        nc.sync.dma_start(wih2_t[:], wihT2[:, :])

        xt = None  # one-hot input halves [vocab, batch] f32r; step 0 folds into bias0

        for t in range(c_steps):
            h1n = [sp.tile([128, n], F32R, tag=f"h1{j}", name=f"h1{j}") for j in range(KH)]
            h2n = [sp.tile([128, n], F32R, tag=f"h2{j}", name=f"h2{j}") for j in range(KH)]
            for j in range(KH):
                g_j = [gp.tile([128, n], F32, tag=f"g{q}", name=f"g{q}") for q in range(4)]
                for q in range(4):  # i, f, g, o
                    m = q * KH + j
                    for n0 in range(nt):
                        ps = pg.tile([128, 512], F32, tag="pg")
                        if t == 0:
                            for k in range(KQ):
                                w02 = w02_halves[2 * k + (m // 8)]
                                c0 = (m % 8) * 128
                                nc.tensor.matmul(
                                    ps[:],
                                    lhsT=w01_t[k][:, m * 128:(m + 1) * 128],
                                    rhs=qr1_t[k][:, n0 * 512:(n0 + 1) * 512],
                                    start=(k == 0), stop=False,
                                )
                                nc.tensor.matmul(
                                    ps[:],
                                    lhsT=w02[:, c0:c0 + 128],
                                    rhs=qr1_t[k][:, n0 * 512:(n0 + 1) * 512],
                                    start=False, stop=False,
                                )
                                nc.tensor.matmul(
                                    ps[:],
                                    lhsT=w01_t[k][:, m * 128:(m + 1) * 128],
                                    rhs=qr2_t[k][:, n0 * 512:(n0 + 1) * 512],
                                    start=False, stop=(k == KQ - 1),
                                )
                        else:
                            for k in range(KH):
                                nc.tensor.matmul(
                                    ps[:],
                                    lhsT=whh1_t[k][:, m * 128:(m + 1) * 128],
                                    rhs=h1[k][:, n0 * 512:(n0 + 1) * 512],
                                    start=(k == 0), stop=False,
                                )
                                nc.tensor.matmul(
                                    ps[:],
                                    lhsT=whh2_t[k][:, m * 128:(m + 1) * 128],
                                    rhs=h1[k][:, n0 * 512:(n0 + 1) * 512],
                                    start=False, stop=False,
                                )
                                nc.tensor.matmul(
                                    ps[:],
                                    lhsT=whh1_t[k][:, m * 128:(m + 1) * 128],
                                    rhs=h2[k][:, n0 * 512:(n0 + 1) * 512],
                                    start=False, stop=False,
                                )
                        if t > 0:
                            nc.tensor.matmul(
                                ps[:],
                                lhsT=wih1_t[:, m * 128:(m + 1) * 128],
                                rhs=xt[n0][:],
                                start=False, stop=False,
                            )
                            nc.tensor.matmul(
                                ps[:],
                                lhsT=wih2_t[:, m * 128:(m + 1) * 128],
                                rhs=xt[n0][:],
                                start=False, stop=True,
                            )
                        bias_ap = (bias0_t if t == 0 else bias_t)[:, m:m + 1]
                        func = AF.Tanh if q == 2 else AF.Sigmoid
                        nc.scalar.activation(
                            g_j[q][:, n0 * 512:(n0 + 1) * 512],
                            ps[:], func, bias=bias_ap,
                        )
                # cell/hidden update block j: c = f*c + i*g ; h = o*tanh(c)
                gi, gf, gg, go = (g[:] for g in g_j)
                cs = ct[j][:]
                t1 = tp.tile([128, n], F32, tag="t1")
                nc.vector.tensor_mul(t1[:], gi, gg)
                nc.vector.tensor_mul(cs, gf, cs)
                nc.vector.tensor_add(cs, cs, t1[:])
                t2 = tp.tile([128, n], F32, tag="t2")
                nc.scalar.activation(t2[:], cs, AF.Tanh)
                hf = tp.tile([128, n], F32, tag="t1")
                nc.vector.tensor_mul(hf[:], go, t2[:])
                nc.vector.tensor_copy(h1n[j][:], hf[:])
                nc.vector.tensor_sub(h2n[j][:], hf[:], h1n[j][:].bitcast(F32))

            # logits = w_out @ h + b_out  (in [vocab, batch] layout)
            lg = lp.tile([128, n], F32, tag="logits")
            for n0 in range(nt):
                ps = pl.tile([128, 512], F32, tag="pl")
                for k in range(KH):
                    nc.tensor.matmul(
                        ps[:],
                        lhsT=wout1_t[:, k * A:(k + 1) * A],
                        rhs=h1n[k][:, n0 * 512:(n0 + 1) * 512],
                        start=(k == 0), stop=False,
                    )
                for k in range(KH):
                    nc.tensor.matmul(
                        ps[:],
                        lhsT=wout2_t[:, k * A:(k + 1) * A],
                        rhs=h1n[k][:, n0 * 512:(n0 + 1) * 512],
                        start=False, stop=False,
                    )
                for k in range(KH):
                    nc.tensor.matmul(
                        ps[:],
                        lhsT=wout1_t[:, k * A:(k + 1) * A],
                        rhs=h2n[k][:, n0 * 512:(n0 + 1) * 512],
                        start=False, stop=(k == KH - 1),
                    )
                nc.scalar.activation(
                    lg[:, n0 * 512:(n0 + 1) * 512], ps[:],
                    AF.Identity, bias=b_out_t[:, 0:1],
                )
                nc.sync.dma_start(
                    out[t, :, n0 * 512:(n0 + 1) * 512],
                    lg[:, n0 * 512:(n0 + 1) * 512],
                )

            # next input: one-hot(argmax(logits)) in [vocab, batch] f32r via
            # GPSIMD cross-partition max + DVE is_ge (no PE transposes)
            if t < c_steps - 1:
                xt = [xp.tile([128, 512], F32R, tag=f"x{h}", name=f"x{h}") for h in range(nt)]
                mxb = tp.tile([128, n], F32, tag="t2")
                for n0 in range(nt):
                    nc.gpsimd.partition_all_reduce(
                        mxb[:, n0 * 512:(n0 + 1) * 512],
                        lg[:, n0 * 512:(n0 + 1) * 512],
                        channels=128, reduce_op=bass_isa.ReduceOp.max)
                    nc.vector.tensor_tensor(
                        xt[n0][:],
                        lg[:, n0 * 512:(n0 + 1) * 512],
                        mxb[:, n0 * 512:(n0 + 1) * 512],
                        ALU.is_ge)

            h1, h2 = h1n, h2n

    nc.compile()
    return nc


def make_in_maps(inputs: dict, c_steps: int = C, n: int = N_FULL, ncores: int = NCORES):
    f32 = np.float32
    qr = np.ascontiguousarray(np.asarray(inputs["quantized_repr"], f32)).reshape(BW, Q)
    w_in = np.asarray(inputs["w_in"], f32)
    b_in = np.asarray(inputs["b_in"], f32)
    w_ih = np.asarray(inputs["w_ih"], f32)
    w_hh = np.asarray(inputs["w_hh"], f32)
    b_ih = np.asarray(inputs["b_ih"], f32)
    b_hh = np.asarray(inputs["b_hh"], f32)
    w_out = np.asarray(inputs["w_out"], f32)
    b_out = np.asarray(inputs["b_out"], f32)

    bias = b_ih + b_hh                    # fp32, same as reference
    # step-0 fold: gates0 = w_hh @ (w_in @ qr + b_in) + bias + w_ih[:, 0]
    #            = W0 @ qr + bias0   with W0, bias0 precomputed in fp64
    W0 = (w_hh.astype(np.float64) @ w_in.astype(np.float64)).astype(f32)
    bias0 = (bias.astype(np.float64) + w_ih[:, 0].astype(np.float64)
             + w_hh.astype(np.float64) @ b_in.astype(np.float64)).astype(f32)

    whh1, whh2 = split12(w_hh.T)
    wih1, wih2 = split12(w_ih.T)
    wout1, wout2 = split12(w_out.T)

    def packk(a, ktiles):  # [ktiles*128, cols] -> [128, ktiles*cols]
        cols = a.shape[1]
        out = np.empty((128, ktiles * cols), np.float32)
        for k in range(ktiles):
            out[:, k * cols:(k + 1) * cols] = a[k * 128:(k + 1) * 128, :]
        return np.ascontiguousarray(out)

    w0p1, w0p2 = split12(packk(np.ascontiguousarray(W0.T), KQ))
    shared = {
        "w0T1": w0p1, "w0T2": w0p2,
        "whhT1": packk(whh1, KH), "whhT2": packk(whh2, KH),
        "wihT1": wih1, "wihT2": wih2,
        "woutT1": packk(wout1, KH), "woutT2": packk(wout2, KH),
        "bias_c": np.ascontiguousarray(bias.reshape(MG, 128).T),
        "bias0_c": np.ascontiguousarray(bias0.reshape(MG, 128).T),
        "b_out_c": np.ascontiguousarray(b_out.reshape(128, 1)),
    }
    in_maps = []
    for i in range(ncores):
        m = dict(shared)
        q1, q2 = split12(np.ascontiguousarray(qr[i * n:(i + 1) * n].T))
        m["qrT1"], m["qrT2"] = q1, q2
        in_maps.append(m)
    return in_maps


def kernel(**inputs) -> np.ndarray:
    global LAST_RESULTS
    assert int(inputs["max_char_len"]) == C
    nc = build_program(C, N_FULL)
    in_maps = make_in_maps(inputs, C, N_FULL, NCORES)
    res = run_bass_kernel_spmd(nc, in_maps, core_ids=list(range(NCORES)))
    LAST_RESULTS = res
    # per-core [C, A, N] -> [N, C, A]; concat cores -> [BW, C, A] -> [B, W, C, A]
    parts = [np.transpose(r["out_logits"], (2, 0, 1)) for r in res.results]
    full = np.concatenate(parts, axis=0).reshape(B, W, C, A)
    return np.ascontiguousarray(full)


# revision 33
# speedup vs baseline: 1.4797x; 1.0004x over previous
"""CharDecoder LSTM kernel for 8 Trainium2 NeuronCores.

Data-parallel over the flattened (B*W)=8192 batch axis: each of the 8 cores
processes 1024 sequences. Small LSTM/projection weights are replicated; the
hidden/cell state stays resident in SBUF (in transposed [feature, batch]
layout) across all 20 decode steps.

Numerics: all matmuls run in float32r (TF32-like: 8-bit exponent, 11-bit
stored mantissa, 1 cycle/row for free dim >= 256) using an exact Dekker
hi/lo split of both operands:
    w = w1 + w2 (exact, host-side), h = h1 + h2 (exact, 2 DVE ops/tile)
    w @ h ~= w1@h1 + w2@h1 + w1@h2     (dropped w2@h2 term ~ 2^-24)
which is fp32-quality (verified ~1e-6 max rel err end-to-end, 0 argmax
flips) at 3 cycles/row instead of fp32's 4 cycles/row on the PE.

Per-core layout ("layout B", feature-on-partitions):
  h1/h2 splits: [512, 1024] as 4+4 SBUF f32r tiles [128, 1024]
  cT: [512, 1024] as 4 fp32 tiles, updated in place
  gatesT: [2048, 1024] computed as 16 (m) x 2 (n) PSUM tiles [128, 512]
  xT (one-hot next-char): [128 vocab, 1024] f32r, 2 half tiles [128, 512]

argmax -> one-hot per step with zero PE work: GPSIMD partition_all_reduce
(max over the 128 vocab partitions, broadcast to all partitions), then a
DVE is_ge against the logits emitting the exact 0/1 one-hot directly in
[vocab, batch] layout as f32r for the next step's x matmul.
"""

import numpy as np
from contextlib import ExitStack

import concourse.bass as bass
import concourse.bass_isa as bass_isa
import concourse.bacc as bacc
import concourse.mybir as mybir
import concourse.tile as tile
from concourse.bass_utils import run_bass_kernel_spmd

B, W, Q, H, A = 64, 128, 256, 512, 128
C = 20
NCORES = 8
BW = B * W
N_FULL = BW // NCORES  # 1024 batch rows per core

F32 = mybir.dt.float32
F32R = mybir.dt.float32r
AF = mybir.ActivationFunctionType
ALU = mybir.AluOpType

KH = H // 128       # 4 hidden k-tiles
KQ = Q // 128       # 2 input k-tiles
MG = 4 * H // 128   # 16 gate m-tiles

LAST_RESULTS = None  # BassKernelResults of the most recent run (for test.py)


def split12(v):
    """Exact Dekker split of fp32 into two 12-bit-significand (f32r) parts."""
    v = np.ascontiguousarray(np.asarray(v, np.float32))
    u = v.view(np.uint32)
    r = u + (np.uint32(0x7FF) + ((u >> np.uint32(12)) & np.uint32(1)))
    hi = (r & np.uint32(0xFFFFF000)).view(np.float32)
    lo = (v - hi).astype(np.float32)
    return hi, lo


def build_program(c_steps: int = C, n: int = N_FULL) -> bass.Bass:
    assert n % 512 == 0
    nt = n // 512    # PSUM n-tiles per row block

    nc = bacc.Bacc("TRN2", target_bir_lowering=False, debug=False)

    qrT1 = nc.dram_tensor("qrT1", [Q, n], F32R, kind="ExternalInput").ap()
    qrT2 = nc.dram_tensor("qrT2", [Q, n], F32R, kind="ExternalInput").ap()
    w0T1 = nc.dram_tensor("w0T1", [128, KQ * 4 * H], F32R, kind="ExternalInput").ap()
    w0T2 = nc.dram_tensor("w0T2", [128, KQ * 4 * H], F32R, kind="ExternalInput").ap()
    whhT1 = nc.dram_tensor("whhT1", [128, KH * 4 * H], F32R, kind="ExternalInput").ap()
    whhT2 = nc.dram_tensor("whhT2", [128, KH * 4 * H], F32R, kind="ExternalInput").ap()
    wihT1 = nc.dram_tensor("wihT1", [A, 4 * H], F32R, kind="ExternalInput").ap()
    wihT2 = nc.dram_tensor("wihT2", [A, 4 * H], F32R, kind="ExternalInput").ap()
    woutT1 = nc.dram_tensor("woutT1", [128, KH * A], F32R, kind="ExternalInput").ap()
    woutT2 = nc.dram_tensor("woutT2", [128, KH * A], F32R, kind="ExternalInput").ap()
    bias_c = nc.dram_tensor("bias_c", [128, MG], F32, kind="ExternalInput").ap()
    bias0_c = nc.dram_tensor("bias0_c", [128, MG], F32, kind="ExternalInput").ap()
    b_out_c = nc.dram_tensor("b_out_c", [128, 1], F32, kind="ExternalInput").ap()
    out = nc.dram_tensor("out_logits", [c_steps, A, n], F32, kind="ExternalOutput").ap()

    with tile.TileContext(nc) as tc, ExitStack() as ctx:
        wp = ctx.enter_context(tc.tile_pool(name="weights", bufs=1))
        sp = ctx.enter_context(tc.tile_pool(name="state", bufs=2))
        cp = ctx.enter_context(tc.tile_pool(name="cell", bufs=1))
        gp = ctx.enter_context(tc.tile_pool(name="gates", bufs=1))
        tp = ctx.enter_context(tc.tile_pool(name="tmp", bufs=1))
        lp = ctx.enter_context(tc.tile_pool(name="logits", bufs=1))
        xp = ctx.enter_context(tc.tile_pool(name="xhot", bufs=1))
        pg = ctx.enter_context(tc.tile_pool(name="pgate", bufs=6, space="PSUM"))
        pl = ctx.enter_context(tc.tile_pool(name="plog", bufs=2, space="PSUM"))

        # --- DMAs ordered by first use: h0 inputs, then hh weights (k-
        #     interleaved hi/lo to match the k-major accumulation order),
        #     then step-0 logits/argmax operands, then step-1 x operands ---
        h1 = [sp.tile([128, n], F32R, tag=f"h1{j}", name=f"h1{j}") for j in range(KH)]
        h2 = [sp.tile([128, n], F32R, tag=f"h2{j}", name=f"h2{j}") for j in range(KH)]
        ct = [cp.tile([128, n], F32, tag=f"c{j}", name=f"c{j}") for j in range(KH)]
        # step-0 gates come straight from W0 = w_hh @ w_in (host-folded, so
        # h0 is never materialized), as f32r hi/lo 3-product like the rest.
        # The gen-1 h tiles are never written (no h0), so qr hi/lo and the
        # W0-lo halves squat in that dead space; W0-hi borrows the w_ih tag
        # buffers (dead until t=1).
        qr1_t = [h1[k] for k in range(KQ)]         # [128, n] f32r each
        qr2_t = [h2[k] for k in range(KQ)]
        w01_t = [wp.tile([128, 4 * H], F32R, tag=f"wih{k + 1}", name=f"w01{k}")
                 for k in range(KQ)]
        w02_halves = [h1[2], h1[3], h2[2], h2[3]]  # [128, n] each, 2 per k
        bias0_t = wp.tile([128, MG], F32, tag="bias0")
        nc.sync.dma_start(bias0_t[:], bias0_c[:, :])
        # warm the PE's HAM clock gate during the startup DMA wait: tiny
        # self-referential matmuls into an unread PSUM bank, so the 3us
        # cold-clock window expires before the real gate matmuls begin
        wu = pg.tile([128, 512], F32, tag="pg")
        for i in range(100):
            nc.tensor.matmul(wu[0:16, 0:16], lhsT=bias0_t[:, 0:16],
                             rhs=bias0_t[:, 0:16],
                             start=(i == 0), stop=(i == 99))
        for k in range(KQ):
            # operand order matches the first group's product order so the
            # PE starts as early as possible
            nc.sync.dma_start(w01_t[k][:, 0:1024], w0T1[:, k * 4 * H:k * 4 * H + 1024])
            nc.sync.dma_start(qr1_t[k][:, 0:512], qrT1[k * 128:(k + 1) * 128, 0:512])
            nc.sync.dma_start(w02_halves[2 * k][:], w0T2[:, 2 * k * n:(2 * k + 1) * n])
            nc.sync.dma_start(qr2_t[k][:, 0:512], qrT2[k * 128:(k + 1) * 128, 0:512])
            nc.sync.dma_start(w01_t[k][:, 1024:2048],
                              w0T1[:, k * 4 * H + 1024:k * 4 * H + 2048])
            nc.sync.dma_start(w02_halves[2 * k + 1][:],
                              w0T2[:, (2 * k + 1) * n:(2 * k + 2) * n])
            nc.sync.dma_start(qr1_t[k][:, 512:1024], qrT1[k * 128:(k + 1) * 128, 512:1024])
            nc.sync.dma_start(qr2_t[k][:, 512:1024], qrT2[k * 128:(k + 1) * 128, 512:1024])
        whh1_all = wp.tile([128, KH * 4 * H], F32R, tag="whh1")
        nc.sync.dma_start(whh1_all[:], whhT1[:, :])
        whh2_all = wp.tile([128, KH * 4 * H], F32R, tag="whh2")
        nc.sync.dma_start(whh2_all[:], whhT2[:, :])
        whh1_t = [whh1_all[:, k * 4 * H:(k + 1) * 4 * H] for k in range(KH)]
        whh2_t = [whh2_all[:, k * 4 * H:(k + 1) * 4 * H] for k in range(KH)]
        wout1_t = wp.tile([128, KH * A], F32R, tag="wout1")
        nc.sync.dma_start(wout1_t[:], woutT1[:, :])
        wout2_t = wp.tile([128, KH * A], F32R, tag="wout2")
        nc.sync.dma_start(wout2_t[:], woutT2[:, :])
        b_out_t = wp.tile([128, 1], F32, tag="b_out")
        nc.sync.dma_start(b_out_t[:], b_out_c[:, :])
        bias_t = wp.tile([128, MG], F32, tag="bias")
        nc.sync.dma_start(bias_t[:], bias_c[:, :])
        for j in range(KH):
            nc.vector.memset(ct[j][:], 0.0)

        # w_ih reuses the W0 tag buffers; issued after the barrier so its
        # wait-for-W0-reads (t=0 gates) cannot deadlock against the barrier
        wih1_t = wp.tile([128, 4 * H], F32R, tag="wih1")
        nc.sync.dma_start(wih1_t[:], wihT1[:, :])
        wih2_t = wp.tile([128, 4 * H], F32R, tag="wih2")
        nc.sync.dma_start(wih2_t[:], wihT2[:, :])

        xt = None  # one-hot input halves [vocab, batch] f32r; step 0 folds into bias0

        for t in range(c_steps):
            h1n = [sp.tile([128, n], F32R, tag=f"h1{j}", name=f"h1{j}") for j in range(KH)]
            h2n = [sp.tile([128, n], F32R, tag=f"h2{j}", name=f"h2{j}") for j in range(KH)]
            for j in range(KH):
                g_j = [gp.tile([128, n], F32, tag=f"g{q}", name=f"g{q}") for q in range(4)]
                for q in range(4):  # i, f, g, o
                    m = q * KH + j
                    for n0 in range(nt):
                        ps = pg.tile([128, 512], F32, tag="pg")
                        if t == 0:
                            for k in range(KQ):
                                w02 = w02_halves[2 * k + (m // 8)]
                                c0 = (m % 8) * 128
                                nc.tensor.matmul(
                                    ps[:],
                                    lhsT=w01_t[k][:, m * 128:(m + 1) * 128],
                                    rhs=qr1_t[k][:, n0 * 512:(n0 + 1) * 512],
                                    start=(k == 0), stop=False,
                                )
                                nc.tensor.matmul(
                                    ps[:],
                                    lhsT=w02[:, c0:c0 + 128],
                                    rhs=qr1_t[k][:, n0 * 512:(n0 + 1) * 512],
                                    start=False, stop=False,
                                )
                                nc.tensor.matmul(
                                    ps[:],
                                    lhsT=w01_t[k][:, m * 128:(m + 1) * 128],
                                    rhs=qr2_t[k][:, n0 * 512:(n0 + 1) * 512],
                                    start=False, stop=(k == KQ - 1),
                                )
                        else:
                            for k in range(KH):
                                nc.tensor.matmul(
                                    ps[:],
                                    lhsT=whh1_t[k][:, m * 128:(m + 1) * 128],
                                    rhs=h1[k][:, n0 * 512:(n0 + 1) * 512],
                                    start=(k == 0), stop=False,
                                )
                                nc.tensor.matmul(
                                    ps[:],
                                    lhsT=whh2_t[k][:, m * 128:(m + 1) * 128],
                                    rhs=h1[k][:, n0 * 512:(n0 + 1) * 512],
                                    start=False, stop=False,
                                )
                                nc.tensor.matmul(
                                    ps[:],
                                    lhsT=whh1_t[k][:, m * 128:(m + 1) * 128],
                                    rhs=h2[k][:, n0 * 512:(n0 + 1) * 512],
                                    start=False, stop=False,
                                )
                        if t > 0:
                            nc.tensor.matmul(
                                ps[:],
                                lhsT=wih1_t[:, m * 128:(m + 1) * 128],
                                rhs=xt[n0][:],
                                start=False, stop=False,
                            )
                            nc.tensor.matmul(
                                ps[:],
                                lhsT=wih2_t[:, m * 128:(m + 1) * 128],
                                rhs=xt[n0][:],
                                start=False, stop=True,
                            )
                        bias_ap = (bias0_t if t == 0 else bias_t)[:, m:m + 1]
                        func = AF.Tanh if q == 2 else AF.Sigmoid
                        nc.scalar.activation(
                            g_j[q][:, n0 * 512:(n0 + 1) * 512],
                            ps[:], func, bias=bias_ap,
                        )
                # cell/hidden update block j: c = f*c + i*g ; h = o*tanh(c)
                gi, gf, gg, go = (g[:] for g in g_j)
                cs = ct[j][:]
                t1 = tp.tile([128, n], F32, tag="t1")
                nc.vector.tensor_mul(t1[:], gi, gg)
                nc.vector.tensor_mul(cs, gf, cs)
                nc.vector.tensor_add(cs, cs, t1[:])
                t2 = tp.tile([128, n], F32, tag="t2")
                nc.scalar.activation(t2[:], cs, AF.Tanh)
                hf = tp.tile([128, n], F32, tag="t1")
                nc.vector.tensor_mul(hf[:], go, t2[:])
                nc.vector.tensor_copy(h1n[j][:], hf[:])
                nc.vector.tensor_sub(h2n[j][:], hf[:], h1n[j][:].bitcast(F32))

            # logits = w_out @ h + b_out  (in [vocab, batch] layout)
            lg = lp.tile([128, n], F32, tag="logits")
            for n0 in range(nt):
                ps = pl.tile([128, 512], F32, tag="pl")
                for k in range(KH):
                    nc.tensor.matmul(
                        ps[:],
                        lhsT=wout1_t[:, k * A:(k + 1) * A],
                        rhs=h1n[k][:, n0 * 512:(n0 + 1) * 512],
                        start=(k == 0), stop=False,
                    )
                for k in range(KH):
                    nc.tensor.matmul(
                        ps[:],
                        lhsT=wout2_t[:, k * A:(k + 1) * A],
                        rhs=h1n[k][:, n0 * 512:(n0 + 1) * 512],
                        start=False, stop=False,
                    )
                for k in range(KH):
                    nc.tensor.matmul(
                        ps[:],
                        lhsT=wout1_t[:, k * A:(k + 1) * A],
                        rhs=h2n[k][:, n0 * 512:(n0 + 1) * 512],
                        start=False, stop=(k == KH - 1),
                    )
                nc.scalar.activation(
                    lg[:, n0 * 512:(n0 + 1) * 512], ps[:],
                    AF.Identity, bias=b_out_t[:, 0:1],
                )
                nc.sync.dma_start(
                    out[t, :, n0 * 512:(n0 + 1) * 512],
                    lg[:, n0 * 512:(n0 + 1) * 512],
                )

            # next input: one-hot(argmax(logits)) in [vocab, batch] f32r via
            # GPSIMD cross-partition max + DVE is_ge (no PE transposes)
            if t < c_steps - 1:
                xt = [xp.tile([128, 512], F32R, tag=f"x{h}", name=f"x{h}") for h in range(nt)]
                mxb = tp.tile([128, n], F32, tag="t2")
                for n0 in range(nt):
                    nc.gpsimd.partition_all_reduce(
                        mxb[:, n0 * 512:(n0 + 1) * 512],
                        lg[:, n0 * 512:(n0 + 1) * 512],
                        channels=128, reduce_op=bass_isa.ReduceOp.max)
                    nc.vector.tensor_tensor(
                        xt[n0][:],
                        lg[:, n0 * 512:(n0 + 1) * 512],
                        mxb[:, n0 * 512:(n0 + 1) * 512],
                        ALU.is_ge)

            h1, h2 = h1n, h2n

    nc.compile()
    return nc


def make_in_maps(inputs: dict, c_steps: int = C, n: int = N_FULL, ncores: int = NCORES):
    f32 = np.float32
    qr = np.ascontiguousarray(np.asarray(inputs["quantized_repr"], f32)).reshape(BW, Q)
    w_in = np.asarray(inputs["w_in"], f32)
    b_in = np.asarray(inputs["b_in"], f32)
    w_ih = np.asarray(inputs["w_ih"], f32)
    w_hh = np.asarray(inputs["w_hh"], f32)
    b_ih = np.asarray(inputs["b_ih"], f32)
    b_hh = np.asarray(inputs["b_hh"], f32)
    w_out = np.asarray(inputs["w_out"], f32)
    b_out = np.asarray(inputs["b_out"], f32)

    bias = b_ih + b_hh                    # fp32, same as reference
    # step-0 fold: gates0 = w_hh @ (w_in @ qr + b_in) + bias + w_ih[:, 0]
    #            = W0 @ qr + bias0   with W0, bias0 precomputed in fp64
    W0 = (w_hh.astype(np.float64) @ w_in.astype(np.float64)).astype(f32)
    bias0 = (bias.astype(np.float64) + w_ih[:, 0].astype(np.float64)
             + w_hh.astype(np.float64) @ b_in.astype(np.float64)).astype(f32)

    whh1, whh2 = split12(w_hh.T)
    wih1, wih2 = split12(w_ih.T)
    wout1, wout2 = split12(w_out.T)

    def packk(a, ktiles):  # [ktiles*128, cols] -> [128, ktiles*cols]
        cols = a.shape[1]
        out = np.empty((128, ktiles * cols), np.float32)
        for k in range(ktiles):
            out[:, k * cols:(k + 1) * cols] = a[k * 128:(k + 1) * 128, :]
        return np.ascontiguousarray(out)

    w0p1, w0p2 = split12(packk(np.ascontiguousarray(W0.T), KQ))
    shared = {
        "w0T1": w0p1, "w0T2": w0p2,
        "whhT1": packk(whh1, KH), "whhT2": packk(whh2, KH),
        "wihT1": wih1, "wihT2": wih2,
        "woutT1": packk(wout1, KH), "woutT2": packk(wout2, KH),
        "bias_c": np.ascontiguousarray(bias.reshape(MG, 128).T),
        "bias0_c": np.ascontiguousarray(bias0.reshape(MG, 128).T),
        "b_out_c": np.ascontiguousarray(b_out.reshape(128, 1)),
    }
    in_maps = []
    for i in range(ncores):
        m = dict(shared)
        q1, q2 = split12(np.ascontiguousarray(qr[i * n:(i + 1) * n].T))
        m["qrT1"], m["qrT2"] = q1, q2
        in_maps.append(m)
    return in_maps


def kernel(**inputs) -> np.ndarray:
    global LAST_RESULTS
    assert int(inputs["max_char_len"]) == C
    nc = build_program(C, N_FULL)
    in_maps = make_in_maps(inputs, C, N_FULL, NCORES)
    res = run_bass_kernel_spmd(nc, in_maps, core_ids=list(range(NCORES)))
    LAST_RESULTS = res
    # per-core [C, A, N] -> [N, C, A]; concat cores -> [BW, C, A] -> [B, W, C, A]
    parts = [np.transpose(r["out_logits"], (2, 0, 1)) for r in res.results]
    full = np.concatenate(parts, axis=0).reshape(B, W, C, A)
    return np.ascontiguousarray(full)


# revision 36
# speedup vs baseline: 1.4810x; 1.0009x over previous
"""CharDecoder LSTM kernel for 8 Trainium2 NeuronCores.

Data-parallel over the flattened (B*W)=8192 batch axis: each of the 8 cores
processes 1024 sequences. Small LSTM/projection weights are replicated; the
hidden/cell state stays resident in SBUF (in transposed [feature, batch]
layout) across all 20 decode steps.

Numerics: all matmuls run in float32r (TF32-like: 8-bit exponent, 11-bit
stored mantissa, 1 cycle/row for free dim >= 256) using an exact Dekker
hi/lo split of both operands:
    w = w1 + w2 (exact, host-side), h = h1 + h2 (exact, 2 DVE ops/tile)
    w @ h ~= w1@h1 + w2@h1 + w1@h2     (dropped w2@h2 term ~ 2^-24)
which is fp32-quality (verified ~1e-6 max rel err end-to-end, 0 argmax
flips) at 3 cycles/row instead of fp32's 4 cycles/row on the PE.

Per-core layout ("layout B", feature-on-partitions):
  h1/h2 splits: [512, 1024] as 4+4 SBUF f32r tiles [128, 1024]
  cT: [512, 1024] as 4 fp32 tiles, updated in place
  gatesT: [2048, 1024] computed as 16 (m) x 2 (n) PSUM tiles [128, 512]
  xT (one-hot next-char): [128 vocab, 1024] f32r, 2 half tiles [128, 512]

argmax -> one-hot per step with zero PE work: GPSIMD partition_all_reduce
(max over the 128 vocab partitions, broadcast to all partitions), then a
DVE is_ge against the logits emitting the exact 0/1 one-hot directly in
[vocab, batch] layout as f32r for the next step's x matmul.
"""

import numpy as np
from contextlib import ExitStack

import concourse.bass as bass
import concourse.bass_isa as bass_isa
import concourse.bacc as bacc
import concourse.mybir as mybir
import concourse.tile as tile
from concourse.bass_utils import run_bass_kernel_spmd

B, W, Q, H, A = 64, 128, 256, 512, 128
C = 20
NCORES = 8
BW = B * W
N_FULL = BW // NCORES  # 1024 batch rows per core

F32 = mybir.dt.float32
F32R = mybir.dt.float32r
AF = mybir.ActivationFunctionType
ALU = mybir.AluOpType

KH = H // 128       # 4 hidden k-tiles
KQ = Q // 128       # 2 input k-tiles
MG = 4 * H // 128   # 16 gate m-tiles

LAST_RESULTS = None  # BassKernelResults of the most recent run (for test.py)


def split12(v):
    """Exact Dekker split of fp32 into two 12-bit-significand (f32r) parts."""
    v = np.ascontiguousarray(np.asarray(v, np.float32))
    u = v.view(np.uint32)
    r = u + (np.uint32(0x7FF) + ((u >> np.uint32(12)) & np.uint32(1)))
    hi = (r & np.uint32(0xFFFFF000)).view(np.float32)
    lo = (v - hi).astype(np.float32)
    return hi, lo


def build_program(c_steps: int = C, n: int = N_FULL) -> bass.Bass:
    assert n % 512 == 0
    nt = n // 512    # PSUM n-tiles per row block

    nc = bacc.Bacc("TRN2", target_bir_lowering=False, debug=False)

    qrT1 = nc.dram_tensor("qrT1", [Q, n], F32R, kind="ExternalInput").ap()
    qrT2 = nc.dram_tensor("qrT2", [Q, n], F32R, kind="ExternalInput").ap()
    w0T1 = nc.dram_tensor("w0T1", [128, KQ * 4 * H], F32R, kind="ExternalInput").ap()
    w0T2 = nc.dram_tensor("w0T2", [128, KQ * 4 * H], F32R, kind="ExternalInput").ap()
    whhT1 = nc.dram_tensor("whhT1", [128, KH * 4 * H], F32R, kind="ExternalInput").ap()
    whhT2 = nc.dram_tensor("whhT2", [128, KH * 4 * H], F32R, kind="ExternalInput").ap()
    wihT1 = nc.dram_tensor("wihT1", [A, 4 * H], F32R, kind="ExternalInput").ap()
    wihT2 = nc.dram_tensor("wihT2", [A, 4 * H], F32R, kind="ExternalInput").ap()
    woutT1 = nc.dram_tensor("woutT1", [128, KH * A], F32R, kind="ExternalInput").ap()
    woutT2 = nc.dram_tensor("woutT2", [128, KH * A], F32R, kind="ExternalInput").ap()
    bias_c = nc.dram_tensor("bias_c", [128, MG], F32, kind="ExternalInput").ap()
    bias0_c = nc.dram_tensor("bias0_c", [128, MG], F32, kind="ExternalInput").ap()
    b_out_c = nc.dram_tensor("b_out_c", [128, 1], F32, kind="ExternalInput").ap()
    out = nc.dram_tensor("out_logits", [c_steps, A, n], F32, kind="ExternalOutput").ap()

    with tile.TileContext(nc) as tc, ExitStack() as ctx:
        wp = ctx.enter_context(tc.tile_pool(name="weights", bufs=1))
        sp = ctx.enter_context(tc.tile_pool(name="state", bufs=2))
        cp = ctx.enter_context(tc.tile_pool(name="cell", bufs=1))
        gp = ctx.enter_context(tc.tile_pool(name="gates", bufs=1))
        tp = ctx.enter_context(tc.tile_pool(name="tmp", bufs=1))
        lp = ctx.enter_context(tc.tile_pool(name="logits", bufs=1))
        xp = ctx.enter_context(tc.tile_pool(name="xhot", bufs=1))
        pg = ctx.enter_context(tc.tile_pool(name="pgate", bufs=6, space="PSUM"))
        pl = ctx.enter_context(tc.tile_pool(name="plog", bufs=2, space="PSUM"))

        # --- DMAs ordered by first use: h0 inputs, then hh weights (k-
        #     interleaved hi/lo to match the k-major accumulation order),
        #     then step-0 logits/argmax operands, then step-1 x operands ---
        h1 = [sp.tile([128, n], F32R, tag=f"h1{j}", name=f"h1{j}") for j in range(KH)]
        h2 = [sp.tile([128, n], F32R, tag=f"h2{j}", name=f"h2{j}") for j in range(KH)]
        ct = [cp.tile([128, n], F32, tag=f"c{j}", name=f"c{j}") for j in range(KH)]
        # step-0 gates come straight from W0 = w_hh @ w_in (host-folded, so
        # h0 is never materialized), as f32r hi/lo 3-product like the rest.
        # The gen-1 h tiles are never written (no h0), so qr hi/lo and the
        # W0-lo halves squat in that dead space; W0-hi borrows the w_ih tag
        # buffers (dead until t=1).
        qr1_t = [h1[k] for k in range(KQ)]         # [128, n] f32r each
        qr2_t = [h2[k] for k in range(KQ)]
        w01_t = [wp.tile([128, 4 * H], F32R, tag=f"wih{k + 1}", name=f"w01{k}")
                 for k in range(KQ)]
        w02_halves = [h1[2], h1[3], h2[2], h2[3]]  # [128, n] each, 2 per k
        bias0_t = wp.tile([128, MG], F32, tag="bias0")
        nc.sync.dma_start(bias0_t[:], bias0_c[:, :])
        # warm the PE's HAM clock gate during the startup DMA wait: tiny
        # self-referential matmuls into an unread PSUM bank, so the 3us
        # cold-clock window expires before the real gate matmuls begin
        wu = pg.tile([128, 512], F32, tag="pg")
        for i in range(280):
            nc.tensor.matmul(wu[0:16, 0:16], lhsT=bias0_t[:, 0:16],
                             rhs=bias0_t[:, 0:16],
                             start=(i == 0), stop=(i == 279))
        for k in range(KQ):
            # operand order matches the first group's product order so the
            # PE starts as early as possible
            nc.sync.dma_start(w01_t[k][:, 0:1024], w0T1[:, k * 4 * H:k * 4 * H + 1024])
            nc.sync.dma_start(qr1_t[k][:, 0:512], qrT1[k * 128:(k + 1) * 128, 0:512])
            nc.sync.dma_start(w02_halves[2 * k][:], w0T2[:, 2 * k * n:(2 * k + 1) * n])
            nc.sync.dma_start(qr2_t[k][:, 0:512], qrT2[k * 128:(k + 1) * 128, 0:512])
            nc.sync.dma_start(w01_t[k][:, 1024:2048],
                              w0T1[:, k * 4 * H + 1024:k * 4 * H + 2048])
            nc.sync.dma_start(w02_halves[2 * k + 1][:],
                              w0T2[:, (2 * k + 1) * n:(2 * k + 2) * n])
            nc.sync.dma_start(qr1_t[k][:, 512:1024], qrT1[k * 128:(k + 1) * 128, 512:1024])
            nc.sync.dma_start(qr2_t[k][:, 512:1024], qrT2[k * 128:(k + 1) * 128, 512:1024])
        whh1_all = wp.tile([128, KH * 4 * H], F32R, tag="whh1")
        nc.sync.dma_start(whh1_all[:], whhT1[:, :])
        whh2_all = wp.tile([128, KH * 4 * H], F32R, tag="whh2")
        nc.sync.dma_start(whh2_all[:], whhT2[:, :])
        whh1_t = [whh1_all[:, k * 4 * H:(k + 1) * 4 * H] for k in range(KH)]
        whh2_t = [whh2_all[:, k * 4 * H:(k + 1) * 4 * H] for k in range(KH)]
        wout1_t = wp.tile([128, KH * A], F32R, tag="wout1")
        nc.sync.dma_start(wout1_t[:], woutT1[:, :])
        wout2_t = wp.tile([128, KH * A], F32R, tag="wout2")
        nc.sync.dma_start(wout2_t[:], woutT2[:, :])
        b_out_t = wp.tile([128, 1], F32, tag="b_out")
        nc.sync.dma_start(b_out_t[:], b_out_c[:, :])
        bias_t = wp.tile([128, MG], F32, tag="bias")
        nc.sync.dma_start(bias_t[:], bias_c[:, :])
        for j in range(KH):
            nc.vector.memset(ct[j][:], 0.0)

        # w_ih reuses the W0 tag buffers; issued after the barrier so its
        # wait-for-W0-reads (t=0 gates) cannot deadlock against the barrier
        wih1_t = wp.tile([128, 4 * H], F32R, tag="wih1")
        nc.sync.dma_start(wih1_t[:], wihT1[:, :])
        wih2_t = wp.tile([128, 4 * H], F32R, tag="wih2")
        nc.sync.dma_start(wih2_t[:], wihT2[:, :])

        xt = None  # one-hot input halves [vocab, batch] f32r; step 0 folds into bias0

        for t in range(c_steps):
            h1n = [sp.tile([128, n], F32R, tag=f"h1{j}", name=f"h1{j}") for j in range(KH)]
            h2n = [sp.tile([128, n], F32R, tag=f"h2{j}", name=f"h2{j}") for j in range(KH)]
            for j in range(KH):
                g_j = [gp.tile([128, n], F32, tag=f"g{q}", name=f"g{q}") for q in range(4)]
                for q in range(4):  # i, f, g, o
                    m = q * KH + j
                    for n0 in range(nt):
                        ps = pg.tile([128, 512], F32, tag="pg")
                        if t == 0:
                            for k in range(KQ):
                                w02 = w02_halves[2 * k + (m // 8)]
                                c0 = (m % 8) * 128
                                nc.tensor.matmul(
                                    ps[:],
                                    lhsT=w01_t[k][:, m * 128:(m + 1) * 128],
                                    rhs=qr1_t[k][:, n0 * 512:(n0 + 1) * 512],
                                    start=(k == 0), stop=False,
                                )
                                nc.tensor.matmul(
                                    ps[:],
                                    lhsT=w02[:, c0:c0 + 128],
                                    rhs=qr1_t[k][:, n0 * 512:(n0 + 1) * 512],
                                    start=False, stop=False,
                                )
                                nc.tensor.matmul(
                                    ps[:],
                                    lhsT=w01_t[k][:, m * 128:(m + 1) * 128],
                                    rhs=qr2_t[k][:, n0 * 512:(n0 + 1) * 512],
                                    start=False, stop=(k == KQ - 1),
                                )
                        else:
                            for k in range(KH):
                                nc.tensor.matmul(
                                    ps[:],
                                    lhsT=whh1_t[k][:, m * 128:(m + 1) * 128],
                                    rhs=h1[k][:, n0 * 512:(n0 + 1) * 512],
                                    start=(k == 0), stop=False,
                                )
                                nc.tensor.matmul(
                                    ps[:],
                                    lhsT=whh2_t[k][:, m * 128:(m + 1) * 128],
                                    rhs=h1[k][:, n0 * 512:(n0 + 1) * 512],
                                    start=False, stop=False,
                                )
                                nc.tensor.matmul(
                                    ps[:],
                                    lhsT=whh1_t[k][:, m * 128:(m + 1) * 128],
                                    rhs=h2[k][:, n0 * 512:(n0 + 1) * 512],
                                    start=False, stop=False,
                                )
                        if t > 0:
                            nc.tensor.matmul(
                                ps[:],
                                lhsT=wih1_t[:, m * 128:(m + 1) * 128],
                                rhs=xt[n0][:],
                                start=False, stop=False,
                            )
                            nc.tensor.matmul(
                                ps[:],
                                lhsT=wih2_t[:, m * 128:(m + 1) * 128],
                                rhs=xt[n0][:],
                                start=False, stop=True,
                            )
                        bias_ap = (bias0_t if t == 0 else bias_t)[:, m:m + 1]
                        func = AF.Tanh if q == 2 else AF.Sigmoid
                        nc.scalar.activation(
                            g_j[q][:, n0 * 512:(n0 + 1) * 512],
                            ps[:], func, bias=bias_ap,
                        )
                # cell/hidden update block j: c = f*c + i*g ; h = o*tanh(c)
                gi, gf, gg, go = (g[:] for g in g_j)
                cs = ct[j][:]
                t1 = tp.tile([128, n], F32, tag="t1")
                nc.vector.tensor_mul(t1[:], gi, gg)
                nc.vector.tensor_mul(cs, gf, cs)
                nc.vector.tensor_add(cs, cs, t1[:])
                t2 = tp.tile([128, n], F32, tag="t2")
                nc.scalar.activation(t2[:], cs, AF.Tanh)
                hf = tp.tile([128, n], F32, tag="t1")
                nc.vector.tensor_mul(hf[:], go, t2[:])
                nc.vector.tensor_copy(h1n[j][:], hf[:])
                nc.vector.tensor_sub(h2n[j][:], hf[:], h1n[j][:].bitcast(F32))

            # logits = w_out @ h + b_out  (in [vocab, batch] layout)
            lg = lp.tile([128, n], F32, tag="logits")
            for n0 in range(nt):
                ps = pl.tile([128, 512], F32, tag="pl")
                for k in range(KH):
                    nc.tensor.matmul(
                        ps[:],
                        lhsT=wout1_t[:, k * A:(k + 1) * A],
                        rhs=h1n[k][:, n0 * 512:(n0 + 1) * 512],
                        start=(k == 0), stop=False,
                    )
                for k in range(KH):
                    nc.tensor.matmul(
                        ps[:],
                        lhsT=wout2_t[:, k * A:(k + 1) * A],
                        rhs=h1n[k][:, n0 * 512:(n0 + 1) * 512],
                        start=False, stop=False,
                    )
                for k in range(KH):
                    nc.tensor.matmul(
                        ps[:],
                        lhsT=wout1_t[:, k * A:(k + 1) * A],
                        rhs=h2n[k][:, n0 * 512:(n0 + 1) * 512],
                        start=False, stop=(k == KH - 1),
                    )
                nc.scalar.activation(
                    lg[:, n0 * 512:(n0 + 1) * 512], ps[:],
                    AF.Identity, bias=b_out_t[:, 0:1],
                )
                nc.sync.dma_start(
                    out[t, :, n0 * 512:(n0 + 1) * 512],
                    lg[:, n0 * 512:(n0 + 1) * 512],
                )

            # next input: one-hot(argmax(logits)) in [vocab, batch] f32r via
            # GPSIMD cross-partition max + DVE is_ge (no PE transposes)
            if t < c_steps - 1:
                xt = [xp.tile([128, 512], F32R, tag=f"x{h}", name=f"x{h}") for h in range(nt)]
                mxb = tp.tile([128, n], F32, tag="t2")
                for n0 in range(nt):
                    nc.gpsimd.partition_all_reduce(
                        mxb[:, n0 * 512:(n0 + 1) * 512],
                        lg[:, n0 * 512:(n0 + 1) * 512],
                        channels=128, reduce_op=bass_isa.ReduceOp.max)
                    nc.vector.tensor_tensor(
                        xt[n0][:],
                        lg[:, n0 * 512:(n0 + 1) * 512],
                        mxb[:, n0 * 512:(n0 + 1) * 512],
                        ALU.is_ge)

            h1, h2 = h1n, h2n

    nc.compile()
    return nc


def make_in_maps(inputs: dict, c_steps: int = C, n: int = N_FULL, ncores: int = NCORES):
    f32 = np.float32
    qr = np.ascontiguousarray(np.asarray(inputs["quantized_repr"], f32)).reshape(BW, Q)
    w_in = np.asarray(inputs["w_in"], f32)
    b_in = np.asarray(inputs["b_in"], f32)
    w_ih = np.asarray(inputs["w_ih"], f32)
    w_hh = np.asarray(inputs["w_hh"], f32)
    b_ih = np.asarray(inputs["b_ih"], f32)
    b_hh = np.asarray(inputs["b_hh"], f32)
    w_out = np.asarray(inputs["w_out"], f32)
    b_out = np.asarray(inputs["b_out"], f32)

    bias = b_ih + b_hh                    # fp32, same as reference
    # step-0 fold: gates0 = w_hh @ (w_in @ qr + b_in) + bias + w_ih[:, 0]
    #            = W0 @ qr + bias0   with W0, bias0 precomputed in fp64
    W0 = (w_hh.astype(np.float64) @ w_in.astype(np.float64)).astype(f32)
    bias0 = (bias.astype(np.float64) + w_ih[:, 0].astype(np.float64)
             + w_hh.astype(np.float64) @ b_in.astype(np.float64)).astype(f32)

    whh1, whh2 = split12(w_hh.T)
    wih1, wih2 = split12(w_ih.T)
    wout1, wout2 = split12(w_out.T)

    def packk(a, ktiles):  # [ktiles*128, cols] -> [128, ktiles*cols]
        cols = a.shape[1]
        out = np.empty((128, ktiles * cols), np.float32)
        for k in range(ktiles):
            out[:, k * cols:(k + 1) * cols] = a[k * 128:(k + 1) * 128, :]
        return np.ascontiguousarray(out)

    w0p1, w0p2 = split12(packk(np.ascontiguousarray(W0.T), KQ))
    shared = {
        "w0T1": w0p1, "w0T2": w0p2,
        "whhT1": packk(whh1, KH), "whhT2": packk(whh2, KH),
        "wihT1": wih1, "wihT2": wih2,
        "woutT1": packk(wout1, KH), "woutT2": packk(wout2, KH),
        "bias_c": np.ascontiguousarray(bias.reshape(MG, 128).T),
        "bias0_c": np.ascontiguousarray(bias0.reshape(MG, 128).T),
        "b_out_c": np.ascontiguousarray(b_out.reshape(128, 1)),
    }
    in_maps = []
    for i in range(ncores):
        m = dict(shared)
        q1, q2 = split12(np.ascontiguousarray(qr[i * n:(i + 1) * n].T))
        m["qrT1"], m["qrT2"] = q1, q2
        in_maps.append(m)
    return in_maps


def kernel(**inputs) -> np.ndarray:
    global LAST_RESULTS
    assert int(inputs["max_char_len"]) == C
    nc = build_program(C, N_FULL)
    in_maps = make_in_maps(inputs, C, N_FULL, NCORES)
    res = run_bass_kernel_spmd(nc, in_maps, core_ids=list(range(NCORES)))
    LAST_RESULTS = res
    # per-core [C, A, N] -> [N, C, A]; concat cores -> [BW, C, A] -> [B, W, C, A]
    parts = [np.transpose(r["out_logits"], (2, 0, 1)) for r in res.results]
    full = np.concatenate(parts, axis=0).reshape(B, W, C, A)
    return np.ascontiguousarray(full)
